# revision 54
# baseline (speedup 1.0000x reference)
"""Trainium2 Bass kernel for nn_GaussianDerivativeESPLayer.

Strategy (per the data-parallel hint, extended since B=4 < 8 cores):
shard (batch b, H-half) across the 8 cores. Each core computes output
rows [H0, H0+93) of one batch element from input rows [g0, g0+105).

Performance design (512us -> 305us in the TimelineSim cost model):
  - fp16 conv matmuls (1 cyc/row on PE vs 4 for fp32); integration
    matmuls fp16 as well.
  - j0 diagonal Gram entry fused: Act-engine Square reads conv PSUM and
    writes the squared map to SBUF fp16 in one instruction (copy+mul).
  - Cross products / squares / channel-fold adds run in fp16 (DVE 2x
    perf mode). Each product+fold chain is pinned to one engine
    (DVE or Pool, greedy by projected load) so the chain rides implicit
    same-engine ordering instead of cross-engine semaphores. PSUM->SBUF
    copies are balanced between Act and DVE (Pool cannot access PSUM;
    TensorTensor cannot take two PSUM sources; DMA cannot read PSUM).
  - hf-merged tiles [96, 2, c, 99] and 2-PSUM-bank batched copies halve
    instruction counts (Act pays ~185ns access-init per instruction).
  - Software pipelining: the next channel block's yconv matmuls+copies
    are interleaved into the current block's xconv/product emission so
    parked matmuls never head-of-line block the PE sequencer.
  - ESP stage in fp32 (Newton-identity cancellations), with Ln/Exp
    calls batched by function to avoid Act table reloads; the
    sqrt-binomial pair weights fold into the integration scale copy.

Per-core pipeline (all layouts [partition, free]):
  1. yconv: data-stationary matmuls X^T @ yband -> Z [w, (s, fy, c, h')]
  2. xconv: band-stationary matmuls xband^T @ Z -> L [w', (c, h')] PSUM
  3. Gram: Square/mult + fold-tree over 64 channels -> 20 S maps [96, 99]
  4. Integration: valid 7x7 separable conv as two matmul stages
  5. ESP: Newton-identity polynomials per pixel, roots via exp/ln.
"""

import math

import numpy as np

B, H, W, C = 4, 192, 192, 64
NH = 99  # h' rows computed per core (pre y-integration)
NOUT = 93  # h'' output rows per core
HL = 105  # input rows per core
NWO = 186  # output cols
CBLK = 8  # channels per block
NBLK = C // CBLK
CSUB = 4  # channels per xconv matmul chunk
RI = 3  # integrator radius
EPS = float(np.finfo(np.float64).eps)

CONV_MODE = "fp16"

_CACHE = {}
RUN_KWARGS = {}  # test harness can set dict(trace=True) before calling kernel()
LAST = None  # BassKernelResults of the most recent kernel() call


# ---------------------------------------------------------------- host math
def _extract_filters(kernels):
    """kernels [6, kh, kw, C, 1] channel-tiled separable. Returns
    (gys, gxs): 1D filters with sigma^order folded in, such that
    kernel(j,k) == outer(gys[j-k], gxs[k])."""
    K = np.asarray(kernels, dtype=np.float64)[:, :, :, 0, 0]
    i0 = K.shape[1] // 2
    s0 = math.sqrt(abs(K[0][i0, i0]))
    g0y = K[0][:, i0] / s0
    g0x = K[0][i0, :] / s0
    g1x = K[2][i0, :] / g0y[i0]  # (j=1,k=1) = s*g1x (x) * g0y (y)
    g1y = K[1][:, i0] / g0x[i0]  # (j=1,k=0) = s*g0x (x) * g1y (y)
    g2x = K[5][i0, :] / g0y[i0]
    g2y = K[3][:, i0] / g0x[i0]
    return [g0y, g1y, g2y], [g0x, g1x, g2x]


def _extract_integrator(dg_int):
    K = np.asarray(dg_int, dtype=np.float64)[:, :, 0, 0]
    i0 = K.shape[0] // 2
    s0 = math.sqrt(abs(K[i0, i0]))
    return K[:, i0] / s0, K[i0, :] / s0  # giy, gix


def _band(k_count, m_count, g, delta):
    """band[k, m] = g[k - m + delta + r] when |k - m + delta| <= r."""
    r = len(g) // 2
    k = np.arange(k_count)[:, None]
    m = np.arange(m_count)[None, :]
    d = k - m + delta
    ok = np.abs(d) <= r
    out = np.zeros((k_count, m_count), dtype=np.float64)
    out[ok] = np.asarray(g)[(d + r)[ok]]
    return out


def _build_host_tensors(kernels0, kernels1, dg_int):
    gys0, gxs0 = _extract_filters(kernels0)
    gys1, gxs1 = _extract_filters(kernels1)
    giy, gix = _extract_integrator(dg_int)
    gys = [gys0, gys1]
    gxs = [gxs0, gxs1]

    # yband per half: [HL, 594] cols = (sigma, fy, h'-local)
    ybands = []
    for half in range(2):
        H0 = half * NOUT
        g0 = 0 if half == 0 else 87
        cols = []
        for s in range(2):
            for fy in range(3):
                cols.append(_band(HL, NH, gys[s][fy], g0 - H0))
        ybands.append(np.concatenate(cols, axis=1).astype(np.float16))

    # xband: [102, 1152] cols = (sigma, half, fx, 96)  -- core-independent
    xcols = []
    for s in range(2):
        for hf in range(2):
            delta = 0 if hf == 0 else (90 - 96)
            for fx in range(3):
                xcols.append(_band(102, 96, gxs[s][fx], delta))
    xband = np.concatenate(xcols, axis=1).astype(np.float16)

    # x-integration bands [96, 372]: valid conv, out w'' n: sum_k S[w'=k+96*hf]*gix[w'-n]
    ix = []
    for hf in range(2):
        k = np.arange(96)[:, None] + 96 * hf
        n = np.arange(NWO)[None, :]
        d = k - n
        ok = (d >= 0) & (d <= 2 * RI)
        b = np.zeros((96, NWO))
        b[ok] = gix[d[ok]]
        ix.append(b)
    intx = np.concatenate(ix, axis=1).astype(np.float16)

    # y-integration band [NH, NOUT]: inty[k, m] = giy[k - m], 0 <= k-m <= 6
    k = np.arange(NH)[:, None]
    m = np.arange(NOUT)[None, :]
    d = k - m
    ok = (d >= 0) & (d <= 2 * RI)
    inty = np.zeros((NH, NOUT))
    inty[ok] = giy[d[ok]]
    inty = inty.astype(np.float16)

    return ybands, xband, intx, inty


def _make_in_maps(inputs, kernels0, kernels1, dg_int):
    x = np.asarray(inputs, dtype=np.float16)
    ybands, xband, intx, inty = _build_host_tensors(kernels0, kernels1, dg_int)
    in_maps = []
    for core in range(8):
        b, half = core // 2, core % 2
        g0 = 0 if half == 0 else 87
        xc = np.ascontiguousarray(
            x[b, g0 : g0 + HL].transpose(0, 2, 1).reshape(HL, C * W)
        )
        in_maps.append(
            {
                "x": xc,
                "yband": ybands[half],
                "xband": xband,
                "intx": intx,
                "inty": inty,
            }
        )
    return in_maps


# pairs per sigma: (m1, m2, weight, diag); m = map id of (j, k):
# (0,0),(1,0),(1,1),(2,0),(2,1),(2,2)
_LMAP = {(0, 0): 0, (1, 0): 1, (1, 1): 2, (2, 0): 3, (2, 1): 4, (2, 2): 5}
_MAP_FYFX = {0: (0, 0), 1: (1, 0), 2: (0, 1), 3: (2, 0), 4: (1, 1), 5: (0, 2)}


def _pair_list():
    pairs = []
    for j in range(3):
        for k1 in range(j + 1):
            for k2 in range(k1, j + 1):
                w = math.sqrt(math.comb(j, k1) * math.comb(j, k2))
                pairs.append((_LMAP[(j, k1)], _LMAP[(j, k2)], w, k1 == k2))
    return pairs  # 10 per sigma


_PAIRS = _pair_list()
W_PAIR = [w for (_, _, w, _) in _PAIRS]
# S indices per sigma: j0: [0]; j1: A=1 B=2 D=3 ; j2: A=4 B=5 C=6 D=7 E=8 F=9


# ---------------------------------------------------------------- bass build
def _build_module():
    import concourse.bacc as bacc
    import concourse.mybir as mybir
    import concourse.tile as tile

    f32 = mybir.dt.float32
    f32r = mybir.dt.float32r
    f16 = mybir.dt.float16

    nc = bacc.Bacc("TRN2", target_bir_lowering=False, debug=False, num_devices=8)
    x_d = nc.dram_tensor("x", [HL, C * W], f16, kind="ExternalInput").ap()
    yb_d = nc.dram_tensor("yband", [HL, 594], f16, kind="ExternalInput").ap()
    xb_d = nc.dram_tensor("xband", [102, 1152], f16, kind="ExternalInput").ap()
    ix_d = nc.dram_tensor("intx", [96, 2 * NWO], f16, kind="ExternalInput").ap()
    iy_d = nc.dram_tensor("inty", [NH, NOUT], f16, kind="ExternalInput").ap()
    out_d = nc.dram_tensor("out", [NOUT, NWO * 12], f32, kind="ExternalOutput").ap()

    with tile.TileContext(nc) as tc:
        _emit(tc, nc, x_d, yb_d, xb_d, ix_d, iy_d, out_d)
    nc.compile()
    return nc


class _Balancer:
    """Greedy engine load balancer with static per-instruction cost
    estimates (ns). Tracks projected busy time per engine."""

    # (kind) -> {engine: (ns_per_row, fixed_ns)} -- calibrated to the
    # TimelineSim cost model (Act pays ~185ns SBUF-access init per instr).
    COSTS = {
        "copy_ps": {"act": (0.833, 185.0), "dve": (1.35, 120.0)},
        "sq_ps": {"act": (0.833, 185.0)},
        "sq16": {"act": (0.833, 185.0), "dve": (0.52, 60.0), "pool": (1.98, 140.0)},
        "mul16": {"dve": (0.52, 60.0), "pool": (1.98, 140.0)},
        "add16": {"dve": (0.52, 60.0), "pool": (1.98, 140.0)},
        "add32": {"dve": (1.04, 60.0), "pool": (1.98, 140.0)},
        "copy32": {"act": (0.833, 185.0), "dve": (1.04, 60.0)},
    }

    def __init__(self, nc):
        self.nc = nc
        # pre-charge with the hardcoded work emitted outside the balancer
        # (ESP chains on DVE, sx/SM/ESP-act + act table loads, ESP pool ops)
        self.load = {"act": 0.0, "dve": 0.0, "pool": 0.0}

    def _pick(self, kind, rows, allowed=None):
        tbl = self.COSTS[kind]
        best, best_t = None, None
        for eng, (per, fix) in tbl.items():
            if allowed and eng not in allowed:
                continue
            t = self.load[eng] + rows * per + fix
            if best_t is None or t < best_t:
                best, best_t = eng, t
        per, fix = tbl[best]
        self.load[best] += rows * per + fix
        return best

    DVE_RATE, DVE_FIX = 0.52, 120.0
    POOL_RATE, POOL_FIX = 1.98, 280.0

    def pick_unit(self, rows):
        """Pick dve/pool for a whole product+fold chain and commit its cost."""
        td = self.load["dve"] + rows * self.DVE_RATE + self.DVE_FIX
        tp = self.load["pool"] + rows * self.POOL_RATE + self.POOL_FIX
        if td <= tp:
            self.load["dve"] = td
            return "dve"
        self.load["pool"] = tp
        return "pool"

    def pick_diag(self, sq_rows, fold_rows):
        """For a diagonal stream: square+fold on dve/pool, or the square
        fused into an Act PSUM-read with the fold elsewhere."""
        t_unit_d = self.load["dve"] + (sq_rows + fold_rows) * 0.52 + 120.0
        t_unit_p = self.load["pool"] + (sq_rows + fold_rows) * 1.98 + 280.0
        act_sq = sq_rows * 0.833 + 2 * 210.0  # two 2-bank sq_ps instrs
        t_act = max(
            self.load["act"] + act_sq,
            min(self.load["dve"] + fold_rows * 0.52 + 120.0,
                self.load["pool"] + fold_rows * 1.98 + 280.0),
        )
        best = min(t_unit_d, t_unit_p, t_act)
        if best == t_unit_d:
            self.load["dve"] = t_unit_d
            return ("unit", "dve")
        if best == t_unit_p:
            self.load["pool"] = t_unit_p
            return ("unit", "pool")
        self.load["act"] += act_sq
        if (self.load["dve"] + fold_rows * 0.52 + 120.0 <=
                self.load["pool"] + fold_rows * 1.98 + 280.0):
            self.load["dve"] += fold_rows * 0.52 + 120.0
            return ("act", "dve")
        self.load["pool"] += fold_rows * 1.98 + 280.0
        return ("act", "pool")

    def copy(self, dst, src, kind="copy_ps", allowed=None):
        rows = src.free_size()
        eng = self._pick(kind, rows, allowed)
        if eng == "act":
            self.nc.scalar.copy(dst, src)
        elif eng == "dve":
            self.nc.vector.tensor_copy(dst, src)
        else:
            self.nc.gpsimd.tensor_copy(dst, src)

    def square(self, dst, src, kind="sq16", allowed=None):
        import concourse.mybir as mybir

        rows = src.free_size()
        eng = self._pick(kind, rows, allowed)
        if eng == "act":
            self.nc.scalar.activation(
                dst, src, mybir.ActivationFunctionType.Square, bias=0.0, scale=1.0
            )
        elif eng == "dve":
            self.nc.vector.tensor_mul(dst, src, src)
        else:
            self.nc.gpsimd.tensor_mul(dst, src, src)

    def mul(self, dst, a, b, kind="mul16", allowed=None):
        eng = self._pick(kind, a.free_size(), allowed)
        if eng == "dve":
            self.nc.vector.tensor_mul(dst, a, b)
        else:
            self.nc.gpsimd.tensor_mul(dst, a, b)

    def add(self, dst, a, b, kind="add16", allowed=None):
        eng = self._pick(kind, a.free_size(), allowed)
        if eng == "dve":
            self.nc.vector.tensor_add(dst, a, b)
        else:
            self.nc.gpsimd.tensor_add(dst, a, b)


def _emit(tc, nc, x_d, yb_d, xb_d, ix_d, iy_d, out_d):
    import concourse.mybir as mybir

    f32 = mybir.dt.float32
    f32r = mybir.dt.float32r
    f16 = mybir.dt.float16
    AF = mybir.ActivationFunctionType
    OP = mybir.AluOpType

    bal = _Balancer(nc)

    cpool = tc.alloc_tile_pool(name="consts", bufs=1)
    yb = cpool.tile([HL, 594], f16, name="yb")
    nc.sync.dma_start(yb[:], yb_d[:])
    xb = cpool.tile([102, 1152], f16, name="xb")
    nc.sync.dma_start(xb[:], xb_d[:])
    ixb = cpool.tile([96, 2 * NWO], f16, name="ixb")
    nc.sync.dma_start(ixb[:], ix_d[:])
    iyb = cpool.tile([NH, NOUT], f16, name="iyb")
    nc.sync.dma_start(iyb[:], iy_d[:])
    c_eps = cpool.tile([128, 1], f32, name="c_eps")
    nc.vector.memset(c_eps[:], EPS)
    c_ln10 = cpool.tile([128, 1], f32, name="c_ln10")
    nc.vector.memset(c_ln10[:], math.log(10.0))
    c_ln100 = cpool.tile([128, 1], f32, name="c_ln100")
    nc.vector.memset(c_ln100[:], math.log(100.0))
    b_eps = c_eps[:NOUT]
    b_ln10 = c_ln10[:NOUT]
    b_ln100 = c_ln100[:NOUT]

    spool = tc.alloc_tile_pool(name="smaps", bufs=1)
    S = {}
    for s in range(2):
        for p in range(10):
            S[(s, p)] = spool.tile(
                [96, 2, NH], f16, name=f"S_{s}_{p}", tag=f"S_{s}_{p}"
            )

    xin = tc.alloc_tile_pool(name="xin", bufs=2)
    zpool = tc.alloc_tile_pool(name="zpool", bufs=2)
    lpool = tc.alloc_tile_pool(name="lpool", bufs=8)
    sqpool = tc.alloc_tile_pool(name="sqpool", bufs=6)
    ppool = tc.alloc_tile_pool(name="ppool", bufs=6)
    fpool = tc.alloc_tile_pool(name="fpool", bufs=8)
    zps = tc.alloc_tile_pool(name="zps", bufs=2, space="PSUM")
    xps = tc.alloc_tile_pool(name="xps", bufs=2, space="PSUM")

    def fold_stream(t, s, p, first, eng):
        """t: [96, 2, CBLK, NH] fp16 product tile; fold c into S[(s,p)].
        Whole chain stays on one engine: same-engine program order means
        no cross-engine semaphore hops inside the chain."""
        v = nc.vector if eng == "dve" else nc.gpsimd
        h = CBLK // 2
        f1 = fpool.tile([96, 2, h, NH], f16, name="f1", tag="f1")
        v.tensor_add(f1[:], t[:, :, :h, :], t[:, :, h:, :])
        f2 = fpool.tile([96, 2, h // 2, NH], f16, name="f2", tag="f2")
        v.tensor_add(f2[:], f1[:, :, : h // 2, :], f1[:, :, h // 2 :, :])
        st = S[(s, p)]
        if first:
            v.tensor_add(st[:], f2[:, :, 0, :], f2[:, :, 1, :])
        else:
            f3 = fpool.tile([96, 2, NH], f16, name="f3", tag="f3")
            v.tensor_add(f3[:], f2[:, :, 0, :], f2[:, :, 1, :])
            v.tensor_add(st[:], st[:], f3[:])

    # diag pair index for map m, and cross pairs (p, m1, m2) per sigma
    _DIAG_P = {0: 0, 1: 1, 2: 3, 3: 4, 4: 7, 5: 9}
    _CROSS = [(2, 1, 2), (5, 3, 4), (6, 3, 5), (8, 4, 5)]

    def yconv_steps(blk):
        """Prepare yconv for a channel block: returns (Z, step closures).
        Each step emits one (ci, hf): 2 matmuls + 1 PSUM->SBUF copy."""
        xt = xin.tile([HL, CBLK * W], f16, name="xt", tag="xt")
        nc.sync.dma_start(xt[:], x_d[:, blk * CBLK * W : (blk + 1) * CBLK * W])
        Z = {}
        for hf in range(2):
            Z[hf] = zpool.tile([102, CBLK, 594], f16, name=f"z{hf}", tag=f"z{hf}")

        def make_step(ci, hf):
            def step():
                w0 = 0 if hf == 0 else 90
                lhs = xt[:, ci * W + w0 : ci * W + w0 + 102]
                zp = zps.tile([102, 2, 512], f32, name="zp", tag="zp")
                for s in range(2):
                    nc.tensor.matmul(
                        zp[:, s, :297],
                        lhs,
                        yb[:, s * 297 : (s + 1) * 297],
                        start=True,
                        stop=True,
                    )
                bal.copy(
                    Z[hf][:, ci, :].rearrange("p (s h) -> p s h", s=2),
                    zp[:, :, :297],
                    kind="copy_ps",
                )
            return step

        steps = [make_step(ci, hf) for ci in range(CBLK) for hf in range(2)]
        return Z, steps

    def gram_blk(blk, Z, ysteps):
        """Emit xconv + products + folds for one channel block,
        interleaving next block's yconv steps to keep PE flowing."""
        ysteps = list(ysteps)
        rows_t = 2 * CBLK * NH
        for s in range(2):
            L = {}
            SQ = {}
            diag_mode = {}
            for m in range(6):
                if ysteps:
                    ysteps.pop(0)()
                fy, fx = _MAP_FYFX[m]
                need_plain = m >= 1
                sq = sqpool.tile([96, 2, CBLK, NH], f16, name=f"sq{m}", tag="sq")
                SQ[m] = sq
                if need_plain:
                    lt = lpool.tile([96, 2, CBLK, NH], f16, name=f"l{m}", tag="lt")
                    L[m] = lt
                    eng = bal.pick_unit(2 * rows_t)
                    diag_mode[m] = (eng, eng)
                else:
                    diag_mode[m] = ("act", None)
                for hf in range(2):
                    xb_col = (s * 2 + hf) * 3 + fx
                    lhsT = xb[:, xb_col * 96 : (xb_col + 1) * 96]
                    xp = xps.tile([96, 2, 512], f32, name="xp", tag="xp")
                    for cs in range(2):
                        rhs = Z[hf][
                            :, cs * CSUB : (cs + 1) * CSUB,
                            s * 297 + fy * NH : s * 297 + (fy + 1) * NH,
                        ]
                        nc.tensor.matmul(
                            xp[:, cs, : CSUB * NH], lhsT, rhs, start=True, stop=True
                        )
                    srcap = xp[:, :, : CSUB * NH].rearrange(
                        "p b (c h) -> p b c h", c=CSUB
                    )
                    sqdst = sq[:, hf].rearrange("p (b c) h -> p b c h", b=2)
                    if need_plain:
                        dst = lt[:, hf].rearrange("p (b c) h -> p b c h", b=2)
                        bal.copy(dst, srcap, kind="copy_ps")
                    else:
                        # m=0: PSUM square must ride Act (single-src)
                        nc.scalar.activation(
                            sqdst, srcap, mybir.ActivationFunctionType.Square,
                            bias=0.0, scale=1.0,
                        )
            # products + folds, both halves at once; each product+fold
            # chain is pinned to one engine (picked by projected load)
            for m in range(6):
                if ysteps and len(ysteps) > (11 - (s * 6 + m)):
                    ysteps.pop(0)()
                sq = SQ[m]
                sqeng, feng = diag_mode[m]
                if m == 0:
                    eng = bal.pick_unit(rows_t)
                else:
                    eng = feng
                    af = L[m][:].rearrange("p f c h -> p (f c h)")
                    sqf = sq[:].rearrange("p f c h -> p (f c h)")
                    if sqeng == "dve":
                        nc.vector.tensor_mul(sqf, af, af)
                    else:
                        nc.gpsimd.tensor_mul(sqf, af, af)
                fold_stream(sq[:], s, _DIAG_P[m], blk == 0, eng)
                for p, m1, m2 in _CROSS:
                    if m2 != m:
                        continue
                    eng = bal.pick_unit(2 * rows_t)
                    pt = ppool.tile([96, 2, CBLK, NH], f16, name="P", tag="P")
                    a1 = L[m1][:].rearrange("p f c h -> p (f c h)")
                    a2 = L[m2][:].rearrange("p f c h -> p (f c h)")
                    ptf = pt[:].rearrange("p f c h -> p (f c h)")
                    if eng == "dve":
                        nc.vector.tensor_mul(ptf, a1, a2)
                    else:
                        nc.gpsimd.tensor_mul(ptf, a1, a2)
                    fold_stream(pt[:], s, p, blk == 0, eng)
        for st in ysteps:
            st()

    # software pipeline: blk 0's yconv runs up front; thereafter blk+1's
    # yconv steps are interleaved into gram(blk)
    Zcur, steps0 = yconv_steps(0)
    for st in steps0:
        st()
    for blk in range(NBLK):
        if blk + 1 < NBLK:
            Znext, ysteps = yconv_steps(blk + 1)
        else:
            Znext, ysteps = None, []
        gram_blk(blk, Zcur, ysteps)
        Zcur = Znext

    for pool in (xps, zps, fpool, ppool, sqpool, lpool, zpool, xin):
        pool.release()

    # ---- integration -> SM maps [NOUT, NWO] (fp32)
    impool = tc.alloc_tile_pool(name="impool", bufs=1)
    SM = {}
    for s in range(2):
        for p in range(10):
            SM[(s, p)] = impool.tile(
                [NOUT, NWO], f32, name=f"SM_{s}_{p}", tag=f"SM_{s}_{p}"
            )
    sxp = tc.alloc_tile_pool(name="sxp", bufs=3)
    ips = tc.alloc_tile_pool(name="ips", bufs=2, space="PSUM")
    yps = tc.alloc_tile_pool(name="yps", bufs=2, space="PSUM")
    for s in range(2):
        for p in range(10):
            ip = ips.tile([NH, NWO], f32, name="ip", tag="ip")
            nc.tensor.matmul(
                ip[:], S[(s, p)][:, 0, :], ixb[:, :NWO], start=True, stop=False
            )
            nc.tensor.matmul(
                ip[:], S[(s, p)][:, 1, :], ixb[:, NWO:], start=False, stop=True
            )
            sx = sxp.tile([NH, NWO], f16, name="sx", tag="sx")
            nc.scalar.activation(sx[:], ip[:], AF.Copy, bias=0.0, scale=W_PAIR[p])
            yp = yps.tile([NOUT, NWO], f32, name="yp", tag="yp")
            nc.tensor.matmul(yp[:], iyb[:], sx[:], start=True, stop=True)
            bal.copy(SM[(s, p)][:], yp[:], kind="copy_ps")
    for pool in (yps, ips, sxp):
        pool.release()

    # ---- ESP + output
    opool = tc.alloc_tile_pool(name="opool", bufs=1)
    OUTT = opool.tile([NOUT, NWO * 12], f32, name="OUTT")
    outv = OUTT[:].rearrange("p (w c) -> p w c", c=12)
    epool = tc.alloc_tile_pool(name="epool", bufs=2)

    def et(name):
        return epool.tile([NOUT, NWO], f32, name=name, tag=name)

    roots = []  # (av_tile, ln_scale, exp_bias, exp_scale, out_ch)

    for s in range(2):
        ch0 = s * 6
        # ---- j = 0
        m0 = SM[(s, 0)]
        t0 = et("t0j0")
        nc.scalar.activation(t0[:], m0[:], AF.Abs)
        nc.vector.tensor_scalar(outv[:, :, ch0 + 0], t0[:], EPS, None, OP.add, OP.bypass)
        # ---- j = 1 : A=1 B=2 D=3
        A, Bm, D = SM[(s, 1)], SM[(s, 2)], SM[(s, 3)]
        p1 = et("p1j1")
        nc.vector.tensor_add(p1[:], A[:], D[:])
        t = et("tj1")
        nc.scalar.activation(t[:], p1[:], AF.Abs)
        nc.vector.tensor_scalar(outv[:, :, ch0 + 1], t[:], EPS, 10.0, OP.add, OP.mult)
        q = et("qj1")
        nc.vector.scalar_tensor_tensor(q[:], p1[:], 1.0, p1[:], OP.mult, OP.mult)
        sA = et("sAj1")
        nc.vector.scalar_tensor_tensor(sA[:], A[:], 1.0, A[:], OP.mult, OP.mult)
        sB2 = et("sBj1")
        nc.vector.scalar_tensor_tensor(sB2[:], Bm[:], 2.0, Bm[:], OP.mult, OP.mult)
        sD = et("sDj1")
        nc.vector.scalar_tensor_tensor(sD[:], D[:], 1.0, D[:], OP.mult, OP.mult)
        p2 = et("p2j1")
        nc.vector.tensor_add(p2[:], sA[:], sB2[:])
        nc.vector.tensor_add(p2[:], p2[:], sD[:])
        v2 = et("v2j1")
        nc.vector.tensor_sub(v2[:], q[:], p2[:])
        av = et("avj1")
        nc.scalar.activation(av[:], v2[:], AF.Abs)
        roots.append((av, 0.5, b_ln10, 0.5, ch0 + 2))
        # ---- j = 2 : A=4 B=5 C=6 D=7 E=8 F=9
        A, Bm, Cm, D, E, F = (SM[(s, i)] for i in range(4, 10))
        sA, sB, sC, sD, sE, sF = (et(f"s{i}j2") for i in range(6))
        for i, (dst, src) in enumerate(
            ((sA, A), (sB, Bm), (sC, Cm), (sD, D), (sE, E), (sF, F))
        ):
            if i % 2 == 0:
                nc.gpsimd.tensor_mul(dst[:], src[:], src[:])
            else:
                nc.vector.scalar_tensor_tensor(
                    dst[:], src[:], 1.0, src[:], OP.mult, OP.mult
                )
        tAD = et("tADj2")
        nc.vector.tensor_add(tAD[:], A[:], D[:])
        p1 = et("p1j2")
        nc.vector.tensor_add(p1[:], tAD[:], F[:])
        t = et("tj2")
        nc.scalar.activation(t[:], p1[:], AF.Abs)
        nc.vector.tensor_scalar(outv[:, :, ch0 + 3], t[:], EPS, 100.0, OP.add, OP.mult)
        p2 = et("p2j2")
        nc.vector.tensor_add(p2[:], sA[:], sD[:])
        nc.vector.tensor_add(p2[:], p2[:], sF[:])
        u = et("uj2")
        nc.vector.tensor_add(u[:], sB[:], sC[:])
        nc.vector.tensor_add(u[:], u[:], sE[:])
        nc.vector.scalar_tensor_tensor(p2[:], u[:], 2.0, p2[:], OP.mult, OP.add)
        q = et("qj2")
        nc.vector.scalar_tensor_tensor(q[:], p1[:], 1.0, p1[:], OP.mult, OP.mult)
        v2 = et("v2j2")
        nc.vector.tensor_sub(v2[:], q[:], p2[:])
        av = et("avj2")
        nc.scalar.activation(av[:], v2[:], AF.Abs)
        roots.append((av, 0.5, b_ln100, 0.5, ch0 + 4))
        # p3 = cubes + 3*(B^2(A+D) + C^2(A+F) + E^2(D+F)) + 6BCE
        cA = et("cAj2")
        nc.vector.scalar_tensor_tensor(cA[:], sA[:], 1.0, A[:], OP.mult, OP.mult)
        cD = et("cDj2")
        nc.vector.scalar_tensor_tensor(cD[:], sD[:], 1.0, D[:], OP.mult, OP.mult)
        cF = et("cFj2")
        nc.vector.scalar_tensor_tensor(cF[:], sF[:], 1.0, F[:], OP.mult, OP.mult)
        w1 = et("w1j2")
        nc.vector.tensor_add(w1[:], cA[:], cD[:])
        nc.vector.tensor_add(w1[:], w1[:], cF[:])
        y1 = et("y1j2")
        nc.vector.scalar_tensor_tensor(y1[:], sB[:], 1.0, tAD[:], OP.mult, OP.mult)
        tAF = et("tAFj2")
        nc.vector.tensor_add(tAF[:], A[:], F[:])
        y2 = et("y2j2")
        nc.vector.scalar_tensor_tensor(y2[:], sC[:], 1.0, tAF[:], OP.mult, OP.mult)
        tDF = et("tDFj2")
        nc.vector.tensor_add(tDF[:], D[:], F[:])
        y3 = et("y3j2")
        nc.vector.scalar_tensor_tensor(y3[:], sE[:], 1.0, tDF[:], OP.mult, OP.mult)
        nc.vector.tensor_add(y1[:], y1[:], y2[:])
        nc.vector.tensor_add(y1[:], y1[:], y3[:])
        z = et("zj2")
        nc.vector.scalar_tensor_tensor(z[:], Bm[:], 6.0, Cm[:], OP.mult, OP.mult)
        nc.vector.scalar_tensor_tensor(z[:], z[:], 1.0, E[:], OP.mult, OP.mult)
        nc.vector.scalar_tensor_tensor(y1[:], y1[:], 3.0, z[:], OP.mult, OP.add)
        p3 = et("p3j2")
        nc.vector.tensor_add(p3[:], w1[:], y1[:])
        # e3*3 = v2/2*p1 - p1*p2 + p3
        a3 = et("a3j2")
        nc.vector.scalar_tensor_tensor(a3[:], v2[:], 0.5, p1[:], OP.mult, OP.mult)
        b3 = et("b3j2")
        nc.vector.scalar_tensor_tensor(b3[:], p1[:], 1.0, p2[:], OP.mult, OP.mult)
        nc.vector.tensor_sub(a3[:], a3[:], b3[:])
        nc.vector.tensor_add(a3[:], a3[:], p3[:])
        av3 = et("av3j2")
        nc.scalar.activation(av3[:], a3[:], AF.Abs)
        roots.append((av3, 1.0 / 3.0, b_ln100, 1.0 / 3.0, ch0 + 5))

    # batched by activation function so the Act engine loads each
    # function table once instead of ping-ponging Ln/Exp per root
    lgs = []
    for i, (av, lns, ebias, escale, ch) in enumerate(roots):
        lg = et(f"lg{i}")
        nc.scalar.activation(lg[:], av[:], AF.Ln, bias=b_eps, scale=lns)
        lgs.append(lg)
    for i, (av, lns, ebias, escale, ch) in enumerate(roots):
        nc.scalar.activation(outv[:, :, ch], lgs[i][:], AF.Exp, bias=ebias, scale=escale)

    nc.sync.dma_start(out_d[:], OUTT[:])
    for pool in (epool, opool, impool, spool, cpool):
        pool.release()


def _get_module():
    key = CONV_MODE
    if key not in _CACHE:
        _CACHE[key] = _build_module()
    return _CACHE[key]


# ---------------------------------------------------------------- entry point
def kernel(inputs, kernels0, kernels1, dg_int):
    from concourse.bass_utils import run_bass_kernel_spmd

    in_maps = _make_in_maps(inputs, kernels0, kernels1, dg_int)
    nc = _get_module()
    res = run_bass_kernel_spmd(nc, in_maps, core_ids=list(range(8)), **RUN_KWARGS)
    global LAST
    LAST = res
    out = np.empty((B, NWO, NWO, 12), dtype=np.float32)
    for core in range(8):
        b, half = core // 2, core % 2
        H0 = half * NOUT
        out[b, H0 : H0 + NOUT] = res.results[core]["out"].reshape(NOUT, NWO, 12)
    return out


# revision 55
# speedup vs baseline: 1.0120x; 1.0120x over previous
"""Trainium2 Bass kernel for nn_GaussianDerivativeESPLayer.

Strategy (per the data-parallel hint, extended since B=4 < 8 cores):
shard (batch b, H-half) across the 8 cores. Each core computes output
rows [H0, H0+93) of one batch element from input rows [g0, g0+105).

Performance design (512us -> 305us in the TimelineSim cost model):
  - fp16 conv matmuls (1 cyc/row on PE vs 4 for fp32); integration
    matmuls fp16 as well.
  - j0 diagonal Gram entry fused: Act-engine Square reads conv PSUM and
    writes the squared map to SBUF fp16 in one instruction (copy+mul).
  - Cross products / squares / channel-fold adds run in fp16 (DVE 2x
    perf mode). Each product+fold chain is pinned to one engine
    (DVE or Pool, greedy by projected load) so the chain rides implicit
    same-engine ordering instead of cross-engine semaphores. PSUM->SBUF
    copies are balanced between Act and DVE (Pool cannot access PSUM;
    TensorTensor cannot take two PSUM sources; DMA cannot read PSUM).
  - hf-merged tiles [96, 2, c, 99] and 2-PSUM-bank batched copies halve
    instruction counts (Act pays ~185ns access-init per instruction).
  - Software pipelining: the next channel block's yconv matmuls+copies
    are interleaved into the current block's xconv/product emission so
    parked matmuls never head-of-line block the PE sequencer.
  - ESP stage in fp32 (Newton-identity cancellations), with Ln/Exp
    calls batched by function to avoid Act table reloads; the
    sqrt-binomial pair weights fold into the integration scale copy.

Per-core pipeline (all layouts [partition, free]):
  1. yconv: data-stationary matmuls X^T @ yband -> Z [w, (s, fy, c, h')]
  2. xconv: band-stationary matmuls xband^T @ Z -> L [w', (c, h')] PSUM
  3. Gram: Square/mult + fold-tree over 64 channels -> 20 S maps [96, 99]
  4. Integration: valid 7x7 separable conv as two matmul stages
  5. ESP: Newton-identity polynomials per pixel, roots via exp/ln.
"""

import math

import numpy as np

B, H, W, C = 4, 192, 192, 64
NH = 99  # h' rows computed per core (pre y-integration)
NOUT = 93  # h'' output rows per core
HL = 105  # input rows per core
NWO = 186  # output cols
CBLK = 8  # channels per block
NBLK = C // CBLK
CSUB = 4  # channels per xconv matmul chunk
RI = 3  # integrator radius
EPS = float(np.finfo(np.float64).eps)

CONV_MODE = "fp16"

_CACHE = {}
RUN_KWARGS = {}  # test harness can set dict(trace=True) before calling kernel()
LAST = None  # BassKernelResults of the most recent kernel() call


# ---------------------------------------------------------------- host math
def _extract_filters(kernels):
    """kernels [6, kh, kw, C, 1] channel-tiled separable. Returns
    (gys, gxs): 1D filters with sigma^order folded in, such that
    kernel(j,k) == outer(gys[j-k], gxs[k])."""
    K = np.asarray(kernels, dtype=np.float64)[:, :, :, 0, 0]
    i0 = K.shape[1] // 2
    s0 = math.sqrt(abs(K[0][i0, i0]))
    g0y = K[0][:, i0] / s0
    g0x = K[0][i0, :] / s0
    g1x = K[2][i0, :] / g0y[i0]  # (j=1,k=1) = s*g1x (x) * g0y (y)
    g1y = K[1][:, i0] / g0x[i0]  # (j=1,k=0) = s*g0x (x) * g1y (y)
    g2x = K[5][i0, :] / g0y[i0]
    g2y = K[3][:, i0] / g0x[i0]
    return [g0y, g1y, g2y], [g0x, g1x, g2x]


def _extract_integrator(dg_int):
    K = np.asarray(dg_int, dtype=np.float64)[:, :, 0, 0]
    i0 = K.shape[0] // 2
    s0 = math.sqrt(abs(K[i0, i0]))
    return K[:, i0] / s0, K[i0, :] / s0  # giy, gix


def _band(k_count, m_count, g, delta):
    """band[k, m] = g[k - m + delta + r] when |k - m + delta| <= r."""
    r = len(g) // 2
    k = np.arange(k_count)[:, None]
    m = np.arange(m_count)[None, :]
    d = k - m + delta
    ok = np.abs(d) <= r
    out = np.zeros((k_count, m_count), dtype=np.float64)
    out[ok] = np.asarray(g)[(d + r)[ok]]
    return out


def _build_host_tensors(kernels0, kernels1, dg_int):
    gys0, gxs0 = _extract_filters(kernels0)
    gys1, gxs1 = _extract_filters(kernels1)
    giy, gix = _extract_integrator(dg_int)
    gys = [gys0, gys1]
    gxs = [gxs0, gxs1]

    # yband per half: [HL, 594] cols = (sigma, fy, h'-local)
    ybands = []
    for half in range(2):
        H0 = half * NOUT
        g0 = 0 if half == 0 else 87
        cols = []
        for s in range(2):
            for fy in range(3):
                cols.append(_band(HL, NH, gys[s][fy], g0 - H0))
        ybands.append(np.concatenate(cols, axis=1).astype(np.float16))

    # xband: [102, 1152] cols = (sigma, half, fx, 96)  -- core-independent
    xcols = []
    for s in range(2):
        for hf in range(2):
            delta = 0 if hf == 0 else (90 - 96)
            for fx in range(3):
                xcols.append(_band(102, 96, gxs[s][fx], delta))
    xband = np.concatenate(xcols, axis=1).astype(np.float16)

    # x-integration bands [96, 372]: valid conv, out w'' n: sum_k S[w'=k+96*hf]*gix[w'-n]
    ix = []
    for hf in range(2):
        k = np.arange(96)[:, None] + 96 * hf
        n = np.arange(NWO)[None, :]
        d = k - n
        ok = (d >= 0) & (d <= 2 * RI)
        b = np.zeros((96, NWO))
        b[ok] = gix[d[ok]]
        ix.append(b)
    intx = np.concatenate(ix, axis=1).astype(np.float16)

    # y-integration band [NH, NOUT]: inty[k, m] = giy[k - m], 0 <= k-m <= 6
    k = np.arange(NH)[:, None]
    m = np.arange(NOUT)[None, :]
    d = k - m
    ok = (d >= 0) & (d <= 2 * RI)
    inty = np.zeros((NH, NOUT))
    inty[ok] = giy[d[ok]]
    inty = inty.astype(np.float16)

    return ybands, xband, intx, inty


def _make_in_maps(inputs, kernels0, kernels1, dg_int):
    x = np.asarray(inputs, dtype=np.float16)
    ybands, xband, intx, inty = _build_host_tensors(kernels0, kernels1, dg_int)
    in_maps = []
    for core in range(8):
        b, half = core // 2, core % 2
        g0 = 0 if half == 0 else 87
        xc = np.ascontiguousarray(
            x[b, g0 : g0 + HL].transpose(0, 2, 1).reshape(HL, C * W)
        )
        in_maps.append(
            {
                "x": xc,
                "yband": ybands[half],
                "xband": xband,
                "intx": intx,
                "inty": inty,
            }
        )
    return in_maps


# pairs per sigma: (m1, m2, weight, diag); m = map id of (j, k):
# (0,0),(1,0),(1,1),(2,0),(2,1),(2,2)
_LMAP = {(0, 0): 0, (1, 0): 1, (1, 1): 2, (2, 0): 3, (2, 1): 4, (2, 2): 5}
_MAP_FYFX = {0: (0, 0), 1: (1, 0), 2: (0, 1), 3: (2, 0), 4: (1, 1), 5: (0, 2)}


def _pair_list():
    pairs = []
    for j in range(3):
        for k1 in range(j + 1):
            for k2 in range(k1, j + 1):
                w = math.sqrt(math.comb(j, k1) * math.comb(j, k2))
                pairs.append((_LMAP[(j, k1)], _LMAP[(j, k2)], w, k1 == k2))
    return pairs  # 10 per sigma


_PAIRS = _pair_list()
W_PAIR = [w for (_, _, w, _) in _PAIRS]
# S indices per sigma: j0: [0]; j1: A=1 B=2 D=3 ; j2: A=4 B=5 C=6 D=7 E=8 F=9


# ---------------------------------------------------------------- bass build
def _build_module():
    import concourse.bacc as bacc
    import concourse.mybir as mybir
    import concourse.tile as tile

    f32 = mybir.dt.float32
    f32r = mybir.dt.float32r
    f16 = mybir.dt.float16

    nc = bacc.Bacc("TRN2", target_bir_lowering=False, debug=False, num_devices=8)
    x_d = nc.dram_tensor("x", [HL, C * W], f16, kind="ExternalInput").ap()
    yb_d = nc.dram_tensor("yband", [HL, 594], f16, kind="ExternalInput").ap()
    xb_d = nc.dram_tensor("xband", [102, 1152], f16, kind="ExternalInput").ap()
    ix_d = nc.dram_tensor("intx", [96, 2 * NWO], f16, kind="ExternalInput").ap()
    iy_d = nc.dram_tensor("inty", [NH, NOUT], f16, kind="ExternalInput").ap()
    out_d = nc.dram_tensor("out", [NOUT, NWO * 12], f32, kind="ExternalOutput").ap()

    with tile.TileContext(nc) as tc:
        _emit(tc, nc, x_d, yb_d, xb_d, ix_d, iy_d, out_d)
    nc.compile()
    return nc


class _Balancer:
    """Greedy engine load balancer with static per-instruction cost
    estimates (ns). Tracks projected busy time per engine."""

    # (kind) -> {engine: (ns_per_row, fixed_ns)} -- calibrated to the
    # TimelineSim cost model (Act pays ~185ns SBUF-access init per instr).
    COSTS = {
        "copy_ps": {"act": (0.833, 185.0), "dve": (1.35, 120.0)},
        "sq_ps": {"act": (0.833, 185.0)},
        "sq16": {"act": (0.833, 185.0), "dve": (0.52, 60.0), "pool": (1.98, 140.0)},
        "mul16": {"dve": (0.52, 60.0), "pool": (1.98, 140.0)},
        "add16": {"dve": (0.52, 60.0), "pool": (1.98, 140.0)},
        "add32": {"dve": (1.04, 60.0), "pool": (1.98, 140.0)},
        "copy32": {"act": (0.833, 185.0), "dve": (1.04, 60.0)},
    }

    def __init__(self, nc):
        self.nc = nc
        # pre-charge with the hardcoded work emitted outside the balancer
        # (ESP chains on DVE, sx/SM/ESP-act + act table loads, ESP pool ops)
        self.load = {"act": 0.0, "dve": 0.0, "pool": 0.0}

    def _pick(self, kind, rows, allowed=None):
        tbl = self.COSTS[kind]
        best, best_t = None, None
        for eng, (per, fix) in tbl.items():
            if allowed and eng not in allowed:
                continue
            t = self.load[eng] + rows * per + fix
            if best_t is None or t < best_t:
                best, best_t = eng, t
        per, fix = tbl[best]
        self.load[best] += rows * per + fix
        return best

    DVE_RATE, DVE_FIX = 0.52, 120.0
    POOL_RATE, POOL_FIX = 1.98, 280.0

    def pick_unit(self, rows):
        """Pick dve/pool for a whole product+fold chain and commit its cost."""
        td = self.load["dve"] + rows * self.DVE_RATE + self.DVE_FIX
        tp = self.load["pool"] + rows * self.POOL_RATE + self.POOL_FIX
        if td <= tp:
            self.load["dve"] = td
            return "dve"
        self.load["pool"] = tp
        return "pool"

    def pick_diag(self, sq_rows, fold_rows):
        """For a diagonal stream: square+fold on dve/pool, or the square
        fused into an Act PSUM-read with the fold elsewhere."""
        t_unit_d = self.load["dve"] + (sq_rows + fold_rows) * 0.52 + 120.0
        t_unit_p = self.load["pool"] + (sq_rows + fold_rows) * 1.98 + 280.0
        act_sq = sq_rows * 0.833 + 2 * 210.0  # two 2-bank sq_ps instrs
        t_act = max(
            self.load["act"] + act_sq,
            min(self.load["dve"] + fold_rows * 0.52 + 120.0,
                self.load["pool"] + fold_rows * 1.98 + 280.0),
        )
        best = min(t_unit_d, t_unit_p, t_act)
        if best == t_unit_d:
            self.load["dve"] = t_unit_d
            return ("unit", "dve")
        if best == t_unit_p:
            self.load["pool"] = t_unit_p
            return ("unit", "pool")
        self.load["act"] += act_sq
        if (self.load["dve"] + fold_rows * 0.52 + 120.0 <=
                self.load["pool"] + fold_rows * 1.98 + 280.0):
            self.load["dve"] += fold_rows * 0.52 + 120.0
            return ("act", "dve")
        self.load["pool"] += fold_rows * 1.98 + 280.0
        return ("act", "pool")

    def copy(self, dst, src, kind="copy_ps", allowed=None):
        rows = src.free_size()
        eng = self._pick(kind, rows, allowed)
        if eng == "act":
            self.nc.scalar.copy(dst, src)
        elif eng == "dve":
            self.nc.vector.tensor_copy(dst, src)
        else:
            self.nc.gpsimd.tensor_copy(dst, src)

    def square(self, dst, src, kind="sq16", allowed=None):
        import concourse.mybir as mybir

        rows = src.free_size()
        eng = self._pick(kind, rows, allowed)
        if eng == "act":
            self.nc.scalar.activation(
                dst, src, mybir.ActivationFunctionType.Square, bias=0.0, scale=1.0
            )
        elif eng == "dve":
            self.nc.vector.tensor_mul(dst, src, src)
        else:
            self.nc.gpsimd.tensor_mul(dst, src, src)

    def mul(self, dst, a, b, kind="mul16", allowed=None):
        eng = self._pick(kind, a.free_size(), allowed)
        if eng == "dve":
            self.nc.vector.tensor_mul(dst, a, b)
        else:
            self.nc.gpsimd.tensor_mul(dst, a, b)

    def add(self, dst, a, b, kind="add16", allowed=None):
        eng = self._pick(kind, a.free_size(), allowed)
        if eng == "dve":
            self.nc.vector.tensor_add(dst, a, b)
        else:
            self.nc.gpsimd.tensor_add(dst, a, b)


def _emit(tc, nc, x_d, yb_d, xb_d, ix_d, iy_d, out_d):
    import concourse.mybir as mybir

    f32 = mybir.dt.float32
    f32r = mybir.dt.float32r
    f16 = mybir.dt.float16
    AF = mybir.ActivationFunctionType
    OP = mybir.AluOpType

    bal = _Balancer(nc)

    cpool = tc.alloc_tile_pool(name="consts", bufs=1)
    yb = cpool.tile([HL, 594], f16, name="yb")
    nc.sync.dma_start(yb[:], yb_d[:])
    xb = cpool.tile([102, 1152], f16, name="xb")
    nc.sync.dma_start(xb[:], xb_d[:])
    ixb = cpool.tile([96, 2 * NWO], f16, name="ixb")
    nc.sync.dma_start(ixb[:], ix_d[:])
    iyb = cpool.tile([NH, NOUT], f16, name="iyb")
    nc.sync.dma_start(iyb[:], iy_d[:])
    c_eps = cpool.tile([128, 1], f32, name="c_eps")
    nc.vector.memset(c_eps[:], EPS)
    c_ln10 = cpool.tile([128, 1], f32, name="c_ln10")
    nc.vector.memset(c_ln10[:], math.log(10.0))
    c_ln100 = cpool.tile([128, 1], f32, name="c_ln100")
    nc.vector.memset(c_ln100[:], math.log(100.0))
    c_eps1e2 = cpool.tile([128, 1], f32, name="c_eps1e2")
    nc.vector.memset(c_eps1e2[:], 100.0 * EPS)
    c_eps1e4 = cpool.tile([128, 1], f32, name="c_eps1e4")
    nc.vector.memset(c_eps1e4[:], 10000.0 * EPS)
    b_eps = c_eps[:NOUT]
    b_ln10 = c_ln10[:NOUT]
    b_ln100 = c_ln100[:NOUT]
    b_eps1e2 = c_eps1e2[:NOUT]
    b_eps1e4 = c_eps1e4[:NOUT]

    spool = tc.alloc_tile_pool(name="smaps", bufs=1)
    S = {}
    for s in range(2):
        for p in range(10):
            S[(s, p)] = spool.tile(
                [96, 2, NH], f16, name=f"S_{s}_{p}", tag=f"S_{s}_{p}"
            )

    xin = tc.alloc_tile_pool(name="xin", bufs=2)
    zpool = tc.alloc_tile_pool(name="zpool", bufs=2)
    lpool = tc.alloc_tile_pool(name="lpool", bufs=8)
    sqpool = tc.alloc_tile_pool(name="sqpool", bufs=6)
    ppool = tc.alloc_tile_pool(name="ppool", bufs=6)
    fpool = tc.alloc_tile_pool(name="fpool", bufs=8)
    zps = tc.alloc_tile_pool(name="zps", bufs=2, space="PSUM")
    xps = tc.alloc_tile_pool(name="xps", bufs=2, space="PSUM")

    def fold_stream(t, s, p, first, eng):
        """t: [96, 2, CBLK, NH] fp16 product tile; fold c into S[(s,p)].
        Whole chain stays on one engine: same-engine program order means
        no cross-engine semaphore hops inside the chain."""
        v = nc.vector if eng == "dve" else nc.gpsimd
        h = CBLK // 2
        f1 = fpool.tile([96, 2, h, NH], f16, name="f1", tag="f1")
        v.tensor_add(f1[:], t[:, :, :h, :], t[:, :, h:, :])
        f2 = fpool.tile([96, 2, h // 2, NH], f16, name="f2", tag="f2")
        v.tensor_add(f2[:], f1[:, :, : h // 2, :], f1[:, :, h // 2 :, :])
        st = S[(s, p)]
        if first:
            v.tensor_add(st[:], f2[:, :, 0, :], f2[:, :, 1, :])
        else:
            f3 = fpool.tile([96, 2, NH], f16, name="f3", tag="f3")
            v.tensor_add(f3[:], f2[:, :, 0, :], f2[:, :, 1, :])
            v.tensor_add(st[:], st[:], f3[:])

    # diag pair index for map m, and cross pairs (p, m1, m2) per sigma
    _DIAG_P = {0: 0, 1: 1, 2: 3, 3: 4, 4: 7, 5: 9}
    _CROSS = [(2, 1, 2), (5, 3, 4), (6, 3, 5), (8, 4, 5)]

    def yconv_steps(blk):
        """Prepare yconv for a channel block: returns (Z, step closures).
        Each step emits one (ci, hf): 2 matmuls + 1 PSUM->SBUF copy."""
        xt = xin.tile([HL, CBLK * W], f16, name="xt", tag="xt")
        nc.sync.dma_start(xt[:], x_d[:, blk * CBLK * W : (blk + 1) * CBLK * W])
        Z = {}
        for hf in range(2):
            Z[hf] = zpool.tile([102, CBLK, 594], f16, name=f"z{hf}", tag=f"z{hf}")

        def make_step(ci, hf):
            def step():
                w0 = 0 if hf == 0 else 90
                lhs = xt[:, ci * W + w0 : ci * W + w0 + 102]
                zp = zps.tile([102, 2, 512], f32, name="zp", tag="zp")
                for s in range(2):
                    nc.tensor.matmul(
                        zp[:, s, :297],
                        lhs,
                        yb[:, s * 297 : (s + 1) * 297],
                        start=True,
                        stop=True,
                    )
                bal.copy(
                    Z[hf][:, ci, :].rearrange("p (s h) -> p s h", s=2),
                    zp[:, :, :297],
                    kind="copy_ps",
                )
            return step

        steps = [make_step(ci, hf) for ci in range(CBLK) for hf in range(2)]
        return Z, steps

    def gram_blk(blk, Z, ysteps):
        """Emit xconv + products + folds for one channel block,
        interleaving next block's yconv steps to keep PE flowing."""
        ysteps = list(ysteps)
        rows_t = 2 * CBLK * NH
        for s in range(2):
            L = {}
            SQ = {}
            diag_mode = {}
            for m in range(6):
                if ysteps:
                    ysteps.pop(0)()
                fy, fx = _MAP_FYFX[m]
                need_plain = m >= 1
                sq = sqpool.tile([96, 2, CBLK, NH], f16, name=f"sq{m}", tag="sq")
                SQ[m] = sq
                if need_plain:
                    lt = lpool.tile([96, 2, CBLK, NH], f16, name=f"l{m}", tag="lt")
                    L[m] = lt
                    eng = bal.pick_unit(2 * rows_t)
                    diag_mode[m] = (eng, eng)
                else:
                    diag_mode[m] = ("act", None)
                for hf in range(2):
                    xb_col = (s * 2 + hf) * 3 + fx
                    lhsT = xb[:, xb_col * 96 : (xb_col + 1) * 96]
                    xp = xps.tile([96, 2, 512], f32, name="xp", tag="xp")
                    for cs in range(2):
                        rhs = Z[hf][
                            :, cs * CSUB : (cs + 1) * CSUB,
                            s * 297 + fy * NH : s * 297 + (fy + 1) * NH,
                        ]
                        nc.tensor.matmul(
                            xp[:, cs, : CSUB * NH], lhsT, rhs, start=True, stop=True
                        )
                    srcap = xp[:, :, : CSUB * NH].rearrange(
                        "p b (c h) -> p b c h", c=CSUB
                    )
                    sqdst = sq[:, hf].rearrange("p (b c) h -> p b c h", b=2)
                    if need_plain:
                        dst = lt[:, hf].rearrange("p (b c) h -> p b c h", b=2)
                        bal.copy(dst, srcap, kind="copy_ps")
                    else:
                        # m=0: PSUM square must ride Act (single-src)
                        nc.scalar.activation(
                            sqdst, srcap, mybir.ActivationFunctionType.Square,
                            bias=0.0, scale=1.0,
                        )
            # products + folds, both halves at once; each product+fold
            # chain is pinned to one engine (picked by projected load)
            for m in range(6):
                if ysteps and len(ysteps) > (11 - (s * 6 + m)):
                    ysteps.pop(0)()
                sq = SQ[m]
                sqeng, feng = diag_mode[m]
                if m == 0:
                    eng = bal.pick_unit(rows_t)
                else:
                    eng = feng
                    af = L[m][:].rearrange("p f c h -> p (f c h)")
                    sqf = sq[:].rearrange("p f c h -> p (f c h)")
                    if sqeng == "dve":
                        nc.vector.tensor_mul(sqf, af, af)
                    else:
                        nc.gpsimd.tensor_mul(sqf, af, af)
                fold_stream(sq[:], s, _DIAG_P[m], blk == 0, eng)
                for p, m1, m2 in _CROSS:
                    if m2 != m:
                        continue
                    eng = bal.pick_unit(2 * rows_t)
                    pt = ppool.tile([96, 2, CBLK, NH], f16, name="P", tag="P")
                    a1 = L[m1][:].rearrange("p f c h -> p (f c h)")
                    a2 = L[m2][:].rearrange("p f c h -> p (f c h)")
                    ptf = pt[:].rearrange("p f c h -> p (f c h)")
                    if eng == "dve":
                        nc.vector.tensor_mul(ptf, a1, a2)
                    else:
                        nc.gpsimd.tensor_mul(ptf, a1, a2)
                    fold_stream(pt[:], s, p, blk == 0, eng)
        for st in ysteps:
            st()

    # software pipeline: blk 0's yconv runs up front; thereafter blk+1's
    # yconv steps are interleaved into gram(blk)
    Zcur, steps0 = yconv_steps(0)
    for st in steps0:
        st()
    for blk in range(NBLK):
        if blk + 1 < NBLK:
            Znext, ysteps = yconv_steps(blk + 1)
        else:
            Znext, ysteps = None, []
        gram_blk(blk, Zcur, ysteps)
        Zcur = Znext

    for pool in (xps, zps, fpool, ppool, sqpool, lpool, zpool, xin):
        pool.release()

    # ---- integration -> SM maps [NOUT, NWO] (fp32)
    impool = tc.alloc_tile_pool(name="impool", bufs=1)
    SM = {}
    for s in range(2):
        for p in range(10):
            SM[(s, p)] = impool.tile(
                [NOUT, NWO], f32, name=f"SM_{s}_{p}", tag=f"SM_{s}_{p}"
            )
    sxp = tc.alloc_tile_pool(name="sxp", bufs=3)
    ips = tc.alloc_tile_pool(name="ips", bufs=2, space="PSUM")
    yps = tc.alloc_tile_pool(name="yps", bufs=2, space="PSUM")
    for s in range(2):
        for p in range(10):
            ip = ips.tile([NH, NWO], f32, name="ip", tag="ip")
            nc.tensor.matmul(
                ip[:], S[(s, p)][:, 0, :], ixb[:, :NWO], start=True, stop=False
            )
            nc.tensor.matmul(
                ip[:], S[(s, p)][:, 1, :], ixb[:, NWO:], start=False, stop=True
            )
            sx = sxp.tile([NH, NWO], f16, name="sx", tag="sx")
            nc.scalar.activation(sx[:], ip[:], AF.Copy, bias=0.0, scale=W_PAIR[p])
            yp = yps.tile([NOUT, NWO], f32, name="yp", tag="yp")
            nc.tensor.matmul(yp[:], iyb[:], sx[:], start=True, stop=True)
            bal.copy(SM[(s, p)][:], yp[:], kind="copy_ps")
    for pool in (yps, ips, sxp):
        pool.release()

    # ---- ESP + output
    opool = tc.alloc_tile_pool(name="opool", bufs=1)
    OUTT = opool.tile([NOUT, NWO * 12], f32, name="OUTT")
    outv = OUTT[:].rearrange("p (w c) -> p w c", c=12)
    epool = tc.alloc_tile_pool(name="epool", bufs=2)

    def et(name):
        return epool.tile([NOUT, NWO], f32, name=name, tag=name)

    roots = []  # (av_tile, ln_scale, exp_bias, exp_scale, out_ch)

    for s in range(2):
        ch0 = s * 6
        # ---- j = 0
        m0 = SM[(s, 0)]
        t0 = et("t0j0")
        nc.scalar.activation(t0[:], m0[:], AF.Abs)
        nc.vector.tensor_scalar(outv[:, :, ch0 + 0], t0[:], EPS, None, OP.add, OP.bypass)
        # ---- j = 1 : A=1 B=2 D=3
        A, Bm, D = SM[(s, 1)], SM[(s, 2)], SM[(s, 3)]
        p1 = et("p1j1")
        nc.vector.tensor_add(p1[:], A[:], D[:])
        t = et("tj1")
        nc.scalar.activation(t[:], p1[:], AF.Abs)
        nc.vector.tensor_scalar(outv[:, :, ch0 + 1], t[:], EPS, 10.0, OP.add, OP.mult)
        q = et("qj1")
        nc.vector.scalar_tensor_tensor(q[:], p1[:], 1.0, p1[:], OP.mult, OP.mult)
        sA = et("sAj1")
        nc.vector.scalar_tensor_tensor(sA[:], A[:], 1.0, A[:], OP.mult, OP.mult)
        sB2 = et("sBj1")
        nc.vector.scalar_tensor_tensor(sB2[:], Bm[:], 2.0, Bm[:], OP.mult, OP.mult)
        sD = et("sDj1")
        nc.vector.scalar_tensor_tensor(sD[:], D[:], 1.0, D[:], OP.mult, OP.mult)
        p2 = et("p2j1")
        nc.vector.tensor_add(p2[:], sA[:], sB2[:])
        nc.vector.tensor_add(p2[:], p2[:], sD[:])
        v2 = et("v2j1")
        nc.vector.tensor_sub(v2[:], q[:], p2[:])
        av = et("avj1")
        nc.scalar.activation(av[:], v2[:], AF.Abs)
        # 10*(|v2|/2 + eps)^0.5 == sqrt(50*|v2| + 100*eps): one Sqrt op,
        # and Sqrt/Abs/Square/Copy share an act-function table set
        nc.scalar.activation(
            outv[:, :, ch0 + 2], av[:], AF.Sqrt, bias=b_eps1e2, scale=50.0
        )
        # ---- j = 2 : A=4 B=5 C=6 D=7 E=8 F=9
        A, Bm, Cm, D, E, F = (SM[(s, i)] for i in range(4, 10))
        sA, sB, sC, sD, sE, sF = (et(f"s{i}j2") for i in range(6))
        for i, (dst, src) in enumerate(
            ((sA, A), (sB, Bm), (sC, Cm), (sD, D), (sE, E), (sF, F))
        ):
            if i % 2 == 0:
                nc.gpsimd.tensor_mul(dst[:], src[:], src[:])
            else:
                nc.vector.scalar_tensor_tensor(
                    dst[:], src[:], 1.0, src[:], OP.mult, OP.mult
                )
        tAD = et("tADj2")
        nc.vector.tensor_add(tAD[:], A[:], D[:])
        p1 = et("p1j2")
        nc.vector.tensor_add(p1[:], tAD[:], F[:])
        t = et("tj2")
        nc.scalar.activation(t[:], p1[:], AF.Abs)
        nc.vector.tensor_scalar(outv[:, :, ch0 + 3], t[:], EPS, 100.0, OP.add, OP.mult)
        p2 = et("p2j2")
        nc.vector.tensor_add(p2[:], sA[:], sD[:])
        nc.vector.tensor_add(p2[:], p2[:], sF[:])
        u = et("uj2")
        nc.vector.tensor_add(u[:], sB[:], sC[:])
        nc.vector.tensor_add(u[:], u[:], sE[:])
        nc.vector.scalar_tensor_tensor(p2[:], u[:], 2.0, p2[:], OP.mult, OP.add)
        q = et("qj2")
        nc.vector.scalar_tensor_tensor(q[:], p1[:], 1.0, p1[:], OP.mult, OP.mult)
        v2 = et("v2j2")
        nc.vector.tensor_sub(v2[:], q[:], p2[:])
        av = et("avj2")
        nc.scalar.activation(av[:], v2[:], AF.Abs)
        # 100*(|v2|/2 + eps)^0.5 == sqrt(5000*|v2| + 10000*eps)
        nc.scalar.activation(
            outv[:, :, ch0 + 4], av[:], AF.Sqrt, bias=b_eps1e4, scale=5000.0
        )
        # p3 = cubes + 3*(B^2(A+D) + C^2(A+F) + E^2(D+F)) + 6BCE
        cA = et("cAj2")
        nc.vector.scalar_tensor_tensor(cA[:], sA[:], 1.0, A[:], OP.mult, OP.mult)
        cD = et("cDj2")
        nc.vector.scalar_tensor_tensor(cD[:], sD[:], 1.0, D[:], OP.mult, OP.mult)
        cF = et("cFj2")
        nc.vector.scalar_tensor_tensor(cF[:], sF[:], 1.0, F[:], OP.mult, OP.mult)
        w1 = et("w1j2")
        nc.vector.tensor_add(w1[:], cA[:], cD[:])
        nc.vector.tensor_add(w1[:], w1[:], cF[:])
        y1 = et("y1j2")
        nc.vector.scalar_tensor_tensor(y1[:], sB[:], 1.0, tAD[:], OP.mult, OP.mult)
        tAF = et("tAFj2")
        nc.vector.tensor_add(tAF[:], A[:], F[:])
        y2 = et("y2j2")
        nc.vector.scalar_tensor_tensor(y2[:], sC[:], 1.0, tAF[:], OP.mult, OP.mult)
        tDF = et("tDFj2")
        nc.vector.tensor_add(tDF[:], D[:], F[:])
        y3 = et("y3j2")
        nc.vector.scalar_tensor_tensor(y3[:], sE[:], 1.0, tDF[:], OP.mult, OP.mult)
        nc.vector.tensor_add(y1[:], y1[:], y2[:])
        nc.vector.tensor_add(y1[:], y1[:], y3[:])
        z = et("zj2")
        nc.vector.scalar_tensor_tensor(z[:], Bm[:], 6.0, Cm[:], OP.mult, OP.mult)
        nc.vector.scalar_tensor_tensor(z[:], z[:], 1.0, E[:], OP.mult, OP.mult)
        nc.vector.scalar_tensor_tensor(y1[:], y1[:], 3.0, z[:], OP.mult, OP.add)
        p3 = et("p3j2")
        nc.vector.tensor_add(p3[:], w1[:], y1[:])
        # e3*3 = v2/2*p1 - p1*p2 + p3
        a3 = et("a3j2")
        nc.vector.scalar_tensor_tensor(a3[:], v2[:], 0.5, p1[:], OP.mult, OP.mult)
        b3 = et("b3j2")
        nc.vector.scalar_tensor_tensor(b3[:], p1[:], 1.0, p2[:], OP.mult, OP.mult)
        nc.vector.tensor_sub(a3[:], a3[:], b3[:])
        nc.vector.tensor_add(a3[:], a3[:], p3[:])
        av3 = et("av3j2")
        nc.scalar.activation(av3[:], a3[:], AF.Abs)
        roots.append((av3, 1.0 / 3.0, b_ln100, 1.0 / 3.0, ch0 + 5))

    # batched by activation function so the Act engine loads each
    # function table once instead of ping-ponging Ln/Exp per root
    lgs = []
    for i, (av, lns, ebias, escale, ch) in enumerate(roots):
        lg = et(f"lg{i}")
        nc.scalar.activation(lg[:], av[:], AF.Ln, bias=b_eps, scale=lns)
        lgs.append(lg)
    for i, (av, lns, ebias, escale, ch) in enumerate(roots):
        nc.scalar.activation(outv[:, :, ch], lgs[i][:], AF.Exp, bias=ebias, scale=escale)

    nc.sync.dma_start(out_d[:], OUTT[:])
    for pool in (epool, opool, impool, spool, cpool):
        pool.release()


def _get_module():
    key = CONV_MODE
    if key not in _CACHE:
        _CACHE[key] = _build_module()
    return _CACHE[key]


# ---------------------------------------------------------------- entry point
def kernel(inputs, kernels0, kernels1, dg_int):
    from concourse.bass_utils import run_bass_kernel_spmd

    in_maps = _make_in_maps(inputs, kernels0, kernels1, dg_int)
    nc = _get_module()
    res = run_bass_kernel_spmd(nc, in_maps, core_ids=list(range(8)), **RUN_KWARGS)
    global LAST
    LAST = res
    out = np.empty((B, NWO, NWO, 12), dtype=np.float32)
    for core in range(8):
        b, half = core // 2, core % 2
        H0 = half * NOUT
        out[b, H0 : H0 + NOUT] = res.results[core]["out"].reshape(NOUT, NWO, 12)
    return out


# revision 56
# speedup vs baseline: 1.0124x; 1.0004x over previous
"""Trainium2 Bass kernel for nn_GaussianDerivativeESPLayer.

Strategy (per the data-parallel hint, extended since B=4 < 8 cores):
shard (batch b, H-half) across the 8 cores. Each core computes output
rows [H0, H0+93) of one batch element from input rows [g0, g0+105).

Performance design (512us -> 305us in the TimelineSim cost model):
  - fp16 conv matmuls (1 cyc/row on PE vs 4 for fp32); integration
    matmuls fp16 as well.
  - j0 diagonal Gram entry fused: Act-engine Square reads conv PSUM and
    writes the squared map to SBUF fp16 in one instruction (copy+mul).
  - Cross products / squares / channel-fold adds run in fp16 (DVE 2x
    perf mode). Each product+fold chain is pinned to one engine
    (DVE or Pool, greedy by projected load) so the chain rides implicit
    same-engine ordering instead of cross-engine semaphores. PSUM->SBUF
    copies are balanced between Act and DVE (Pool cannot access PSUM;
    TensorTensor cannot take two PSUM sources; DMA cannot read PSUM).
  - hf-merged tiles [96, 2, c, 99] and 2-PSUM-bank batched copies halve
    instruction counts (Act pays ~185ns access-init per instruction).
  - Software pipelining: the next channel block's yconv matmuls+copies
    are interleaved into the current block's xconv/product emission so
    parked matmuls never head-of-line block the PE sequencer.
  - ESP stage in fp32 (Newton-identity cancellations), with Ln/Exp
    calls batched by function to avoid Act table reloads; the
    sqrt-binomial pair weights fold into the integration scale copy.

Per-core pipeline (all layouts [partition, free]):
  1. yconv: data-stationary matmuls X^T @ yband -> Z [w, (s, fy, c, h')]
  2. xconv: band-stationary matmuls xband^T @ Z -> L [w', (c, h')] PSUM
  3. Gram: Square/mult + fold-tree over 64 channels -> 20 S maps [96, 99]
  4. Integration: valid 7x7 separable conv as two matmul stages
  5. ESP: Newton-identity polynomials per pixel, roots via exp/ln.
"""

import math

import numpy as np

B, H, W, C = 4, 192, 192, 64
NH = 99  # h' rows computed per core (pre y-integration)
NOUT = 93  # h'' output rows per core
HL = 105  # input rows per core
NWO = 186  # output cols
CBLK = 8  # channels per block
NBLK = C // CBLK
CSUB = 4  # channels per xconv matmul chunk
RI = 3  # integrator radius
EPS = float(np.finfo(np.float64).eps)

CONV_MODE = "fp16"

_CACHE = {}
RUN_KWARGS = {}  # test harness can set dict(trace=True) before calling kernel()
LAST = None  # BassKernelResults of the most recent kernel() call


# ---------------------------------------------------------------- host math
def _extract_filters(kernels):
    """kernels [6, kh, kw, C, 1] channel-tiled separable. Returns
    (gys, gxs): 1D filters with sigma^order folded in, such that
    kernel(j,k) == outer(gys[j-k], gxs[k])."""
    K = np.asarray(kernels, dtype=np.float64)[:, :, :, 0, 0]
    i0 = K.shape[1] // 2
    s0 = math.sqrt(abs(K[0][i0, i0]))
    g0y = K[0][:, i0] / s0
    g0x = K[0][i0, :] / s0
    g1x = K[2][i0, :] / g0y[i0]  # (j=1,k=1) = s*g1x (x) * g0y (y)
    g1y = K[1][:, i0] / g0x[i0]  # (j=1,k=0) = s*g0x (x) * g1y (y)
    g2x = K[5][i0, :] / g0y[i0]
    g2y = K[3][:, i0] / g0x[i0]
    return [g0y, g1y, g2y], [g0x, g1x, g2x]


def _extract_integrator(dg_int):
    K = np.asarray(dg_int, dtype=np.float64)[:, :, 0, 0]
    i0 = K.shape[0] // 2
    s0 = math.sqrt(abs(K[i0, i0]))
    return K[:, i0] / s0, K[i0, :] / s0  # giy, gix


def _band(k_count, m_count, g, delta):
    """band[k, m] = g[k - m + delta + r] when |k - m + delta| <= r."""
    r = len(g) // 2
    k = np.arange(k_count)[:, None]
    m = np.arange(m_count)[None, :]
    d = k - m + delta
    ok = np.abs(d) <= r
    out = np.zeros((k_count, m_count), dtype=np.float64)
    out[ok] = np.asarray(g)[(d + r)[ok]]
    return out


def _build_host_tensors(kernels0, kernels1, dg_int):
    gys0, gxs0 = _extract_filters(kernels0)
    gys1, gxs1 = _extract_filters(kernels1)
    giy, gix = _extract_integrator(dg_int)
    gys = [gys0, gys1]
    gxs = [gxs0, gxs1]

    # yband per half: [HL, 594] cols = (sigma, fy, h'-local)
    ybands = []
    for half in range(2):
        H0 = half * NOUT
        g0 = 0 if half == 0 else 87
        cols = []
        for s in range(2):
            for fy in range(3):
                cols.append(_band(HL, NH, gys[s][fy], g0 - H0))
        ybands.append(np.concatenate(cols, axis=1).astype(np.float16))

    # xband: [102, 1152] cols = (sigma, half, fx, 96)  -- core-independent
    xcols = []
    for s in range(2):
        for hf in range(2):
            delta = 0 if hf == 0 else (90 - 96)
            for fx in range(3):
                xcols.append(_band(102, 96, gxs[s][fx], delta))
    xband = np.concatenate(xcols, axis=1).astype(np.float16)

    # x-integration bands [96, 372]: valid conv, out w'' n: sum_k S[w'=k+96*hf]*gix[w'-n]
    ix = []
    for hf in range(2):
        k = np.arange(96)[:, None] + 96 * hf
        n = np.arange(NWO)[None, :]
        d = k - n
        ok = (d >= 0) & (d <= 2 * RI)
        b = np.zeros((96, NWO))
        b[ok] = gix[d[ok]]
        ix.append(b)
    intx = np.concatenate(ix, axis=1).astype(np.float16)

    # y-integration band [NH, NOUT]: inty[k, m] = giy[k - m], 0 <= k-m <= 6
    k = np.arange(NH)[:, None]
    m = np.arange(NOUT)[None, :]
    d = k - m
    ok = (d >= 0) & (d <= 2 * RI)
    inty = np.zeros((NH, NOUT))
    inty[ok] = giy[d[ok]]
    inty = inty.astype(np.float16)

    return ybands, xband, intx, inty


def _make_in_maps(inputs, kernels0, kernels1, dg_int):
    x = np.asarray(inputs, dtype=np.float16)
    ybands, xband, intx, inty = _build_host_tensors(kernels0, kernels1, dg_int)
    in_maps = []
    for core in range(8):
        b, half = core // 2, core % 2
        g0 = 0 if half == 0 else 87
        xc = np.ascontiguousarray(
            x[b, g0 : g0 + HL].transpose(0, 2, 1).reshape(HL, C * W)
        )
        in_maps.append(
            {
                "x": xc,
                "yband": ybands[half],
                "xband": xband,
                "intx": intx,
                "inty": inty,
            }
        )
    return in_maps


# pairs per sigma: (m1, m2, weight, diag); m = map id of (j, k):
# (0,0),(1,0),(1,1),(2,0),(2,1),(2,2)
_LMAP = {(0, 0): 0, (1, 0): 1, (1, 1): 2, (2, 0): 3, (2, 1): 4, (2, 2): 5}
_MAP_FYFX = {0: (0, 0), 1: (1, 0), 2: (0, 1), 3: (2, 0), 4: (1, 1), 5: (0, 2)}


def _pair_list():
    pairs = []
    for j in range(3):
        for k1 in range(j + 1):
            for k2 in range(k1, j + 1):
                w = math.sqrt(math.comb(j, k1) * math.comb(j, k2))
                pairs.append((_LMAP[(j, k1)], _LMAP[(j, k2)], w, k1 == k2))
    return pairs  # 10 per sigma


_PAIRS = _pair_list()
W_PAIR = [w for (_, _, w, _) in _PAIRS]
# S indices per sigma: j0: [0]; j1: A=1 B=2 D=3 ; j2: A=4 B=5 C=6 D=7 E=8 F=9


# ---------------------------------------------------------------- bass build
def _build_module():
    import concourse.bacc as bacc
    import concourse.mybir as mybir
    import concourse.tile as tile

    f32 = mybir.dt.float32
    f32r = mybir.dt.float32r
    f16 = mybir.dt.float16

    nc = bacc.Bacc("TRN2", target_bir_lowering=False, debug=False, num_devices=8)
    x_d = nc.dram_tensor("x", [HL, C * W], f16, kind="ExternalInput").ap()
    yb_d = nc.dram_tensor("yband", [HL, 594], f16, kind="ExternalInput").ap()
    xb_d = nc.dram_tensor("xband", [102, 1152], f16, kind="ExternalInput").ap()
    ix_d = nc.dram_tensor("intx", [96, 2 * NWO], f16, kind="ExternalInput").ap()
    iy_d = nc.dram_tensor("inty", [NH, NOUT], f16, kind="ExternalInput").ap()
    out_d = nc.dram_tensor("out", [NOUT, NWO * 12], f32, kind="ExternalOutput").ap()

    with tile.TileContext(nc) as tc:
        _emit(tc, nc, x_d, yb_d, xb_d, ix_d, iy_d, out_d)
    nc.compile()
    return nc


class _Balancer:
    """Greedy engine load balancer with static per-instruction cost
    estimates (ns). Tracks projected busy time per engine."""

    # (kind) -> {engine: (ns_per_row, fixed_ns)} -- calibrated to the
    # TimelineSim cost model (Act pays ~185ns SBUF-access init per instr).
    COSTS = {
        "copy_ps": {"act": (0.833, 185.0), "dve": (1.35, 120.0)},
        "sq_ps": {"act": (0.833, 185.0)},
        "sq16": {"act": (0.833, 185.0), "dve": (0.52, 60.0), "pool": (1.98, 140.0)},
        "mul16": {"dve": (0.52, 60.0), "pool": (1.98, 140.0)},
        "add16": {"dve": (0.52, 60.0), "pool": (1.98, 140.0)},
        "add32": {"dve": (1.04, 60.0), "pool": (1.98, 140.0)},
        "copy32": {"act": (0.833, 185.0), "dve": (1.04, 60.0)},
    }

    def __init__(self, nc):
        self.nc = nc
        # pre-charge with the hardcoded work emitted outside the balancer
        # (ESP chains on DVE, sx/SM/ESP-act + act table loads, ESP pool ops)
        self.load = {"act": 0.0, "dve": 0.0, "pool": 0.0}

    def _pick(self, kind, rows, allowed=None):
        tbl = self.COSTS[kind]
        best, best_t = None, None
        for eng, (per, fix) in tbl.items():
            if allowed and eng not in allowed:
                continue
            t = self.load[eng] + rows * per + fix
            if best_t is None or t < best_t:
                best, best_t = eng, t
        per, fix = tbl[best]
        self.load[best] += rows * per + fix
        return best

    DVE_RATE, DVE_FIX = 0.52, 120.0
    POOL_RATE, POOL_FIX = 1.98, 280.0

    def pick_unit(self, rows):
        """Pick dve/pool for a whole product+fold chain and commit its cost."""
        td = self.load["dve"] + rows * self.DVE_RATE + self.DVE_FIX
        tp = self.load["pool"] + rows * self.POOL_RATE + self.POOL_FIX
        if td <= tp:
            self.load["dve"] = td
            return "dve"
        self.load["pool"] = tp
        return "pool"

    def pick_diag(self, sq_rows, fold_rows):
        """For a diagonal stream: square+fold on dve/pool, or the square
        fused into an Act PSUM-read with the fold elsewhere."""
        t_unit_d = self.load["dve"] + (sq_rows + fold_rows) * 0.52 + 120.0
        t_unit_p = self.load["pool"] + (sq_rows + fold_rows) * 1.98 + 280.0
        act_sq = sq_rows * 0.833 + 2 * 210.0  # two 2-bank sq_ps instrs
        t_act = max(
            self.load["act"] + act_sq,
            min(self.load["dve"] + fold_rows * 0.52 + 120.0,
                self.load["pool"] + fold_rows * 1.98 + 280.0),
        )
        best = min(t_unit_d, t_unit_p, t_act)
        if best == t_unit_d:
            self.load["dve"] = t_unit_d
            return ("unit", "dve")
        if best == t_unit_p:
            self.load["pool"] = t_unit_p
            return ("unit", "pool")
        self.load["act"] += act_sq
        if (self.load["dve"] + fold_rows * 0.52 + 120.0 <=
                self.load["pool"] + fold_rows * 1.98 + 280.0):
            self.load["dve"] += fold_rows * 0.52 + 120.0
            return ("act", "dve")
        self.load["pool"] += fold_rows * 1.98 + 280.0
        return ("act", "pool")

    def copy(self, dst, src, kind="copy_ps", allowed=None):
        rows = src.free_size()
        eng = self._pick(kind, rows, allowed)
        if eng == "act":
            self.nc.scalar.copy(dst, src)
        elif eng == "dve":
            self.nc.vector.tensor_copy(dst, src)
        else:
            self.nc.gpsimd.tensor_copy(dst, src)

    def square(self, dst, src, kind="sq16", allowed=None):
        import concourse.mybir as mybir

        rows = src.free_size()
        eng = self._pick(kind, rows, allowed)
        if eng == "act":
            self.nc.scalar.activation(
                dst, src, mybir.ActivationFunctionType.Square, bias=0.0, scale=1.0
            )
        elif eng == "dve":
            self.nc.vector.tensor_mul(dst, src, src)
        else:
            self.nc.gpsimd.tensor_mul(dst, src, src)

    def mul(self, dst, a, b, kind="mul16", allowed=None):
        eng = self._pick(kind, a.free_size(), allowed)
        if eng == "dve":
            self.nc.vector.tensor_mul(dst, a, b)
        else:
            self.nc.gpsimd.tensor_mul(dst, a, b)

    def add(self, dst, a, b, kind="add16", allowed=None):
        eng = self._pick(kind, a.free_size(), allowed)
        if eng == "dve":
            self.nc.vector.tensor_add(dst, a, b)
        else:
            self.nc.gpsimd.tensor_add(dst, a, b)


def _emit(tc, nc, x_d, yb_d, xb_d, ix_d, iy_d, out_d):
    import concourse.mybir as mybir

    f32 = mybir.dt.float32
    f32r = mybir.dt.float32r
    f16 = mybir.dt.float16
    AF = mybir.ActivationFunctionType
    OP = mybir.AluOpType

    bal = _Balancer(nc)

    cpool = tc.alloc_tile_pool(name="consts", bufs=1)
    yb = cpool.tile([HL, 594], f16, name="yb")
    nc.sync.dma_start(yb[:], yb_d[:])
    xb = cpool.tile([102, 1152], f16, name="xb")
    nc.sync.dma_start(xb[:], xb_d[:])
    ixb = cpool.tile([96, 2 * NWO], f16, name="ixb")
    nc.sync.dma_start(ixb[:], ix_d[:])
    iyb = cpool.tile([NH, NOUT], f16, name="iyb")
    nc.sync.dma_start(iyb[:], iy_d[:])
    c_eps = cpool.tile([128, 1], f32, name="c_eps")
    nc.vector.memset(c_eps[:], EPS)
    c_ln10 = cpool.tile([128, 1], f32, name="c_ln10")
    nc.vector.memset(c_ln10[:], math.log(10.0))
    c_ln100 = cpool.tile([128, 1], f32, name="c_ln100")
    nc.vector.memset(c_ln100[:], math.log(100.0))
    c_eps1e2 = cpool.tile([128, 1], f32, name="c_eps1e2")
    nc.vector.memset(c_eps1e2[:], 100.0 * EPS)
    c_eps1e4 = cpool.tile([128, 1], f32, name="c_eps1e4")
    nc.vector.memset(c_eps1e4[:], 10000.0 * EPS)
    b_eps = c_eps[:NOUT]
    b_ln10 = c_ln10[:NOUT]
    b_ln100 = c_ln100[:NOUT]
    b_eps1e2 = c_eps1e2[:NOUT]
    b_eps1e4 = c_eps1e4[:NOUT]

    spool = tc.alloc_tile_pool(name="smaps", bufs=1)
    S = {}
    for s in range(2):
        for p in range(10):
            S[(s, p)] = spool.tile(
                [96, 2, NH], f16, name=f"S_{s}_{p}", tag=f"S_{s}_{p}"
            )

    xin = tc.alloc_tile_pool(name="xin", bufs=2)
    zpool = tc.alloc_tile_pool(name="zpool", bufs=2)
    lpool = tc.alloc_tile_pool(name="lpool", bufs=8)
    sqpool = tc.alloc_tile_pool(name="sqpool", bufs=6)
    ppool = tc.alloc_tile_pool(name="ppool", bufs=6)
    fpool = tc.alloc_tile_pool(name="fpool", bufs=8)
    zps = tc.alloc_tile_pool(name="zps", bufs=2, space="PSUM")
    xps = tc.alloc_tile_pool(name="xps", bufs=2, space="PSUM")

    def fold_stream(t, s, p, first, eng):
        """t: [96, 2, CBLK, NH] fp16 product tile; fold c into S[(s,p)].
        Whole chain stays on one engine: same-engine program order means
        no cross-engine semaphore hops inside the chain."""
        v = nc.vector if eng == "dve" else nc.gpsimd
        h = CBLK // 2
        f1 = fpool.tile([96, 2, h, NH], f16, name="f1", tag="f1")
        v.tensor_add(f1[:], t[:, :, :h, :], t[:, :, h:, :])
        f2 = fpool.tile([96, 2, h // 2, NH], f16, name="f2", tag="f2")
        v.tensor_add(f2[:], f1[:, :, : h // 2, :], f1[:, :, h // 2 :, :])
        st = S[(s, p)]
        if first:
            v.tensor_add(st[:], f2[:, :, 0, :], f2[:, :, 1, :])
        else:
            f3 = fpool.tile([96, 2, NH], f16, name="f3", tag="f3")
            v.tensor_add(f3[:], f2[:, :, 0, :], f2[:, :, 1, :])
            v.tensor_add(st[:], st[:], f3[:])

    # diag pair index for map m, and cross pairs (p, m1, m2) per sigma
    _DIAG_P = {0: 0, 1: 1, 2: 3, 3: 4, 4: 7, 5: 9}
    _CROSS = [(2, 1, 2), (5, 3, 4), (6, 3, 5), (8, 4, 5)]

    def yconv_steps(blk):
        """Prepare yconv for a channel block: returns (Z, step closures).
        Each step emits one (ci, hf): 2 matmuls + 1 PSUM->SBUF copy."""
        xt = xin.tile([HL, CBLK * W], f16, name="xt", tag="xt")
        nc.sync.dma_start(xt[:], x_d[:, blk * CBLK * W : (blk + 1) * CBLK * W])
        Z = {}
        for hf in range(2):
            Z[hf] = zpool.tile([102, CBLK, 594], f16, name=f"z{hf}", tag=f"z{hf}")

        def make_step(ci, hf):
            def step():
                w0 = 0 if hf == 0 else 90
                lhs = xt[:, ci * W + w0 : ci * W + w0 + 102]
                zp = zps.tile([102, 2, 512], f32, name="zp", tag="zp")
                for s in range(2):
                    nc.tensor.matmul(
                        zp[:, s, :297],
                        lhs,
                        yb[:, s * 297 : (s + 1) * 297],
                        start=True,
                        stop=True,
                    )
                bal.copy(
                    Z[hf][:, ci, :].rearrange("p (s h) -> p s h", s=2),
                    zp[:, :, :297],
                    kind="copy_ps",
                )
            return step

        steps = [make_step(ci, hf) for ci in range(CBLK) for hf in range(2)]
        return Z, steps

    def gram_blk(blk, Z, ysteps):
        """Emit xconv + products + folds for one channel block,
        interleaving next block's yconv steps to keep PE flowing."""
        ysteps = list(ysteps)
        rows_t = 2 * CBLK * NH
        for s in range(2):
            L = {}
            SQ = {}
            diag_mode = {}
            for m in range(6):
                if ysteps:
                    ysteps.pop(0)()
                fy, fx = _MAP_FYFX[m]
                need_plain = m >= 1
                sq = sqpool.tile([96, 2, CBLK, NH], f16, name=f"sq{m}", tag="sq")
                SQ[m] = sq
                if need_plain:
                    lt = lpool.tile([96, 2, CBLK, NH], f16, name=f"l{m}", tag="lt")
                    L[m] = lt
                    eng = bal.pick_unit(2 * rows_t)
                    diag_mode[m] = (eng, eng)
                else:
                    diag_mode[m] = ("act", None)
                for hf in range(2):
                    xb_col = (s * 2 + hf) * 3 + fx
                    lhsT = xb[:, xb_col * 96 : (xb_col + 1) * 96]
                    xp = xps.tile([96, 2, 512], f32, name="xp", tag="xp")
                    for cs in range(2):
                        rhs = Z[hf][
                            :, cs * CSUB : (cs + 1) * CSUB,
                            s * 297 + fy * NH : s * 297 + (fy + 1) * NH,
                        ]
                        nc.tensor.matmul(
                            xp[:, cs, : CSUB * NH], lhsT, rhs, start=True, stop=True
                        )
                    srcap = xp[:, :, : CSUB * NH].rearrange(
                        "p b (c h) -> p b c h", c=CSUB
                    )
                    sqdst = sq[:, hf].rearrange("p (b c) h -> p b c h", b=2)
                    if need_plain:
                        dst = lt[:, hf].rearrange("p (b c) h -> p b c h", b=2)
                        bal.copy(dst, srcap, kind="copy_ps")
                    else:
                        # m=0: PSUM square must ride Act (single-src)
                        nc.scalar.activation(
                            sqdst, srcap, mybir.ActivationFunctionType.Square,
                            bias=0.0, scale=1.0,
                        )
            # products + folds, both halves at once; each product+fold
            # chain is pinned to one engine (picked by projected load)
            for m in range(6):
                if ysteps and len(ysteps) > (11 - (s * 6 + m)):
                    ysteps.pop(0)()
                sq = SQ[m]
                sqeng, feng = diag_mode[m]
                if m == 0:
                    eng = bal.pick_unit(rows_t)
                else:
                    eng = feng
                    af = L[m][:].rearrange("p f c h -> p (f c h)")
                    sqf = sq[:].rearrange("p f c h -> p (f c h)")
                    if sqeng == "dve":
                        nc.vector.tensor_mul(sqf, af, af)
                    else:
                        nc.gpsimd.tensor_mul(sqf, af, af)
                fold_stream(sq[:], s, _DIAG_P[m], blk == 0, eng)
                for p, m1, m2 in _CROSS:
                    if m2 != m:
                        continue
                    eng = bal.pick_unit(2 * rows_t)
                    pt = ppool.tile([96, 2, CBLK, NH], f16, name="P", tag="P")
                    a1 = L[m1][:].rearrange("p f c h -> p (f c h)")
                    a2 = L[m2][:].rearrange("p f c h -> p (f c h)")
                    ptf = pt[:].rearrange("p f c h -> p (f c h)")
                    if eng == "dve":
                        nc.vector.tensor_mul(ptf, a1, a2)
                    else:
                        nc.gpsimd.tensor_mul(ptf, a1, a2)
                    fold_stream(pt[:], s, p, blk == 0, eng)
        for st in ysteps:
            st()

    # software pipeline: blk 0's yconv runs up front; thereafter blk+1's
    # yconv steps are interleaved into gram(blk)
    Zcur, steps0 = yconv_steps(0)
    for st in steps0:
        st()
    for blk in range(NBLK):
        if blk + 1 < NBLK:
            Znext, ysteps = yconv_steps(blk + 1)
        else:
            Znext, ysteps = None, []
        gram_blk(blk, Zcur, ysteps)
        Zcur = Znext

    for pool in (xps, zps, fpool, ppool, sqpool, lpool, zpool, xin):
        pool.release()

    # ---- integration -> SM maps [NOUT, NWO] (fp32)
    impool = tc.alloc_tile_pool(name="impool", bufs=1)
    SM = {}
    for s in range(2):
        for p in range(10):
            SM[(s, p)] = impool.tile(
                [NOUT, NWO], f32, name=f"SM_{s}_{p}", tag=f"SM_{s}_{p}"
            )
    sxp = tc.alloc_tile_pool(name="sxp", bufs=3)
    ips = tc.alloc_tile_pool(name="ips", bufs=2, space="PSUM")
    yps = tc.alloc_tile_pool(name="yps", bufs=2, space="PSUM")
    for s in range(2):
        for p in range(10):
            ip = ips.tile([NH, NWO], f32, name="ip", tag="ip")
            nc.tensor.matmul(
                ip[:], S[(s, p)][:, 0, :], ixb[:, :NWO], start=True, stop=False
            )
            nc.tensor.matmul(
                ip[:], S[(s, p)][:, 1, :], ixb[:, NWO:], start=False, stop=True
            )
            sx = sxp.tile([NH, NWO], f16, name="sx", tag="sx")
            nc.scalar.activation(sx[:], ip[:], AF.Copy, bias=0.0, scale=W_PAIR[p])
            yp = yps.tile([NOUT, NWO], f32, name="yp", tag="yp")
            nc.tensor.matmul(yp[:], iyb[:], sx[:], start=True, stop=True)
            bal.copy(SM[(s, p)][:], yp[:], kind="copy_ps")
    for pool in (yps, ips, sxp):
        pool.release()

    # ---- ESP + output
    opool = tc.alloc_tile_pool(name="opool", bufs=1)
    OUTT = opool.tile([NOUT, NWO * 12], f32, name="OUTT")
    outv = OUTT[:].rearrange("p (w c) -> p w c", c=12)
    epool = tc.alloc_tile_pool(name="epool", bufs=2)

    def et(name):
        return epool.tile([NOUT, NWO], f32, name=name, tag=name)

    roots = []  # (av_tile, ln_scale, exp_bias, exp_scale, out_ch)

    for s in range(2):
        ch0 = s * 6
        # ---- j = 0
        m0 = SM[(s, 0)]
        t0 = et("t0j0")
        nc.scalar.activation(t0[:], m0[:], AF.Abs)
        nc.vector.tensor_scalar(outv[:, :, ch0 + 0], t0[:], EPS, None, OP.add, OP.bypass)
        # ---- j = 1 : A=1 B=2 D=3
        A, Bm, D = SM[(s, 1)], SM[(s, 2)], SM[(s, 3)]
        p1 = et("p1j1")
        nc.vector.tensor_add(p1[:], A[:], D[:])
        t = et("tj1")
        nc.scalar.activation(t[:], p1[:], AF.Abs)
        nc.vector.tensor_scalar(outv[:, :, ch0 + 1], t[:], EPS, 10.0, OP.add, OP.mult)
        q = et("qj1")
        nc.vector.scalar_tensor_tensor(q[:], p1[:], 1.0, p1[:], OP.mult, OP.mult)
        sA = et("sAj1")
        nc.vector.scalar_tensor_tensor(sA[:], A[:], 1.0, A[:], OP.mult, OP.mult)
        sB2 = et("sBj1")
        nc.vector.scalar_tensor_tensor(sB2[:], Bm[:], 2.0, Bm[:], OP.mult, OP.mult)
        sD = et("sDj1")
        nc.vector.scalar_tensor_tensor(sD[:], D[:], 1.0, D[:], OP.mult, OP.mult)
        p2 = et("p2j1")
        nc.vector.tensor_add(p2[:], sA[:], sB2[:])
        nc.vector.tensor_add(p2[:], p2[:], sD[:])
        v2 = et("v2j1")
        nc.vector.tensor_sub(v2[:], q[:], p2[:])
        av = et("avj1")
        nc.scalar.activation(av[:], v2[:], AF.Abs)
        # 10*(|v2|/2 + eps)^0.5 == sqrt(50*|v2| + 100*eps): one Sqrt op,
        # and Sqrt/Abs/Square/Copy share an act-function table set
        nc.scalar.activation(
            outv[:, :, ch0 + 2], av[:], AF.Sqrt, bias=b_eps1e2, scale=50.0
        )
        # ---- j = 2 : A=4 B=5 C=6 D=7 E=8 F=9
        A, Bm, Cm, D, E, F = (SM[(s, i)] for i in range(4, 10))
        sA, sB, sC, sD, sE, sF = (et(f"s{i}j2") for i in range(6))
        for i, (dst, src) in enumerate(
            ((sA, A), (sB, Bm), (sC, Cm), (sD, D), (sE, E), (sF, F))
        ):
            if i % 2 == 0:
                nc.gpsimd.tensor_mul(dst[:], src[:], src[:])
            else:
                nc.vector.scalar_tensor_tensor(
                    dst[:], src[:], 1.0, src[:], OP.mult, OP.mult
                )
        tAD = et("tADj2")
        nc.vector.tensor_add(tAD[:], A[:], D[:])
        p1 = et("p1j2")
        nc.vector.tensor_add(p1[:], tAD[:], F[:])
        t = et("tj2")
        nc.scalar.activation(t[:], p1[:], AF.Abs)
        nc.vector.tensor_scalar(outv[:, :, ch0 + 3], t[:], EPS, 100.0, OP.add, OP.mult)
        p2 = et("p2j2")
        nc.vector.tensor_add(p2[:], sA[:], sD[:])
        nc.vector.tensor_add(p2[:], p2[:], sF[:])
        u = et("uj2")
        nc.vector.tensor_add(u[:], sB[:], sC[:])
        nc.vector.tensor_add(u[:], u[:], sE[:])
        nc.vector.scalar_tensor_tensor(p2[:], u[:], 2.0, p2[:], OP.mult, OP.add)
        q = et("qj2")
        nc.vector.scalar_tensor_tensor(q[:], p1[:], 1.0, p1[:], OP.mult, OP.mult)
        v2 = et("v2j2")
        nc.vector.tensor_sub(v2[:], q[:], p2[:])
        av = et("avj2")
        nc.scalar.activation(av[:], v2[:], AF.Abs)
        # 100*(|v2|/2 + eps)^0.5 == sqrt(5000*|v2| + 10000*eps)
        nc.scalar.activation(
            outv[:, :, ch0 + 4], av[:], AF.Sqrt, bias=b_eps1e4, scale=5000.0
        )
        # p3 = cubes + 3*(B^2(A+D) + C^2(A+F) + E^2(D+F)) + 6BCE
        cA = et("cAj2")
        nc.vector.scalar_tensor_tensor(cA[:], sA[:], 1.0, A[:], OP.mult, OP.mult)
        cD = et("cDj2")
        nc.vector.scalar_tensor_tensor(cD[:], sD[:], 1.0, D[:], OP.mult, OP.mult)
        cF = et("cFj2")
        nc.vector.scalar_tensor_tensor(cF[:], sF[:], 1.0, F[:], OP.mult, OP.mult)
        w1 = et("w1j2")
        nc.vector.tensor_add(w1[:], cA[:], cD[:])
        nc.vector.tensor_add(w1[:], w1[:], cF[:])
        y1 = et("y1j2")
        nc.vector.scalar_tensor_tensor(y1[:], sB[:], 1.0, tAD[:], OP.mult, OP.mult)
        tAF = et("tAFj2")
        nc.vector.tensor_add(tAF[:], A[:], F[:])
        y2 = et("y2j2")
        nc.vector.scalar_tensor_tensor(y2[:], sC[:], 1.0, tAF[:], OP.mult, OP.mult)
        tDF = et("tDFj2")
        nc.vector.tensor_add(tDF[:], D[:], F[:])
        y3 = et("y3j2")
        nc.vector.scalar_tensor_tensor(y3[:], sE[:], 1.0, tDF[:], OP.mult, OP.mult)
        nc.vector.tensor_add(y1[:], y1[:], y2[:])
        nc.vector.tensor_add(y1[:], y1[:], y3[:])
        z = et("zj2")
        nc.vector.scalar_tensor_tensor(z[:], Bm[:], 6.0, Cm[:], OP.mult, OP.mult)
        nc.vector.scalar_tensor_tensor(z[:], z[:], 1.0, E[:], OP.mult, OP.mult)
        nc.vector.scalar_tensor_tensor(y1[:], y1[:], 3.0, z[:], OP.mult, OP.add)
        p3 = et("p3j2")
        nc.vector.tensor_add(p3[:], w1[:], y1[:])
        # e3*3 = v2/2*p1 - p1*p2 + p3
        a3 = et("a3j2")
        nc.vector.scalar_tensor_tensor(a3[:], v2[:], 0.5, p1[:], OP.mult, OP.mult)
        b3 = et("b3j2")
        nc.vector.scalar_tensor_tensor(b3[:], p1[:], 1.0, p2[:], OP.mult, OP.mult)
        nc.vector.tensor_sub(a3[:], a3[:], b3[:])
        nc.vector.tensor_add(a3[:], a3[:], p3[:])
        av3 = et("av3j2")
        nc.scalar.activation(av3[:], a3[:], AF.Abs)
        roots.append((av3, 1.0 / 3.0, b_ln100, 1.0 / 3.0, ch0 + 5))

    # bias gate: a tiny chain reading both |a3| tiles so the batched Ln ops
    # cannot be scheduled (and force an act-table reload) before the last
    # Abs of either sigma has run
    gate = epool.tile([NOUT, 1], f32, name="gate", tag="gate")
    nc.vector.scalar_tensor_tensor(
        gate[:], roots[0][0][:, 0:1], 0.0, roots[1][0][:, 0:1], OP.mult, OP.mult
    )
    nc.vector.scalar_tensor_tensor(
        gate[:], gate[:], 1.0, c_eps[:NOUT], OP.mult, OP.add
    )
    b_eps_gated = gate

    # batched by activation function so the Act engine loads each
    # function table once instead of ping-ponging Ln/Exp per root
    lgs = []
    for i, (av, lns, ebias, escale, ch) in enumerate(roots):
        lg = et(f"lg{i}")
        nc.scalar.activation(lg[:], av[:], AF.Ln, bias=b_eps_gated, scale=lns)
        lgs.append(lg)
    for i, (av, lns, ebias, escale, ch) in enumerate(roots):
        nc.scalar.activation(outv[:, :, ch], lgs[i][:], AF.Exp, bias=ebias, scale=escale)

    nc.sync.dma_start(out_d[:], OUTT[:])
    for pool in (epool, opool, impool, spool, cpool):
        pool.release()


def _get_module():
    key = CONV_MODE
    if key not in _CACHE:
        _CACHE[key] = _build_module()
    return _CACHE[key]


# ---------------------------------------------------------------- entry point
def kernel(inputs, kernels0, kernels1, dg_int):
    from concourse.bass_utils import run_bass_kernel_spmd

    in_maps = _make_in_maps(inputs, kernels0, kernels1, dg_int)
    nc = _get_module()
    res = run_bass_kernel_spmd(nc, in_maps, core_ids=list(range(8)), **RUN_KWARGS)
    global LAST
    LAST = res
    out = np.empty((B, NWO, NWO, 12), dtype=np.float32)
    for core in range(8):
        b, half = core // 2, core % 2
        H0 = half * NOUT
        out[b, H0 : H0 + NOUT] = res.results[core]["out"].reshape(NOUT, NWO, 12)
    return out


# revision 58
# speedup vs baseline: 1.0175x; 1.0050x over previous
"""Trainium2 Bass kernel for nn_GaussianDerivativeESPLayer.

Strategy (per the data-parallel hint, extended since B=4 < 8 cores):
shard (batch b, H-half) across the 8 cores. Each core computes output
rows [H0, H0+93) of one batch element from input rows [g0, g0+105).

Performance design (512us -> 301us in the TimelineSim cost model):
  - fp16 conv matmuls (1 cyc/row on PE vs 4 for fp32); integration
    matmuls fp16 as well.
  - j0 diagonal Gram entry fused: Act-engine Square reads conv PSUM and
    writes the squared map to SBUF fp16 in one instruction (copy+mul).
  - Cross products / squares / channel-fold adds run in fp16 (DVE 2x
    perf mode). Each product+fold chain is pinned to one engine
    (DVE or Pool, greedy by projected load) so the chain rides implicit
    same-engine ordering instead of cross-engine semaphores. PSUM->SBUF
    copies are balanced between Act and DVE (Pool cannot access PSUM;
    TensorTensor cannot take two PSUM sources; DMA cannot read PSUM).
  - hf-merged tiles [96, 2, c, 99] and 2-PSUM-bank batched copies halve
    instruction counts (Act pays ~185ns access-init per instruction).
  - Software pipelining: the next channel block's yconv matmuls+copies
    are interleaved into the current block's xconv/product emission so
    parked matmuls never head-of-line block the PE sequencer.
  - ESP stage in fp32 (Newton-identity cancellations), with Ln/Exp
    calls batched by function to avoid Act table reloads; the
    sqrt-binomial pair weights fold into the integration scale copy.

Per-core pipeline (all layouts [partition, free]):
  1. yconv: data-stationary matmuls X^T @ yband -> Z [w, (s, fy, c, h')]
  2. xconv: band-stationary matmuls xband^T @ Z -> L [w', (c, h')] PSUM
  3. Gram: Square/mult + fold-tree over 64 channels -> 20 S maps [96, 99]
  4. Integration: valid 7x7 separable conv as two matmul stages
  5. ESP: Newton-identity polynomials per pixel, roots via exp/ln.
"""

import math

import numpy as np

B, H, W, C = 4, 192, 192, 64
NH = 99  # h' rows computed per core (pre y-integration)
NOUT = 93  # h'' output rows per core
HL = 105  # input rows per core
NWO = 186  # output cols
CBLK = 8  # channels per block
NBLK = C // CBLK
CSUB = 4  # channels per xconv matmul chunk
RI = 3  # integrator radius
EPS = float(np.finfo(np.float64).eps)

CONV_MODE = "fp16"

_CACHE = {}
RUN_KWARGS = {}  # test harness can set dict(trace=True) before calling kernel()
LAST = None  # BassKernelResults of the most recent kernel() call


# ---------------------------------------------------------------- host math
def _extract_filters(kernels):
    """kernels [6, kh, kw, C, 1] channel-tiled separable. Returns
    (gys, gxs): 1D filters with sigma^order folded in, such that
    kernel(j,k) == outer(gys[j-k], gxs[k])."""
    K = np.asarray(kernels, dtype=np.float64)[:, :, :, 0, 0]
    i0 = K.shape[1] // 2
    s0 = math.sqrt(abs(K[0][i0, i0]))
    g0y = K[0][:, i0] / s0
    g0x = K[0][i0, :] / s0
    g1x = K[2][i0, :] / g0y[i0]  # (j=1,k=1) = s*g1x (x) * g0y (y)
    g1y = K[1][:, i0] / g0x[i0]  # (j=1,k=0) = s*g0x (x) * g1y (y)
    g2x = K[5][i0, :] / g0y[i0]
    g2y = K[3][:, i0] / g0x[i0]
    return [g0y, g1y, g2y], [g0x, g1x, g2x]


def _extract_integrator(dg_int):
    K = np.asarray(dg_int, dtype=np.float64)[:, :, 0, 0]
    i0 = K.shape[0] // 2
    s0 = math.sqrt(abs(K[i0, i0]))
    return K[:, i0] / s0, K[i0, :] / s0  # giy, gix


def _band(k_count, m_count, g, delta):
    """band[k, m] = g[k - m + delta + r] when |k - m + delta| <= r."""
    r = len(g) // 2
    k = np.arange(k_count)[:, None]
    m = np.arange(m_count)[None, :]
    d = k - m + delta
    ok = np.abs(d) <= r
    out = np.zeros((k_count, m_count), dtype=np.float64)
    out[ok] = np.asarray(g)[(d + r)[ok]]
    return out


def _build_host_tensors(kernels0, kernels1, dg_int):
    gys0, gxs0 = _extract_filters(kernels0)
    gys1, gxs1 = _extract_filters(kernels1)
    giy, gix = _extract_integrator(dg_int)
    gys = [gys0, gys1]
    gxs = [gxs0, gxs1]

    # yband per half: [HL, 594] cols = (sigma, fy, h'-local)
    ybands = []
    for half in range(2):
        H0 = half * NOUT
        g0 = 0 if half == 0 else 87
        cols = []
        for s in range(2):
            for fy in range(3):
                cols.append(_band(HL, NH, gys[s][fy], g0 - H0))
        ybands.append(np.concatenate(cols, axis=1).astype(np.float16))

    # xband: [102, 1152] cols = (sigma, half, fx, 96)  -- core-independent
    xcols = []
    for s in range(2):
        for hf in range(2):
            delta = 0 if hf == 0 else (90 - 96)
            for fx in range(3):
                xcols.append(_band(102, 96, gxs[s][fx], delta))
    xband = np.concatenate(xcols, axis=1).astype(np.float16)

    # x-integration bands [96, 372]: valid conv, out w'' n: sum_k S[w'=k+96*hf]*gix[w'-n]
    ix = []
    for hf in range(2):
        k = np.arange(96)[:, None] + 96 * hf
        n = np.arange(NWO)[None, :]
        d = k - n
        ok = (d >= 0) & (d <= 2 * RI)
        b = np.zeros((96, NWO))
        b[ok] = gix[d[ok]]
        ix.append(b)
    intx = np.concatenate(ix, axis=1).astype(np.float16)

    # y-integration band [NH, NOUT]: inty[k, m] = giy[k - m], 0 <= k-m <= 6
    k = np.arange(NH)[:, None]
    m = np.arange(NOUT)[None, :]
    d = k - m
    ok = (d >= 0) & (d <= 2 * RI)
    inty = np.zeros((NH, NOUT))
    inty[ok] = giy[d[ok]]
    inty = inty.astype(np.float16)

    return ybands, xband, intx, inty


def _make_in_maps(inputs, kernels0, kernels1, dg_int):
    x = np.asarray(inputs, dtype=np.float16)
    ybands, xband, intx, inty = _build_host_tensors(kernels0, kernels1, dg_int)
    in_maps = []
    for core in range(8):
        b, half = core // 2, core % 2
        g0 = 0 if half == 0 else 87
        xc = np.ascontiguousarray(
            x[b, g0 : g0 + HL].transpose(0, 2, 1).reshape(HL, C * W)
        )
        in_maps.append(
            {
                "x": xc,
                "yband": ybands[half],
                "xband": xband,
                "intx": intx,
                "inty": inty,
            }
        )
    return in_maps


# pairs per sigma: (m1, m2, weight, diag); m = map id of (j, k):
# (0,0),(1,0),(1,1),(2,0),(2,1),(2,2)
_LMAP = {(0, 0): 0, (1, 0): 1, (1, 1): 2, (2, 0): 3, (2, 1): 4, (2, 2): 5}
_MAP_FYFX = {0: (0, 0), 1: (1, 0), 2: (0, 1), 3: (2, 0), 4: (1, 1), 5: (0, 2)}


def _pair_list():
    pairs = []
    for j in range(3):
        for k1 in range(j + 1):
            for k2 in range(k1, j + 1):
                w = math.sqrt(math.comb(j, k1) * math.comb(j, k2))
                pairs.append((_LMAP[(j, k1)], _LMAP[(j, k2)], w, k1 == k2))
    return pairs  # 10 per sigma


_PAIRS = _pair_list()
W_PAIR = [w for (_, _, w, _) in _PAIRS]
# S indices per sigma: j0: [0]; j1: A=1 B=2 D=3 ; j2: A=4 B=5 C=6 D=7 E=8 F=9


# ---------------------------------------------------------------- bass build
def _build_module():
    import concourse.bacc as bacc
    import concourse.mybir as mybir
    import concourse.tile as tile

    f32 = mybir.dt.float32
    f32r = mybir.dt.float32r
    f16 = mybir.dt.float16

    nc = bacc.Bacc("TRN2", target_bir_lowering=False, debug=False, num_devices=8)
    x_d = nc.dram_tensor("x", [HL, C * W], f16, kind="ExternalInput").ap()
    yb_d = nc.dram_tensor("yband", [HL, 594], f16, kind="ExternalInput").ap()
    xb_d = nc.dram_tensor("xband", [102, 1152], f16, kind="ExternalInput").ap()
    ix_d = nc.dram_tensor("intx", [96, 2 * NWO], f16, kind="ExternalInput").ap()
    iy_d = nc.dram_tensor("inty", [NH, NOUT], f16, kind="ExternalInput").ap()
    out_d = nc.dram_tensor("out", [NOUT, NWO * 12], f32, kind="ExternalOutput").ap()

    with tile.TileContext(nc) as tc:
        _emit(tc, nc, x_d, yb_d, xb_d, ix_d, iy_d, out_d)
    nc.compile()
    return nc


class _Balancer:
    """Greedy engine load balancer with static per-instruction cost
    estimates (ns). Tracks projected busy time per engine."""

    # (kind) -> {engine: (ns_per_row, fixed_ns)} -- calibrated to the
    # TimelineSim cost model (Act pays ~185ns SBUF-access init per instr).
    COSTS = {
        "copy_ps": {"act": (0.833, 185.0), "dve": (1.35, 120.0)},
        "sq_ps": {"act": (0.833, 185.0)},
        "sq16": {"act": (0.833, 185.0), "dve": (0.52, 60.0), "pool": (1.98, 140.0)},
        "mul16": {"dve": (0.52, 60.0), "pool": (1.98, 140.0)},
        "add16": {"dve": (0.52, 60.0), "pool": (1.98, 140.0)},
        "add32": {"dve": (1.04, 60.0), "pool": (1.98, 140.0)},
        "copy32": {"act": (0.833, 185.0), "dve": (1.04, 60.0)},
        "scale_copy": {"act": (0.833, 190.0), "dve": (1.04, 125.0)},
    }

    def __init__(self, nc):
        self.nc = nc
        # pre-charge with the hardcoded work emitted outside the balancer
        # (ESP chains on DVE, sx/SM/ESP-act + act table loads, ESP pool ops)
        self.load = {"act": 0.0, "dve": 0.0, "pool": 0.0}

    def _pick(self, kind, rows, allowed=None):
        tbl = self.COSTS[kind]
        best, best_t = None, None
        for eng, (per, fix) in tbl.items():
            if allowed and eng not in allowed:
                continue
            t = self.load[eng] + rows * per + fix
            if best_t is None or t < best_t:
                best, best_t = eng, t
        per, fix = tbl[best]
        self.load[best] += rows * per + fix
        return best

    DVE_RATE, DVE_FIX = 0.52, 120.0
    POOL_RATE, POOL_FIX = 1.98, 280.0

    def pick_unit(self, rows):
        """Pick dve/pool for a whole product+fold chain and commit its cost."""
        td = self.load["dve"] + rows * self.DVE_RATE + self.DVE_FIX
        tp = self.load["pool"] + rows * self.POOL_RATE + self.POOL_FIX
        if td <= tp:
            self.load["dve"] = td
            return "dve"
        self.load["pool"] = tp
        return "pool"

    def pick_diag(self, sq_rows, fold_rows):
        """For a diagonal stream: square+fold on dve/pool, or the square
        fused into an Act PSUM-read with the fold elsewhere."""
        t_unit_d = self.load["dve"] + (sq_rows + fold_rows) * 0.52 + 120.0
        t_unit_p = self.load["pool"] + (sq_rows + fold_rows) * 1.98 + 280.0
        act_sq = sq_rows * 0.833 + 2 * 210.0  # two 2-bank sq_ps instrs
        t_act = max(
            self.load["act"] + act_sq,
            min(self.load["dve"] + fold_rows * 0.52 + 120.0,
                self.load["pool"] + fold_rows * 1.98 + 280.0),
        )
        best = min(t_unit_d, t_unit_p, t_act)
        if best == t_unit_d:
            self.load["dve"] = t_unit_d
            return ("unit", "dve")
        if best == t_unit_p:
            self.load["pool"] = t_unit_p
            return ("unit", "pool")
        self.load["act"] += act_sq
        if (self.load["dve"] + fold_rows * 0.52 + 120.0 <=
                self.load["pool"] + fold_rows * 1.98 + 280.0):
            self.load["dve"] += fold_rows * 0.52 + 120.0
            return ("act", "dve")
        self.load["pool"] += fold_rows * 1.98 + 280.0
        return ("act", "pool")

    def copy(self, dst, src, kind="copy_ps", allowed=None):
        rows = src.free_size()
        eng = self._pick(kind, rows, allowed)
        if eng == "act":
            self.nc.scalar.copy(dst, src)
        elif eng == "dve":
            self.nc.vector.tensor_copy(dst, src)
        else:
            self.nc.gpsimd.tensor_copy(dst, src)

    def square(self, dst, src, kind="sq16", allowed=None):
        import concourse.mybir as mybir

        rows = src.free_size()
        eng = self._pick(kind, rows, allowed)
        if eng == "act":
            self.nc.scalar.activation(
                dst, src, mybir.ActivationFunctionType.Square, bias=0.0, scale=1.0
            )
        elif eng == "dve":
            self.nc.vector.tensor_mul(dst, src, src)
        else:
            self.nc.gpsimd.tensor_mul(dst, src, src)

    def mul(self, dst, a, b, kind="mul16", allowed=None):
        eng = self._pick(kind, a.free_size(), allowed)
        if eng == "dve":
            self.nc.vector.tensor_mul(dst, a, b)
        else:
            self.nc.gpsimd.tensor_mul(dst, a, b)

    def add(self, dst, a, b, kind="add16", allowed=None):
        eng = self._pick(kind, a.free_size(), allowed)
        if eng == "dve":
            self.nc.vector.tensor_add(dst, a, b)
        else:
            self.nc.gpsimd.tensor_add(dst, a, b)


def _emit(tc, nc, x_d, yb_d, xb_d, ix_d, iy_d, out_d):
    import concourse.mybir as mybir

    f32 = mybir.dt.float32
    f32r = mybir.dt.float32r
    f16 = mybir.dt.float16
    AF = mybir.ActivationFunctionType
    OP = mybir.AluOpType

    bal = _Balancer(nc)

    cpool = tc.alloc_tile_pool(name="consts", bufs=1)
    yb = cpool.tile([HL, 594], f16, name="yb")
    nc.sync.dma_start(yb[:], yb_d[:])
    xb = cpool.tile([102, 1152], f16, name="xb")
    nc.sync.dma_start(xb[:], xb_d[:])
    ixb = cpool.tile([96, 2 * NWO], f16, name="ixb")
    nc.sync.dma_start(ixb[:], ix_d[:])
    iyb = cpool.tile([NH, NOUT], f16, name="iyb")
    nc.sync.dma_start(iyb[:], iy_d[:])
    c_eps = cpool.tile([128, 1], f32, name="c_eps")
    nc.vector.memset(c_eps[:], EPS)
    c_ln10 = cpool.tile([128, 1], f32, name="c_ln10")
    nc.vector.memset(c_ln10[:], math.log(10.0))
    c_ln100 = cpool.tile([128, 1], f32, name="c_ln100")
    nc.vector.memset(c_ln100[:], math.log(100.0))
    c_eps1e2 = cpool.tile([128, 1], f32, name="c_eps1e2")
    nc.vector.memset(c_eps1e2[:], 100.0 * EPS)
    c_eps1e4 = cpool.tile([128, 1], f32, name="c_eps1e4")
    nc.vector.memset(c_eps1e4[:], 10000.0 * EPS)
    b_eps = c_eps[:NOUT]
    b_ln10 = c_ln10[:NOUT]
    b_ln100 = c_ln100[:NOUT]
    b_eps1e2 = c_eps1e2[:NOUT]
    b_eps1e4 = c_eps1e4[:NOUT]

    spool = tc.alloc_tile_pool(name="smaps", bufs=1)
    S = {}
    for s in range(2):
        for p in range(10):
            S[(s, p)] = spool.tile(
                [96, 2, NH], f16, name=f"S_{s}_{p}", tag=f"S_{s}_{p}"
            )

    xin = tc.alloc_tile_pool(name="xin", bufs=2)
    zpool = tc.alloc_tile_pool(name="zpool", bufs=2)
    lpool = tc.alloc_tile_pool(name="lpool", bufs=8)
    sqpool = tc.alloc_tile_pool(name="sqpool", bufs=6)
    ppool = tc.alloc_tile_pool(name="ppool", bufs=6)
    fpool = tc.alloc_tile_pool(name="fpool", bufs=8)
    zps = tc.alloc_tile_pool(name="zps", bufs=2, space="PSUM")
    xps = tc.alloc_tile_pool(name="xps", bufs=2, space="PSUM")

    def fold_stream(t, s, p, first, eng):
        """t: [96, 2, CBLK, NH] fp16 product tile; fold c into S[(s,p)].
        Whole chain stays on one engine: same-engine program order means
        no cross-engine semaphore hops inside the chain."""
        v = nc.vector if eng == "dve" else nc.gpsimd
        h = CBLK // 2
        f1 = fpool.tile([96, 2, h, NH], f16, name="f1", tag="f1")
        v.tensor_add(f1[:], t[:, :, :h, :], t[:, :, h:, :])
        f2 = fpool.tile([96, 2, h // 2, NH], f16, name="f2", tag="f2")
        v.tensor_add(f2[:], f1[:, :, : h // 2, :], f1[:, :, h // 2 :, :])
        st = S[(s, p)]
        if first:
            v.tensor_add(st[:], f2[:, :, 0, :], f2[:, :, 1, :])
        else:
            f3 = fpool.tile([96, 2, NH], f16, name="f3", tag="f3")
            v.tensor_add(f3[:], f2[:, :, 0, :], f2[:, :, 1, :])
            v.tensor_add(st[:], st[:], f3[:])

    # diag pair index for map m, and cross pairs (p, m1, m2) per sigma
    _DIAG_P = {0: 0, 1: 1, 2: 3, 3: 4, 4: 7, 5: 9}
    _CROSS = [(2, 1, 2), (5, 3, 4), (6, 3, 5), (8, 4, 5)]

    def yconv_steps(blk):
        """Prepare yconv for a channel block: returns (Z, step closures).
        Each step emits one (ci, hf): 2 matmuls + 1 PSUM->SBUF copy."""
        xt = xin.tile([HL, CBLK * W], f16, name="xt", tag="xt")
        nc.sync.dma_start(xt[:], x_d[:, blk * CBLK * W : (blk + 1) * CBLK * W])
        Z = {}
        for hf in range(2):
            Z[hf] = zpool.tile([102, CBLK, 594], f16, name=f"z{hf}", tag=f"z{hf}")

        def make_step(ci, hf):
            def step():
                w0 = 0 if hf == 0 else 90
                lhs = xt[:, ci * W + w0 : ci * W + w0 + 102]
                zp = zps.tile([102, 2, 512], f32, name="zp", tag="zp")
                for s in range(2):
                    nc.tensor.matmul(
                        zp[:, s, :297],
                        lhs,
                        yb[:, s * 297 : (s + 1) * 297],
                        start=True,
                        stop=True,
                    )
                bal.copy(
                    Z[hf][:, ci, :].rearrange("p (s h) -> p s h", s=2),
                    zp[:, :, :297],
                    kind="copy_ps",
                )
            return step

        steps = [make_step(ci, hf) for ci in range(CBLK) for hf in range(2)]
        return Z, steps

    def gram_blk(blk, Z, ysteps):
        """Emit xconv + products + folds for one channel block,
        interleaving next block's yconv steps to keep PE flowing."""
        ysteps = list(ysteps)
        rows_t = 2 * CBLK * NH
        for s in range(2):
            L = {}
            SQ = {}
            diag_mode = {}
            for m in range(6):
                if ysteps:
                    ysteps.pop(0)()
                fy, fx = _MAP_FYFX[m]
                need_plain = m >= 1
                sq = sqpool.tile([96, 2, CBLK, NH], f16, name=f"sq{m}", tag="sq")
                SQ[m] = sq
                if need_plain:
                    lt = lpool.tile([96, 2, CBLK, NH], f16, name=f"l{m}", tag="lt")
                    L[m] = lt
                    eng = bal.pick_unit(2 * rows_t)
                    diag_mode[m] = (eng, eng)
                else:
                    diag_mode[m] = ("act", None)
                for hf in range(2):
                    xb_col = (s * 2 + hf) * 3 + fx
                    lhsT = xb[:, xb_col * 96 : (xb_col + 1) * 96]
                    xp = xps.tile([96, 2, 512], f32, name="xp", tag="xp")
                    for cs in range(2):
                        rhs = Z[hf][
                            :, cs * CSUB : (cs + 1) * CSUB,
                            s * 297 + fy * NH : s * 297 + (fy + 1) * NH,
                        ]
                        nc.tensor.matmul(
                            xp[:, cs, : CSUB * NH], lhsT, rhs, start=True, stop=True
                        )
                    srcap = xp[:, :, : CSUB * NH].rearrange(
                        "p b (c h) -> p b c h", c=CSUB
                    )
                    sqdst = sq[:, hf].rearrange("p (b c) h -> p b c h", b=2)
                    if need_plain:
                        dst = lt[:, hf].rearrange("p (b c) h -> p b c h", b=2)
                        bal.copy(dst, srcap, kind="copy_ps")
                    else:
                        # m=0: PSUM square must ride Act (single-src)
                        nc.scalar.activation(
                            sqdst, srcap, mybir.ActivationFunctionType.Square,
                            bias=0.0, scale=1.0,
                        )
            # products + folds, both halves at once; each product+fold
            # chain is pinned to one engine (picked by projected load)
            for m in range(6):
                if ysteps and len(ysteps) > (11 - (s * 6 + m)):
                    ysteps.pop(0)()
                sq = SQ[m]
                sqeng, feng = diag_mode[m]
                if m == 0:
                    eng = bal.pick_unit(rows_t)
                else:
                    eng = feng
                    af = L[m][:].rearrange("p f c h -> p (f c h)")
                    sqf = sq[:].rearrange("p f c h -> p (f c h)")
                    if sqeng == "dve":
                        nc.vector.tensor_mul(sqf, af, af)
                    else:
                        nc.gpsimd.tensor_mul(sqf, af, af)
                fold_stream(sq[:], s, _DIAG_P[m], blk == 0, eng)
                for p, m1, m2 in _CROSS:
                    if m2 != m:
                        continue
                    eng = bal.pick_unit(2 * rows_t)
                    pt = ppool.tile([96, 2, CBLK, NH], f16, name="P", tag="P")
                    a1 = L[m1][:].rearrange("p f c h -> p (f c h)")
                    a2 = L[m2][:].rearrange("p f c h -> p (f c h)")
                    ptf = pt[:].rearrange("p f c h -> p (f c h)")
                    if eng == "dve":
                        nc.vector.tensor_mul(ptf, a1, a2)
                    else:
                        nc.gpsimd.tensor_mul(ptf, a1, a2)
                    fold_stream(pt[:], s, p, blk == 0, eng)
        for st in ysteps:
            st()

    # software pipeline: blk 0's yconv runs up front; thereafter blk+1's
    # yconv steps are interleaved into gram(blk)
    Zcur, steps0 = yconv_steps(0)
    for st in steps0:
        st()
    for blk in range(NBLK):
        if blk + 1 < NBLK:
            Znext, ysteps = yconv_steps(blk + 1)
        else:
            Znext, ysteps = None, []
        gram_blk(blk, Zcur, ysteps)
        Zcur = Znext

    for pool in (xps, zps, fpool, ppool, sqpool, lpool, zpool, xin):
        pool.release()

    # ---- integration -> SM maps [NOUT, NWO] (fp32)
    impool = tc.alloc_tile_pool(name="impool", bufs=1)
    SM = {}
    for s in range(2):
        for p in range(10):
            SM[(s, p)] = impool.tile(
                [NOUT, NWO], f32, name=f"SM_{s}_{p}", tag=f"SM_{s}_{p}"
            )
    sxp = tc.alloc_tile_pool(name="sxp", bufs=3)
    ips = tc.alloc_tile_pool(name="ips", bufs=2, space="PSUM")
    yps = tc.alloc_tile_pool(name="yps", bufs=2, space="PSUM")
    for s in range(2):
        for p in range(10):
            ip = ips.tile([NH, NWO], f32, name="ip", tag="ip")
            nc.tensor.matmul(
                ip[:], S[(s, p)][:, 0, :], ixb[:, :NWO], start=True, stop=False
            )
            nc.tensor.matmul(
                ip[:], S[(s, p)][:, 1, :], ixb[:, NWO:], start=False, stop=True
            )
            sx = sxp.tile([NH, NWO], f16, name="sx", tag="sx")
            if bal._pick("scale_copy", NWO) == "act":
                nc.scalar.activation(sx[:], ip[:], AF.Copy, bias=0.0, scale=W_PAIR[p])
            else:
                nc.vector.tensor_scalar(
                    sx[:], ip[:], W_PAIR[p], None, OP.mult, OP.bypass
                )
            yp = yps.tile([NOUT, NWO], f32, name="yp", tag="yp")
            nc.tensor.matmul(yp[:], iyb[:], sx[:], start=True, stop=True)
            bal.copy(SM[(s, p)][:], yp[:], kind="copy_ps")
    for pool in (yps, ips, sxp):
        pool.release()

    # ---- ESP + output
    opool = tc.alloc_tile_pool(name="opool", bufs=1)
    OUTT = opool.tile([NOUT, NWO * 12], f32, name="OUTT")
    outv = OUTT[:].rearrange("p (w c) -> p w c", c=12)
    epool = tc.alloc_tile_pool(name="epool", bufs=2)

    def et(name):
        return epool.tile([NOUT, NWO], f32, name=name, tag=name)

    roots = []  # (av_tile, ln_scale, exp_bias, exp_scale, out_ch)

    for s in range(2):
        ch0 = s * 6
        # ---- j = 0
        m0 = SM[(s, 0)]
        t0 = et("t0j0")
        nc.scalar.activation(t0[:], m0[:], AF.Abs)
        nc.vector.tensor_scalar(outv[:, :, ch0 + 0], t0[:], EPS, None, OP.add, OP.bypass)
        # ---- j = 1 : A=1 B=2 D=3
        A, Bm, D = SM[(s, 1)], SM[(s, 2)], SM[(s, 3)]
        p1 = et("p1j1")
        nc.vector.tensor_add(p1[:], A[:], D[:])
        t = et("tj1")
        nc.scalar.activation(t[:], p1[:], AF.Abs)
        nc.vector.tensor_scalar(outv[:, :, ch0 + 1], t[:], EPS, 10.0, OP.add, OP.mult)
        q = et("qj1")
        nc.gpsimd.tensor_mul(q[:], p1[:], p1[:])
        sA = et("sAj1")
        nc.gpsimd.tensor_mul(sA[:], A[:], A[:])
        sB2 = et("sBj1")
        nc.vector.scalar_tensor_tensor(sB2[:], Bm[:], 2.0, Bm[:], OP.mult, OP.mult)
        sD = et("sDj1")
        nc.gpsimd.tensor_mul(sD[:], D[:], D[:])
        p2 = et("p2j1")
        nc.vector.tensor_add(p2[:], sA[:], sB2[:])
        nc.vector.tensor_add(p2[:], p2[:], sD[:])
        v2 = et("v2j1")
        nc.vector.tensor_sub(v2[:], q[:], p2[:])
        av = et("avj1")
        nc.scalar.activation(av[:], v2[:], AF.Abs)
        # 10*(|v2|/2 + eps)^0.5 == sqrt(50*|v2| + 100*eps): one Sqrt op,
        # and Sqrt/Abs/Square/Copy share an act-function table set
        nc.scalar.activation(
            outv[:, :, ch0 + 2], av[:], AF.Sqrt, bias=b_eps1e2, scale=50.0
        )
        # ---- j = 2 : A=4 B=5 C=6 D=7 E=8 F=9
        A, Bm, Cm, D, E, F = (SM[(s, i)] for i in range(4, 10))
        sA, sB, sC, sD, sE, sF = (et(f"s{i}j2") for i in range(6))
        for i, (dst, src) in enumerate(
            ((sA, A), (sB, Bm), (sC, Cm), (sD, D), (sE, E), (sF, F))
        ):
            if i % 2 == 0:
                nc.gpsimd.tensor_mul(dst[:], src[:], src[:])
            else:
                nc.vector.scalar_tensor_tensor(
                    dst[:], src[:], 1.0, src[:], OP.mult, OP.mult
                )
        tAD = et("tADj2")
        nc.vector.tensor_add(tAD[:], A[:], D[:])
        p1 = et("p1j2")
        nc.vector.tensor_add(p1[:], tAD[:], F[:])
        t = et("tj2")
        nc.scalar.activation(t[:], p1[:], AF.Abs)
        nc.vector.tensor_scalar(outv[:, :, ch0 + 3], t[:], EPS, 100.0, OP.add, OP.mult)
        p2 = et("p2j2")
        nc.vector.tensor_add(p2[:], sA[:], sD[:])
        nc.vector.tensor_add(p2[:], p2[:], sF[:])
        u = et("uj2")
        nc.vector.tensor_add(u[:], sB[:], sC[:])
        nc.vector.tensor_add(u[:], u[:], sE[:])
        nc.vector.scalar_tensor_tensor(p2[:], u[:], 2.0, p2[:], OP.mult, OP.add)
        q = et("qj2")
        nc.vector.scalar_tensor_tensor(q[:], p1[:], 1.0, p1[:], OP.mult, OP.mult)
        v2 = et("v2j2")
        nc.vector.tensor_sub(v2[:], q[:], p2[:])
        av = et("avj2")
        nc.scalar.activation(av[:], v2[:], AF.Abs)
        # 100*(|v2|/2 + eps)^0.5 == sqrt(5000*|v2| + 10000*eps)
        nc.scalar.activation(
            outv[:, :, ch0 + 4], av[:], AF.Sqrt, bias=b_eps1e4, scale=5000.0
        )
        # p3 = cubes + 3*(B^2(A+D) + C^2(A+F) + E^2(D+F)) + 6BCE
        cA = et("cAj2")
        nc.vector.scalar_tensor_tensor(cA[:], sA[:], 1.0, A[:], OP.mult, OP.mult)
        cD = et("cDj2")
        nc.vector.scalar_tensor_tensor(cD[:], sD[:], 1.0, D[:], OP.mult, OP.mult)
        cF = et("cFj2")
        nc.vector.scalar_tensor_tensor(cF[:], sF[:], 1.0, F[:], OP.mult, OP.mult)
        w1 = et("w1j2")
        nc.vector.tensor_add(w1[:], cA[:], cD[:])
        nc.vector.tensor_add(w1[:], w1[:], cF[:])
        y1 = et("y1j2")
        nc.vector.scalar_tensor_tensor(y1[:], sB[:], 1.0, tAD[:], OP.mult, OP.mult)
        tAF = et("tAFj2")
        nc.vector.tensor_add(tAF[:], A[:], F[:])
        y2 = et("y2j2")
        nc.vector.scalar_tensor_tensor(y2[:], sC[:], 1.0, tAF[:], OP.mult, OP.mult)
        tDF = et("tDFj2")
        nc.vector.tensor_add(tDF[:], D[:], F[:])
        y3 = et("y3j2")
        nc.vector.scalar_tensor_tensor(y3[:], sE[:], 1.0, tDF[:], OP.mult, OP.mult)
        nc.vector.tensor_add(y1[:], y1[:], y2[:])
        nc.vector.tensor_add(y1[:], y1[:], y3[:])
        z = et("zj2")
        nc.vector.scalar_tensor_tensor(z[:], Bm[:], 6.0, Cm[:], OP.mult, OP.mult)
        nc.vector.scalar_tensor_tensor(z[:], z[:], 1.0, E[:], OP.mult, OP.mult)
        nc.vector.scalar_tensor_tensor(y1[:], y1[:], 3.0, z[:], OP.mult, OP.add)
        p3 = et("p3j2")
        nc.vector.tensor_add(p3[:], w1[:], y1[:])
        # e3*3 = v2/2*p1 - p1*p2 + p3
        a3 = et("a3j2")
        nc.vector.scalar_tensor_tensor(a3[:], v2[:], 0.5, p1[:], OP.mult, OP.mult)
        b3 = et("b3j2")
        nc.vector.scalar_tensor_tensor(b3[:], p1[:], 1.0, p2[:], OP.mult, OP.mult)
        nc.vector.tensor_sub(a3[:], a3[:], b3[:])
        nc.vector.tensor_add(a3[:], a3[:], p3[:])
        av3 = et("av3j2")
        nc.scalar.activation(av3[:], a3[:], AF.Abs)
        roots.append((av3, 1.0 / 3.0, b_ln100, 1.0 / 3.0, ch0 + 5))

    # bias gate: a tiny chain reading both |a3| tiles so the batched Ln ops
    # cannot be scheduled (and force an act-table reload) before the last
    # Abs of either sigma has run
    gate = epool.tile([NOUT, 1], f32, name="gate", tag="gate")
    nc.vector.scalar_tensor_tensor(
        gate[:], roots[0][0][:, 0:1], 0.0, roots[1][0][:, 0:1], OP.mult, OP.mult
    )
    nc.vector.scalar_tensor_tensor(
        gate[:], gate[:], 1.0, c_eps[:NOUT], OP.mult, OP.add
    )
    b_eps_gated = gate

    # batched by activation function so the Act engine loads each
    # function table once instead of ping-ponging Ln/Exp per root
    lgs = []
    for i, (av, lns, ebias, escale, ch) in enumerate(roots):
        lg = et(f"lg{i}")
        nc.scalar.activation(lg[:], av[:], AF.Ln, bias=b_eps_gated, scale=lns)
        lgs.append(lg)
    for i, (av, lns, ebias, escale, ch) in enumerate(roots):
        nc.scalar.activation(outv[:, :, ch], lgs[i][:], AF.Exp, bias=ebias, scale=escale)

    nc.sync.dma_start(out_d[:], OUTT[:])
    for pool in (epool, opool, impool, spool, cpool):
        pool.release()


def _get_module():
    key = CONV_MODE
    if key not in _CACHE:
        _CACHE[key] = _build_module()
    return _CACHE[key]


# ---------------------------------------------------------------- entry point
def kernel(inputs, kernels0, kernels1, dg_int):
    from concourse.bass_utils import run_bass_kernel_spmd

    in_maps = _make_in_maps(inputs, kernels0, kernels1, dg_int)
    nc = _get_module()
    res = run_bass_kernel_spmd(nc, in_maps, core_ids=list(range(8)), **RUN_KWARGS)
    global LAST
    LAST = res
    out = np.empty((B, NWO, NWO, 12), dtype=np.float32)
    for core in range(8):
        b, half = core // 2, core % 2
        H0 = half * NOUT
        out[b, H0 : H0 + NOUT] = res.results[core]["out"].reshape(NOUT, NWO, 12)
    return out


# revision 61
# speedup vs baseline: 1.0208x; 1.0033x over previous
"""Trainium2 Bass kernel for nn_GaussianDerivativeESPLayer.

Strategy (per the data-parallel hint, extended since B=4 < 8 cores):
shard (batch b, H-half) across the 8 cores. Each core computes output
rows [H0, H0+93) of one batch element from input rows [g0, g0+105).

Performance design (512us -> 301us in the TimelineSim cost model):
  - fp16 conv matmuls (1 cyc/row on PE vs 4 for fp32); integration
    matmuls fp16 as well.
  - j0 diagonal Gram entry fused: Act-engine Square reads conv PSUM and
    writes the squared map to SBUF fp16 in one instruction (copy+mul).
  - Cross products / squares / channel-fold adds run in fp16 (DVE 2x
    perf mode). Each product+fold chain is pinned to one engine
    (DVE or Pool, greedy by projected load) so the chain rides implicit
    same-engine ordering instead of cross-engine semaphores. PSUM->SBUF
    copies are balanced between Act and DVE (Pool cannot access PSUM;
    TensorTensor cannot take two PSUM sources; DMA cannot read PSUM).
  - hf-merged tiles [96, 2, c, 99] and 2-PSUM-bank batched copies halve
    instruction counts (Act pays ~185ns access-init per instruction).
  - Software pipelining: the next channel block's yconv matmuls+copies
    are interleaved into the current block's xconv/product emission so
    parked matmuls never head-of-line block the PE sequencer.
  - ESP stage in fp32 (Newton-identity cancellations), with Ln/Exp
    calls batched by function to avoid Act table reloads; the
    sqrt-binomial pair weights fold into the integration scale copy.

Per-core pipeline (all layouts [partition, free]):
  1. yconv: data-stationary matmuls X^T @ yband -> Z [w, (s, fy, c, h')]
  2. xconv: band-stationary matmuls xband^T @ Z -> L [w', (c, h')] PSUM
  3. Gram: Square/mult + fold-tree over 64 channels -> 20 S maps [96, 99]
  4. Integration: valid 7x7 separable conv as two matmul stages
  5. ESP: Newton-identity polynomials per pixel, roots via exp/ln.
"""

import math

import numpy as np

B, H, W, C = 4, 192, 192, 64
NH = 99  # h' rows computed per core (pre y-integration)
NOUT = 93  # h'' output rows per core
HL = 105  # input rows per core
NWO = 186  # output cols
CBLK = 8  # channels per block
NBLK = C // CBLK
CSUB = 4  # channels per xconv matmul chunk
RI = 3  # integrator radius
EPS = float(np.finfo(np.float64).eps)

CONV_MODE = "fp16"

_CACHE = {}
RUN_KWARGS = {}  # test harness can set dict(trace=True) before calling kernel()
LAST = None  # BassKernelResults of the most recent kernel() call


# ---------------------------------------------------------------- host math
def _extract_filters(kernels):
    """kernels [6, kh, kw, C, 1] channel-tiled separable. Returns
    (gys, gxs): 1D filters with sigma^order folded in, such that
    kernel(j,k) == outer(gys[j-k], gxs[k])."""
    K = np.asarray(kernels, dtype=np.float64)[:, :, :, 0, 0]
    i0 = K.shape[1] // 2
    s0 = math.sqrt(abs(K[0][i0, i0]))
    g0y = K[0][:, i0] / s0
    g0x = K[0][i0, :] / s0
    g1x = K[2][i0, :] / g0y[i0]  # (j=1,k=1) = s*g1x (x) * g0y (y)
    g1y = K[1][:, i0] / g0x[i0]  # (j=1,k=0) = s*g0x (x) * g1y (y)
    g2x = K[5][i0, :] / g0y[i0]
    g2y = K[3][:, i0] / g0x[i0]
    return [g0y, g1y, g2y], [g0x, g1x, g2x]


def _extract_integrator(dg_int):
    K = np.asarray(dg_int, dtype=np.float64)[:, :, 0, 0]
    i0 = K.shape[0] // 2
    s0 = math.sqrt(abs(K[i0, i0]))
    return K[:, i0] / s0, K[i0, :] / s0  # giy, gix


def _band(k_count, m_count, g, delta):
    """band[k, m] = g[k - m + delta + r] when |k - m + delta| <= r."""
    r = len(g) // 2
    k = np.arange(k_count)[:, None]
    m = np.arange(m_count)[None, :]
    d = k - m + delta
    ok = np.abs(d) <= r
    out = np.zeros((k_count, m_count), dtype=np.float64)
    out[ok] = np.asarray(g)[(d + r)[ok]]
    return out


def _build_host_tensors(kernels0, kernels1, dg_int):
    gys0, gxs0 = _extract_filters(kernels0)
    gys1, gxs1 = _extract_filters(kernels1)
    giy, gix = _extract_integrator(dg_int)
    gys = [gys0, gys1]
    gxs = [gxs0, gxs1]

    # yband per half: [HL, 594] cols = (sigma, fy, h'-local)
    ybands = []
    for half in range(2):
        H0 = half * NOUT
        g0 = 0 if half == 0 else 87
        cols = []
        for s in range(2):
            for fy in range(3):
                cols.append(_band(HL, NH, gys[s][fy], g0 - H0))
        ybands.append(np.concatenate(cols, axis=1).astype(np.float16))

    # xband: [102, 1152] cols = (sigma, half, fx, 96)  -- core-independent
    xcols = []
    for s in range(2):
        for hf in range(2):
            delta = 0 if hf == 0 else (90 - 96)
            for fx in range(3):
                xcols.append(_band(102, 96, gxs[s][fx], delta))
    xband = np.concatenate(xcols, axis=1).astype(np.float16)

    # x-integration bands [96, 372]: valid conv, out w'' n: sum_k S[w'=k+96*hf]*gix[w'-n]
    ix = []
    for hf in range(2):
        k = np.arange(96)[:, None] + 96 * hf
        n = np.arange(NWO)[None, :]
        d = k - n
        ok = (d >= 0) & (d <= 2 * RI)
        b = np.zeros((96, NWO))
        b[ok] = gix[d[ok]]
        ix.append(b)
    intx = np.concatenate(ix, axis=1).astype(np.float16)

    # y-integration band [NH, NOUT]: inty[k, m] = giy[k - m], 0 <= k-m <= 6
    k = np.arange(NH)[:, None]
    m = np.arange(NOUT)[None, :]
    d = k - m
    ok = (d >= 0) & (d <= 2 * RI)
    inty = np.zeros((NH, NOUT))
    inty[ok] = giy[d[ok]]
    inty = inty.astype(np.float16)

    return ybands, xband, intx, inty


def _make_in_maps(inputs, kernels0, kernels1, dg_int):
    x = np.asarray(inputs, dtype=np.float16)
    ybands, xband, intx, inty = _build_host_tensors(kernels0, kernels1, dg_int)
    in_maps = []
    for core in range(8):
        b, half = core // 2, core % 2
        g0 = 0 if half == 0 else 87
        xc = np.ascontiguousarray(
            x[b, g0 : g0 + HL].transpose(0, 2, 1).reshape(HL, C * W)
        )
        in_maps.append(
            {
                "x": xc,
                "yband": ybands[half],
                "xband": xband,
                "intx": intx,
                "inty": inty,
            }
        )
    return in_maps


# pairs per sigma: (m1, m2, weight, diag); m = map id of (j, k):
# (0,0),(1,0),(1,1),(2,0),(2,1),(2,2)
_LMAP = {(0, 0): 0, (1, 0): 1, (1, 1): 2, (2, 0): 3, (2, 1): 4, (2, 2): 5}
_MAP_FYFX = {0: (0, 0), 1: (1, 0), 2: (0, 1), 3: (2, 0), 4: (1, 1), 5: (0, 2)}


def _pair_list():
    pairs = []
    for j in range(3):
        for k1 in range(j + 1):
            for k2 in range(k1, j + 1):
                w = math.sqrt(math.comb(j, k1) * math.comb(j, k2))
                pairs.append((_LMAP[(j, k1)], _LMAP[(j, k2)], w, k1 == k2))
    return pairs  # 10 per sigma


_PAIRS = _pair_list()
W_PAIR = [w for (_, _, w, _) in _PAIRS]
# S indices per sigma: j0: [0]; j1: A=1 B=2 D=3 ; j2: A=4 B=5 C=6 D=7 E=8 F=9


# ---------------------------------------------------------------- bass build
def _build_module():
    import concourse.bacc as bacc
    import concourse.mybir as mybir
    import concourse.tile as tile

    f32 = mybir.dt.float32
    f32r = mybir.dt.float32r
    f16 = mybir.dt.float16

    nc = bacc.Bacc("TRN2", target_bir_lowering=False, debug=False, num_devices=8)
    x_d = nc.dram_tensor("x", [HL, C * W], f16, kind="ExternalInput").ap()
    yb_d = nc.dram_tensor("yband", [HL, 594], f16, kind="ExternalInput").ap()
    xb_d = nc.dram_tensor("xband", [102, 1152], f16, kind="ExternalInput").ap()
    ix_d = nc.dram_tensor("intx", [96, 2 * NWO], f16, kind="ExternalInput").ap()
    iy_d = nc.dram_tensor("inty", [NH, NOUT], f16, kind="ExternalInput").ap()
    out_d = nc.dram_tensor("out", [NOUT, NWO * 12], f32, kind="ExternalOutput").ap()

    with tile.TileContext(nc) as tc:
        _emit(tc, nc, x_d, yb_d, xb_d, ix_d, iy_d, out_d)
    nc.compile()
    return nc


class _Balancer:
    """Greedy engine load balancer with static per-instruction cost
    estimates (ns). Tracks projected busy time per engine."""

    # (kind) -> {engine: (ns_per_row, fixed_ns)} -- calibrated to the
    # TimelineSim cost model (Act pays ~185ns SBUF-access init per instr).
    COSTS = {
        "copy_ps": {"act": (0.833, 185.0), "dve": (1.35, 120.0)},
        "sq_ps": {"act": (0.833, 185.0)},
        "sq16": {"act": (0.833, 185.0), "dve": (0.52, 60.0), "pool": (1.98, 140.0)},
        "mul16": {"dve": (0.52, 60.0), "pool": (1.98, 140.0)},
        "add16": {"dve": (0.52, 60.0), "pool": (1.98, 140.0)},
        "add32": {"dve": (1.04, 60.0), "pool": (1.98, 140.0)},
        "copy32": {"act": (0.833, 185.0), "dve": (1.04, 60.0)},
        "scale_copy": {"act": (0.833, 190.0), "dve": (1.04, 125.0)},
    }

    def __init__(self, nc):
        self.nc = nc
        # pre-charge with the hardcoded work emitted outside the balancer
        # (ESP chains on DVE, sx/SM/ESP-act + act table loads, ESP pool ops)
        self.load = {"act": 0.0, "dve": 0.0, "pool": 0.0}

    def _pick(self, kind, rows, allowed=None):
        tbl = self.COSTS[kind]
        best, best_t = None, None
        for eng, (per, fix) in tbl.items():
            if allowed and eng not in allowed:
                continue
            t = self.load[eng] + rows * per + fix
            if best_t is None or t < best_t:
                best, best_t = eng, t
        per, fix = tbl[best]
        self.load[best] += rows * per + fix
        return best

    DVE_RATE, DVE_FIX = 0.52, 120.0
    POOL_RATE, POOL_FIX = 1.98, 280.0

    def pick_unit(self, rows):
        """Pick dve/pool for a whole product+fold chain and commit its cost."""
        td = self.load["dve"] + rows * self.DVE_RATE + self.DVE_FIX
        tp = self.load["pool"] + rows * self.POOL_RATE + self.POOL_FIX
        if td <= tp:
            self.load["dve"] = td
            return "dve"
        self.load["pool"] = tp
        return "pool"

    def pick_diag(self, sq_rows, fold_rows):
        """For a diagonal stream: square+fold on dve/pool, or the square
        fused into an Act PSUM-read with the fold elsewhere."""
        t_unit_d = self.load["dve"] + (sq_rows + fold_rows) * 0.52 + 120.0
        t_unit_p = self.load["pool"] + (sq_rows + fold_rows) * 1.98 + 280.0
        act_sq = sq_rows * 0.833 + 2 * 210.0  # two 2-bank sq_ps instrs
        t_act = max(
            self.load["act"] + act_sq,
            min(self.load["dve"] + fold_rows * 0.52 + 120.0,
                self.load["pool"] + fold_rows * 1.98 + 280.0),
        )
        best = min(t_unit_d, t_unit_p, t_act)
        if best == t_unit_d:
            self.load["dve"] = t_unit_d
            return ("unit", "dve")
        if best == t_unit_p:
            self.load["pool"] = t_unit_p
            return ("unit", "pool")
        self.load["act"] += act_sq
        if (self.load["dve"] + fold_rows * 0.52 + 120.0 <=
                self.load["pool"] + fold_rows * 1.98 + 280.0):
            self.load["dve"] += fold_rows * 0.52 + 120.0
            return ("act", "dve")
        self.load["pool"] += fold_rows * 1.98 + 280.0
        return ("act", "pool")

    def copy(self, dst, src, kind="copy_ps", allowed=None):
        rows = src.free_size()
        eng = self._pick(kind, rows, allowed)
        if eng == "act":
            self.nc.scalar.copy(dst, src)
        elif eng == "dve":
            self.nc.vector.tensor_copy(dst, src)
        else:
            self.nc.gpsimd.tensor_copy(dst, src)

    def square(self, dst, src, kind="sq16", allowed=None):
        import concourse.mybir as mybir

        rows = src.free_size()
        eng = self._pick(kind, rows, allowed)
        if eng == "act":
            self.nc.scalar.activation(
                dst, src, mybir.ActivationFunctionType.Square, bias=0.0, scale=1.0
            )
        elif eng == "dve":
            self.nc.vector.tensor_mul(dst, src, src)
        else:
            self.nc.gpsimd.tensor_mul(dst, src, src)

    def mul(self, dst, a, b, kind="mul16", allowed=None):
        eng = self._pick(kind, a.free_size(), allowed)
        if eng == "dve":
            self.nc.vector.tensor_mul(dst, a, b)
        else:
            self.nc.gpsimd.tensor_mul(dst, a, b)

    def add(self, dst, a, b, kind="add16", allowed=None):
        eng = self._pick(kind, a.free_size(), allowed)
        if eng == "dve":
            self.nc.vector.tensor_add(dst, a, b)
        else:
            self.nc.gpsimd.tensor_add(dst, a, b)


def _emit(tc, nc, x_d, yb_d, xb_d, ix_d, iy_d, out_d):
    import concourse.mybir as mybir

    f32 = mybir.dt.float32
    f32r = mybir.dt.float32r
    f16 = mybir.dt.float16
    AF = mybir.ActivationFunctionType
    OP = mybir.AluOpType

    bal = _Balancer(nc)

    cpool = tc.alloc_tile_pool(name="consts", bufs=1)
    yb = cpool.tile([HL, 594], f16, name="yb")
    nc.sync.dma_start(yb[:], yb_d[:])
    xb = cpool.tile([102, 1152], f16, name="xb")
    nc.sync.dma_start(xb[:], xb_d[:])
    ixb = cpool.tile([96, 2 * NWO], f16, name="ixb")
    nc.sync.dma_start(ixb[:], ix_d[:])
    iyb = cpool.tile([NH, NOUT], f16, name="iyb")
    nc.sync.dma_start(iyb[:], iy_d[:])
    c_eps = cpool.tile([128, 1], f32, name="c_eps")
    nc.vector.memset(c_eps[:], EPS)
    c_ln10 = cpool.tile([128, 1], f32, name="c_ln10")
    nc.vector.memset(c_ln10[:], math.log(10.0))
    c_ln100 = cpool.tile([128, 1], f32, name="c_ln100")
    nc.vector.memset(c_ln100[:], math.log(100.0))
    c_eps1e2 = cpool.tile([128, 1], f32, name="c_eps1e2")
    nc.vector.memset(c_eps1e2[:], 100.0 * EPS)
    c_eps1e4 = cpool.tile([128, 1], f32, name="c_eps1e4")
    nc.vector.memset(c_eps1e4[:], 10000.0 * EPS)
    b_eps = c_eps[:NOUT]
    b_ln10 = c_ln10[:NOUT]
    b_ln100 = c_ln100[:NOUT]
    b_eps1e2 = c_eps1e2[:NOUT]
    b_eps1e4 = c_eps1e4[:NOUT]

    spool = tc.alloc_tile_pool(name="smaps", bufs=1)
    S = {}
    for s in range(2):
        for p in range(10):
            S[(s, p)] = spool.tile(
                [96, 2, NH], f16, name=f"S_{s}_{p}", tag=f"S_{s}_{p}"
            )

    xin = tc.alloc_tile_pool(name="xin", bufs=2)
    zpool = tc.alloc_tile_pool(name="zpool", bufs=2)
    lpool = tc.alloc_tile_pool(name="lpool", bufs=8)
    sqpool = tc.alloc_tile_pool(name="sqpool", bufs=6)
    ppool = tc.alloc_tile_pool(name="ppool", bufs=6)
    fpool = tc.alloc_tile_pool(name="fpool", bufs=8)
    zps = tc.alloc_tile_pool(name="zps", bufs=2, space="PSUM")
    xps = tc.alloc_tile_pool(name="xps", bufs=2, space="PSUM")

    def fold_stream(t, s, p, first, eng):
        """t: [96, 2, CBLK, NH] fp16 product tile; fold c into S[(s,p)].
        Whole chain stays on one engine: same-engine program order means
        no cross-engine semaphore hops inside the chain."""
        v = nc.vector if eng == "dve" else nc.gpsimd
        h = CBLK // 2
        f1 = fpool.tile([96, 2, h, NH], f16, name="f1", tag="f1")
        v.tensor_add(f1[:], t[:, :, :h, :], t[:, :, h:, :])
        f2 = fpool.tile([96, 2, h // 2, NH], f16, name="f2", tag="f2")
        v.tensor_add(f2[:], f1[:, :, : h // 2, :], f1[:, :, h // 2 :, :])
        st = S[(s, p)]
        if first:
            v.tensor_add(st[:], f2[:, :, 0, :], f2[:, :, 1, :])
        else:
            f3 = fpool.tile([96, 2, NH], f16, name="f3", tag="f3")
            v.tensor_add(f3[:], f2[:, :, 0, :], f2[:, :, 1, :])
            v.tensor_add(st[:], st[:], f3[:])

    # diag pair index for map m, and cross pairs (p, m1, m2) per sigma
    _DIAG_P = {0: 0, 1: 1, 2: 3, 3: 4, 4: 7, 5: 9}
    _CROSS = [(2, 1, 2), (5, 3, 4), (6, 3, 5), (8, 4, 5)]

    def yconv_steps(blk):
        """Prepare yconv for a channel block: returns (Z, step closures).
        Each step emits one (ci, hf): 2 matmuls + 1 PSUM->SBUF copy."""
        xt = xin.tile([HL, CBLK * W], f16, name="xt", tag="xt")
        nc.sync.dma_start(xt[:], x_d[:, blk * CBLK * W : (blk + 1) * CBLK * W])
        Z = {}
        for hf in range(2):
            Z[hf] = zpool.tile([102, CBLK, 594], f16, name=f"z{hf}", tag=f"z{hf}")

        def make_step(ci, hf):
            def step():
                w0 = 0 if hf == 0 else 90
                lhs = xt[:, ci * W + w0 : ci * W + w0 + 102]
                zp = zps.tile([102, 2, 512], f32, name="zp", tag="zp")
                for s in range(2):
                    nc.tensor.matmul(
                        zp[:, s, :297],
                        lhs,
                        yb[:, s * 297 : (s + 1) * 297],
                        start=True,
                        stop=True,
                    )
                bal.copy(
                    Z[hf][:, ci, :].rearrange("p (s h) -> p s h", s=2),
                    zp[:, :, :297],
                    kind="copy_ps",
                )
            return step

        steps = [make_step(ci, hf) for ci in range(CBLK) for hf in range(2)]
        return Z, steps

    def gram_blk(blk, Z, ysteps):
        """Emit xconv + products + folds for one channel block,
        interleaving next block's yconv steps to keep PE flowing."""
        ysteps = list(ysteps)
        rows_t = 2 * CBLK * NH
        for s in range(2):
            L = {}
            SQ = {}
            diag_mode = {}
            for m in range(6):
                if ysteps:
                    ysteps.pop(0)()
                fy, fx = _MAP_FYFX[m]
                need_plain = m >= 1
                sq = sqpool.tile([96, 2, CBLK, NH], f16, name=f"sq{m}", tag="sq")
                SQ[m] = sq
                if need_plain:
                    lt = lpool.tile([96, 2, CBLK, NH], f16, name=f"l{m}", tag="lt")
                    L[m] = lt
                    eng = bal.pick_unit(2 * rows_t)
                    diag_mode[m] = (eng, eng)
                else:
                    diag_mode[m] = ("act", None)
                for hf in range(2):
                    xb_col = (s * 2 + hf) * 3 + fx
                    lhsT = xb[:, xb_col * 96 : (xb_col + 1) * 96]
                    xp = xps.tile([96, 2, 512], f32, name="xp", tag="xp")
                    for cs in range(2):
                        rhs = Z[hf][
                            :, cs * CSUB : (cs + 1) * CSUB,
                            s * 297 + fy * NH : s * 297 + (fy + 1) * NH,
                        ]
                        nc.tensor.matmul(
                            xp[:, cs, : CSUB * NH], lhsT, rhs, start=True, stop=True
                        )
                    srcap = xp[:, :, : CSUB * NH].rearrange(
                        "p b (c h) -> p b c h", c=CSUB
                    )
                    sqdst = sq[:, hf].rearrange("p (b c) h -> p b c h", b=2)
                    if need_plain:
                        dst = lt[:, hf].rearrange("p (b c) h -> p b c h", b=2)
                        bal.copy(dst, srcap, kind="copy_ps")
                    else:
                        # m=0: PSUM square must ride Act (single-src)
                        nc.scalar.activation(
                            sqdst, srcap, mybir.ActivationFunctionType.Square,
                            bias=0.0, scale=1.0,
                        )
            # products + folds, both halves at once; each product+fold
            # chain is pinned to one engine (picked by projected load)
            for m in range(6):
                if ysteps and len(ysteps) > (11 - (s * 6 + m)):
                    ysteps.pop(0)()
                sq = SQ[m]
                sqeng, feng = diag_mode[m]
                if m == 0:
                    eng = bal.pick_unit(rows_t)
                else:
                    eng = feng
                    af = L[m][:].rearrange("p f c h -> p (f c h)")
                    sqf = sq[:].rearrange("p f c h -> p (f c h)")
                    if sqeng == "dve":
                        nc.vector.tensor_mul(sqf, af, af)
                    else:
                        nc.gpsimd.tensor_mul(sqf, af, af)
                fold_stream(sq[:], s, _DIAG_P[m], blk == 0, eng)
                for p, m1, m2 in _CROSS:
                    if m2 != m:
                        continue
                    eng = bal.pick_unit(2 * rows_t)
                    pt = ppool.tile([96, 2, CBLK, NH], f16, name="P", tag="P")
                    a1 = L[m1][:].rearrange("p f c h -> p (f c h)")
                    a2 = L[m2][:].rearrange("p f c h -> p (f c h)")
                    ptf = pt[:].rearrange("p f c h -> p (f c h)")
                    if eng == "dve":
                        nc.vector.tensor_mul(ptf, a1, a2)
                    else:
                        nc.gpsimd.tensor_mul(ptf, a1, a2)
                    fold_stream(pt[:], s, p, blk == 0, eng)
        for st in ysteps:
            st()

    # software pipeline: blk 0's yconv runs up front; thereafter blk+1's
    # yconv steps are interleaved into gram(blk)
    Zcur, steps0 = yconv_steps(0)
    for st in steps0:
        st()
    for blk in range(NBLK):
        if blk + 1 < NBLK:
            Znext, ysteps = yconv_steps(blk + 1)
        else:
            Znext, ysteps = None, []
        gram_blk(blk, Zcur, ysteps)
        Zcur = Znext

    for pool in (xps, zps, fpool, ppool, sqpool, lpool, zpool, xin):
        pool.release()

    # ---- integration -> SM maps [NOUT, NWO] (fp32)
    impool = tc.alloc_tile_pool(name="impool", bufs=1)
    SM = {}
    for s in range(2):
        for p in range(10):
            SM[(s, p)] = impool.tile(
                [NOUT, NWO], f32, name=f"SM_{s}_{p}", tag=f"SM_{s}_{p}"
            )
    sxp = tc.alloc_tile_pool(name="sxp", bufs=3)
    ips = tc.alloc_tile_pool(name="ips", bufs=2, space="PSUM")
    yps = tc.alloc_tile_pool(name="yps", bufs=2, space="PSUM")
    for s in range(2):
        for p in range(10):
            ip = ips.tile([NH, NWO], f32, name="ip", tag="ip")
            nc.tensor.matmul(
                ip[:], S[(s, p)][:, 0, :], ixb[:, :NWO], start=True, stop=False
            )
            nc.tensor.matmul(
                ip[:], S[(s, p)][:, 1, :], ixb[:, NWO:], start=False, stop=True
            )
            sx = sxp.tile([NH, NWO], f16, name="sx", tag="sx")
            if bal._pick("scale_copy", NWO) == "act":
                nc.scalar.activation(sx[:], ip[:], AF.Copy, bias=0.0, scale=W_PAIR[p])
            else:
                nc.vector.tensor_scalar(
                    sx[:], ip[:], W_PAIR[p], None, OP.mult, OP.bypass
                )
            yp = yps.tile([NOUT, NWO], f32, name="yp", tag="yp")
            nc.tensor.matmul(yp[:], iyb[:], sx[:], start=True, stop=True)
            bal.copy(SM[(s, p)][:], yp[:], kind="copy_ps")
    for pool in (yps, ips, sxp):
        pool.release()

    # ---- ESP + output
    opool = tc.alloc_tile_pool(name="opool", bufs=1)
    OUTT = opool.tile([NOUT, NWO * 12], f32, name="OUTT")
    outv = OUTT[:].rearrange("p (w c) -> p w c", c=12)
    epool = tc.alloc_tile_pool(name="epool", bufs=2)

    def et(name):
        return epool.tile([NOUT, NWO], f32, name=name, tag=name)

    roots = []  # (av_tile, ln_scale, exp_bias, exp_scale, out_ch)

    for s in range(2):
        ch0 = s * 6
        # ---- j = 0
        m0 = SM[(s, 0)]
        t0 = et("t0j0")
        nc.scalar.activation(t0[:], m0[:], AF.Abs)
        nc.vector.tensor_scalar(outv[:, :, ch0 + 0], t0[:], EPS, None, OP.add, OP.bypass)
        # ---- j = 1 : A=1 B=2 D=3
        A, Bm, D = SM[(s, 1)], SM[(s, 2)], SM[(s, 3)]
        p1 = et("p1j1")
        nc.vector.tensor_add(p1[:], A[:], D[:])
        t = et("tj1")
        nc.scalar.activation(t[:], p1[:], AF.Abs)
        nc.vector.tensor_scalar(outv[:, :, ch0 + 1], t[:], EPS, 10.0, OP.add, OP.mult)
        q = et("qj1")
        nc.gpsimd.tensor_mul(q[:], p1[:], p1[:])
        sA = et("sAj1")
        nc.gpsimd.tensor_mul(sA[:], A[:], A[:])
        sB2 = et("sBj1")
        nc.vector.scalar_tensor_tensor(sB2[:], Bm[:], 2.0, Bm[:], OP.mult, OP.mult)
        sD = et("sDj1")
        nc.gpsimd.tensor_mul(sD[:], D[:], D[:])
        p2 = et("p2j1")
        nc.vector.tensor_add(p2[:], sA[:], sB2[:])
        nc.vector.tensor_add(p2[:], p2[:], sD[:])
        v2 = et("v2j1")
        nc.vector.tensor_sub(v2[:], q[:], p2[:])
        av = et("avj1")
        nc.scalar.activation(av[:], v2[:], AF.Abs)
        # 10*(|v2|/2 + eps)^0.5 == sqrt(50*|v2| + 100*eps): one Sqrt op,
        # and Sqrt/Abs/Square/Copy share an act-function table set
        nc.scalar.activation(
            outv[:, :, ch0 + 2], av[:], AF.Sqrt, bias=b_eps1e2, scale=50.0
        )
        # ---- j = 2 : A=4 B=5 C=6 D=7 E=8 F=9
        A, Bm, Cm, D, E, F = (SM[(s, i)] for i in range(4, 10))
        sA, sB, sC, sD, sE, sF = (et(f"s{i}j2") for i in range(6))
        for i, (dst, s_in) in enumerate(
            ((sA, A), (sB, Bm), (sC, Cm), (sD, D), (sE, E), (sF, F))
        ):
            if i % 2 == 0:
                nc.gpsimd.tensor_mul(dst[:], s_in[:], s_in[:])
            else:
                nc.vector.scalar_tensor_tensor(
                    dst[:], s_in[:], 1.0, s_in[:], OP.mult, OP.mult
                )
        tAD = et("tADj2")
        nc.vector.tensor_add(tAD[:], A[:], D[:])
        p1 = et("p1j2")
        nc.vector.tensor_add(p1[:], tAD[:], F[:])
        t = et("tj2")
        nc.scalar.activation(t[:], p1[:], AF.Abs)
        nc.vector.tensor_scalar(outv[:, :, ch0 + 3], t[:], EPS, 100.0, OP.add, OP.mult)
        p2 = et("p2j2")
        nc.vector.tensor_add(p2[:], sA[:], sD[:])
        nc.vector.tensor_add(p2[:], p2[:], sF[:])
        u = et("uj2")
        nc.vector.tensor_add(u[:], sB[:], sC[:])
        nc.vector.tensor_add(u[:], u[:], sE[:])
        nc.vector.scalar_tensor_tensor(p2[:], u[:], 2.0, p2[:], OP.mult, OP.add)
        q = et("qj2")
        nc.gpsimd.tensor_mul(q[:], p1[:], p1[:])
        v2 = et("v2j2")
        nc.vector.tensor_sub(v2[:], q[:], p2[:])
        av = et("avj2")
        nc.scalar.activation(av[:], v2[:], AF.Abs)
        # 100*(|v2|/2 + eps)^0.5 == sqrt(5000*|v2| + 10000*eps)
        nc.scalar.activation(
            outv[:, :, ch0 + 4], av[:], AF.Sqrt, bias=b_eps1e4, scale=5000.0
        )
        # p3 = cubes + 3*(B^2(A+D) + C^2(A+F) + E^2(D+F)) + 6BCE
        cA = et("cAj2")
        nc.gpsimd.tensor_mul(cA[:], sA[:], A[:])
        cD = et("cDj2")
        nc.gpsimd.tensor_mul(cD[:], sD[:], D[:])
        cF = et("cFj2")
        nc.gpsimd.tensor_mul(cF[:], sF[:], F[:])
        w1 = et("w1j2")
        nc.vector.tensor_add(w1[:], cA[:], cD[:])
        nc.vector.tensor_add(w1[:], w1[:], cF[:])
        y1 = et("y1j2")
        nc.vector.scalar_tensor_tensor(y1[:], sB[:], 1.0, tAD[:], OP.mult, OP.mult)
        tAF = et("tAFj2")
        nc.vector.tensor_add(tAF[:], A[:], F[:])
        y2 = et("y2j2")
        nc.gpsimd.tensor_mul(y2[:], sC[:], tAF[:])
        tDF = et("tDFj2")
        nc.vector.tensor_add(tDF[:], D[:], F[:])
        y3 = et("y3j2")
        nc.gpsimd.tensor_mul(y3[:], sE[:], tDF[:])
        nc.vector.tensor_add(y1[:], y1[:], y2[:])
        nc.vector.tensor_add(y1[:], y1[:], y3[:])
        z = et("zj2")
        nc.vector.scalar_tensor_tensor(z[:], Bm[:], 6.0, Cm[:], OP.mult, OP.mult)
        nc.vector.scalar_tensor_tensor(z[:], z[:], 1.0, E[:], OP.mult, OP.mult)
        nc.vector.scalar_tensor_tensor(y1[:], y1[:], 3.0, z[:], OP.mult, OP.add)
        p3 = et("p3j2")
        nc.vector.tensor_add(p3[:], w1[:], y1[:])
        # e3*3 = v2/2*p1 - p1*p2 + p3
        a3 = et("a3j2")
        nc.vector.scalar_tensor_tensor(a3[:], v2[:], 0.5, p1[:], OP.mult, OP.mult)
        b3 = et("b3j2")
        nc.gpsimd.tensor_mul(b3[:], p1[:], p2[:])
        nc.vector.tensor_sub(a3[:], a3[:], b3[:])
        nc.vector.tensor_add(a3[:], a3[:], p3[:])
        av3 = et("av3j2")
        nc.scalar.activation(av3[:], a3[:], AF.Abs)
        roots.append((av3, 1.0 / 3.0, b_ln100, 1.0 / 3.0, ch0 + 5))

    # bias gate: a tiny chain reading both |a3| tiles so the batched Ln ops
    # cannot be scheduled (and force an act-table reload) before the last
    # Abs of either sigma has run
    gate = epool.tile([NOUT, 1], f32, name="gate", tag="gate")
    nc.vector.scalar_tensor_tensor(
        gate[:], roots[0][0][:, 0:1], 0.0, roots[1][0][:, 0:1], OP.mult, OP.mult
    )
    nc.vector.scalar_tensor_tensor(
        gate[:], gate[:], 1.0, c_eps[:NOUT], OP.mult, OP.add
    )
    b_eps_gated = gate

    # batched by activation function so the Act engine loads each
    # function table once instead of ping-ponging Ln/Exp per root
    lgs = []
    for i, (av, lns, ebias, escale, ch) in enumerate(roots):
        lg = et(f"lg{i}")
        nc.scalar.activation(lg[:], av[:], AF.Ln, bias=b_eps_gated, scale=lns)
        lgs.append(lg)
    for i, (av, lns, ebias, escale, ch) in enumerate(roots):
        nc.scalar.activation(outv[:, :, ch], lgs[i][:], AF.Exp, bias=ebias, scale=escale)

    nc.sync.dma_start(out_d[:], OUTT[:])
    for pool in (epool, opool, impool, spool, cpool):
        pool.release()


def _get_module():
    key = CONV_MODE
    if key not in _CACHE:
        _CACHE[key] = _build_module()
    return _CACHE[key]


# ---------------------------------------------------------------- entry point
def kernel(inputs, kernels0, kernels1, dg_int):
    from concourse.bass_utils import run_bass_kernel_spmd

    in_maps = _make_in_maps(inputs, kernels0, kernels1, dg_int)
    nc = _get_module()
    res = run_bass_kernel_spmd(nc, in_maps, core_ids=list(range(8)), **RUN_KWARGS)
    global LAST
    LAST = res
    out = np.empty((B, NWO, NWO, 12), dtype=np.float32)
    for core in range(8):
        b, half = core // 2, core % 2
        H0 = half * NOUT
        out[b, H0 : H0 + NOUT] = res.results[core]["out"].reshape(NOUT, NWO, 12)
    return out


# revision 70
# speedup vs baseline: 1.0525x; 1.0310x over previous
"""Trainium2 Bass kernel for nn_GaussianDerivativeESPLayer.

Strategy (per the data-parallel hint, extended since B=4 < 8 cores):
shard (batch b, H-half) across the 8 cores. Each core computes output
rows [H0, H0+93) of one batch element from input rows [g0, g0+105).

Performance design (512us -> 301us in the TimelineSim cost model):
  - fp16 conv matmuls (1 cyc/row on PE vs 4 for fp32); integration
    matmuls fp16 as well.
  - j0 diagonal Gram entry fused: Act-engine Square reads conv PSUM and
    writes the squared map to SBUF fp16 in one instruction (copy+mul).
  - Cross products / squares / channel-fold adds run in fp16 (DVE 2x
    perf mode). Each product+fold chain is pinned to one engine
    (DVE or Pool, greedy by projected load) so the chain rides implicit
    same-engine ordering instead of cross-engine semaphores. PSUM->SBUF
    copies are balanced between Act and DVE (Pool cannot access PSUM;
    TensorTensor cannot take two PSUM sources; DMA cannot read PSUM).
  - hf-merged tiles [96, 2, c, 99] and 2-PSUM-bank batched copies halve
    instruction counts (Act pays ~185ns access-init per instruction).
  - Software pipelining: the next channel block's yconv matmuls+copies
    are interleaved into the current block's xconv/product emission so
    parked matmuls never head-of-line block the PE sequencer.
  - ESP stage in fp32 (Newton-identity cancellations), with Ln/Exp
    calls batched by function to avoid Act table reloads; the
    sqrt-binomial pair weights fold into the integration scale copy.

Per-core pipeline (all layouts [partition, free]):
  1. yconv: data-stationary matmuls X^T @ yband -> Z [w, (s, fy, c, h')]
  2. xconv: band-stationary matmuls xband^T @ Z -> L [w', (c, h')] PSUM
  3. Gram: Square/mult + fold-tree over 64 channels -> 20 S maps [96, 99]
  4. Integration: valid 7x7 separable conv as two matmul stages
  5. ESP: Newton-identity polynomials per pixel, roots via exp/ln.
"""

import math

import numpy as np

B, H, W, C = 4, 192, 192, 64
NH = 99  # h' rows computed per core (pre y-integration)
NOUT = 93  # h'' output rows per core
HL = 105  # input rows per core
NWO = 186  # output cols
CBLK = 8  # channels per block
NBLK = C // CBLK
CSUB = 4  # channels per xconv matmul chunk
RI = 3  # integrator radius
EPS = float(np.finfo(np.float64).eps)

CONV_MODE = "fp16"

_CACHE = {}
RUN_KWARGS = {}  # test harness can set dict(trace=True) before calling kernel()
LAST = None  # BassKernelResults of the most recent kernel() call


# ---------------------------------------------------------------- host math
def _extract_filters(kernels):
    """kernels [6, kh, kw, C, 1] channel-tiled separable. Returns
    (gys, gxs): 1D filters with sigma^order folded in, such that
    kernel(j,k) == outer(gys[j-k], gxs[k])."""
    K = np.asarray(kernels, dtype=np.float64)[:, :, :, 0, 0]
    i0 = K.shape[1] // 2
    s0 = math.sqrt(abs(K[0][i0, i0]))
    g0y = K[0][:, i0] / s0
    g0x = K[0][i0, :] / s0
    g1x = K[2][i0, :] / g0y[i0]  # (j=1,k=1) = s*g1x (x) * g0y (y)
    g1y = K[1][:, i0] / g0x[i0]  # (j=1,k=0) = s*g0x (x) * g1y (y)
    g2x = K[5][i0, :] / g0y[i0]
    g2y = K[3][:, i0] / g0x[i0]
    return [g0y, g1y, g2y], [g0x, g1x, g2x]


def _extract_integrator(dg_int):
    K = np.asarray(dg_int, dtype=np.float64)[:, :, 0, 0]
    i0 = K.shape[0] // 2
    s0 = math.sqrt(abs(K[i0, i0]))
    return K[:, i0] / s0, K[i0, :] / s0  # giy, gix


def _band(k_count, m_count, g, delta):
    """band[k, m] = g[k - m + delta + r] when |k - m + delta| <= r."""
    r = len(g) // 2
    k = np.arange(k_count)[:, None]
    m = np.arange(m_count)[None, :]
    d = k - m + delta
    ok = np.abs(d) <= r
    out = np.zeros((k_count, m_count), dtype=np.float64)
    out[ok] = np.asarray(g)[(d + r)[ok]]
    return out


def _build_host_tensors(kernels0, kernels1, dg_int):
    gys0, gxs0 = _extract_filters(kernels0)
    gys1, gxs1 = _extract_filters(kernels1)
    giy, gix = _extract_integrator(dg_int)
    gys = [gys0, gys1]
    gxs = [gxs0, gxs1]

    # yband per half: [HL, 594] cols = (sigma, fy, h'-local)
    ybands = []
    for half in range(2):
        H0 = half * NOUT
        g0 = 0 if half == 0 else 87
        cols = []
        for s in range(2):
            for fy in range(3):
                cols.append(_band(HL, NH, gys[s][fy], g0 - H0))
        ybands.append(np.concatenate(cols, axis=1).astype(np.float16))

    # xband: [102, 1152] cols = (sigma, half, fx, 96)  -- core-independent
    xcols = []
    for s in range(2):
        for hf in range(2):
            delta = 0 if hf == 0 else (90 - 96)
            for fx in range(3):
                xcols.append(_band(102, 96, gxs[s][fx], delta))
    xband = np.concatenate(xcols, axis=1).astype(np.float16)

    # x-integration bands [96, 372]: valid conv, out w'' n: sum_k S[w'=k+96*hf]*gix[w'-n]
    ix = []
    for hf in range(2):
        k = np.arange(96)[:, None] + 96 * hf
        n = np.arange(NWO)[None, :]
        d = k - n
        ok = (d >= 0) & (d <= 2 * RI)
        b = np.zeros((96, NWO))
        b[ok] = gix[d[ok]]
        ix.append(b)
    intx = np.concatenate(ix, axis=1).astype(np.float16)

    # y-integration band [NH, NOUT]: inty[k, m] = giy[k - m], 0 <= k-m <= 6
    k = np.arange(NH)[:, None]
    m = np.arange(NOUT)[None, :]
    d = k - m
    ok = (d >= 0) & (d <= 2 * RI)
    inty = np.zeros((NH, NOUT))
    inty[ok] = giy[d[ok]]
    inty = inty.astype(np.float16)

    return ybands, xband, intx, inty


def _make_in_maps(inputs, kernels0, kernels1, dg_int):
    x = np.asarray(inputs, dtype=np.float16)
    ybands, xband, intx, inty = _build_host_tensors(kernels0, kernels1, dg_int)
    in_maps = []
    for core in range(8):
        b, half = core // 2, core % 2
        g0 = 0 if half == 0 else 87
        xc = np.ascontiguousarray(
            x[b, g0 : g0 + HL].transpose(0, 2, 1).reshape(HL, C * W)
        )
        in_maps.append(
            {
                "x": xc,
                "yband": ybands[half],
                "xband": xband,
                "intx": intx,
                "inty": inty,
            }
        )
    return in_maps


# pairs per sigma: (m1, m2, weight, diag); m = map id of (j, k):
# (0,0),(1,0),(1,1),(2,0),(2,1),(2,2)
_LMAP = {(0, 0): 0, (1, 0): 1, (1, 1): 2, (2, 0): 3, (2, 1): 4, (2, 2): 5}
_MAP_FYFX = {0: (0, 0), 1: (1, 0), 2: (0, 1), 3: (2, 0), 4: (1, 1), 5: (0, 2)}


def _pair_list():
    pairs = []
    for j in range(3):
        for k1 in range(j + 1):
            for k2 in range(k1, j + 1):
                w = math.sqrt(math.comb(j, k1) * math.comb(j, k2))
                pairs.append((_LMAP[(j, k1)], _LMAP[(j, k2)], w, k1 == k2))
    return pairs  # 10 per sigma


_PAIRS = _pair_list()
W_PAIR = [w for (_, _, w, _) in _PAIRS]
# S indices per sigma: j0: [0]; j1: A=1 B=2 D=3 ; j2: A=4 B=5 C=6 D=7 E=8 F=9


# ---------------------------------------------------------------- bass build
def _build_module():
    import concourse.bacc as bacc
    import concourse.mybir as mybir
    import concourse.tile as tile

    f32 = mybir.dt.float32
    f32r = mybir.dt.float32r
    f16 = mybir.dt.float16

    nc = bacc.Bacc("TRN2", target_bir_lowering=False, debug=False, num_devices=8)
    x_d = nc.dram_tensor("x", [HL, C * W], f16, kind="ExternalInput").ap()
    yb_d = nc.dram_tensor("yband", [HL, 594], f16, kind="ExternalInput").ap()
    xb_d = nc.dram_tensor("xband", [102, 1152], f16, kind="ExternalInput").ap()
    ix_d = nc.dram_tensor("intx", [96, 2 * NWO], f16, kind="ExternalInput").ap()
    iy_d = nc.dram_tensor("inty", [NH, NOUT], f16, kind="ExternalInput").ap()
    out_d = nc.dram_tensor("out", [NOUT, NWO * 12], f32, kind="ExternalOutput").ap()

    with tile.TileContext(nc) as tc:
        _emit(tc, nc, x_d, yb_d, xb_d, ix_d, iy_d, out_d)
    nc.compile()
    return nc


class _Balancer:
    """Greedy engine load balancer with static per-instruction cost
    estimates (ns). Tracks projected busy time per engine."""

    # (kind) -> {engine: (ns_per_row, fixed_ns)} -- calibrated to the
    # TimelineSim cost model (Act pays ~185ns SBUF-access init per instr).
    COSTS = {
        "copy_ps": {"act": (0.833, 185.0), "dve": (1.22, 100.0)},
        "sq_ps": {"act": (0.833, 185.0)},
        "sq16": {"act": (0.833, 185.0), "dve": (0.52, 60.0), "pool": (1.98, 140.0)},
        "mul16": {"dve": (0.52, 60.0), "pool": (1.98, 140.0)},
        "add16": {"dve": (0.52, 60.0), "pool": (1.98, 140.0)},
        "add32": {"dve": (1.04, 60.0), "pool": (1.98, 140.0)},
        "copy32": {"act": (0.833, 185.0), "dve": (1.04, 60.0)},
        "scale_copy": {"act": (0.833, 190.0), "dve": (1.04, 125.0)},
    }

    def __init__(self, nc):
        self.nc = nc
        # pre-charge with the hardcoded work emitted outside the balancer
        # (ESP chains on DVE, sx/SM/ESP-act + act table loads, ESP pool ops)
        self.load = {"act": 0.0, "dve": 0.0, "pool": 0.0}

    def _pick(self, kind, rows, allowed=None):
        tbl = self.COSTS[kind]
        best, best_t = None, None
        for eng, (per, fix) in tbl.items():
            if allowed and eng not in allowed:
                continue
            t = self.load[eng] + rows * per + fix
            if best_t is None or t < best_t:
                best, best_t = eng, t
        per, fix = tbl[best]
        self.load[best] += rows * per + fix
        return best

    DVE_RATE, DVE_FIX = 0.52, 120.0
    POOL_RATE, POOL_FIX = 1.98, 280.0

    def pick_unit(self, rows):
        """Pick dve/pool for a whole product+fold chain and commit its cost."""
        td = self.load["dve"] + rows * self.DVE_RATE + self.DVE_FIX
        tp = self.load["pool"] + rows * self.POOL_RATE + self.POOL_FIX
        if td <= tp:
            self.load["dve"] = td
            return "dve"
        self.load["pool"] = tp
        return "pool"

    def pick_diag(self, sq_rows, fold_rows):
        """For a diagonal stream: square+fold on dve/pool, or the square
        fused into an Act PSUM-read with the fold elsewhere."""
        t_unit_d = self.load["dve"] + (sq_rows + fold_rows) * 0.52 + 120.0
        t_unit_p = self.load["pool"] + (sq_rows + fold_rows) * 1.98 + 280.0
        act_sq = sq_rows * 0.833 + 2 * 210.0  # two 2-bank sq_ps instrs
        t_act = max(
            self.load["act"] + act_sq,
            min(self.load["dve"] + fold_rows * 0.52 + 120.0,
                self.load["pool"] + fold_rows * 1.98 + 280.0),
        )
        best = min(t_unit_d, t_unit_p, t_act)
        if best == t_unit_d:
            self.load["dve"] = t_unit_d
            return ("unit", "dve")
        if best == t_unit_p:
            self.load["pool"] = t_unit_p
            return ("unit", "pool")
        self.load["act"] += act_sq
        if (self.load["dve"] + fold_rows * 0.52 + 120.0 <=
                self.load["pool"] + fold_rows * 1.98 + 280.0):
            self.load["dve"] += fold_rows * 0.52 + 120.0
            return ("act", "dve")
        self.load["pool"] += fold_rows * 1.98 + 280.0
        return ("act", "pool")

    def copy(self, dst, src, kind="copy_ps", allowed=None):
        rows = src.free_size()
        eng = self._pick(kind, rows, allowed)
        if eng == "act":
            self.nc.scalar.copy(dst, src)
        elif eng == "dve":
            self.nc.vector.tensor_copy(dst, src)
        else:
            self.nc.gpsimd.tensor_copy(dst, src)

    def square(self, dst, src, kind="sq16", allowed=None):
        import concourse.mybir as mybir

        rows = src.free_size()
        eng = self._pick(kind, rows, allowed)
        if eng == "act":
            self.nc.scalar.activation(
                dst, src, mybir.ActivationFunctionType.Square, bias=0.0, scale=1.0
            )
        elif eng == "dve":
            self.nc.vector.tensor_mul(dst, src, src)
        else:
            self.nc.gpsimd.tensor_mul(dst, src, src)

    def mul(self, dst, a, b, kind="mul16", allowed=None):
        eng = self._pick(kind, a.free_size(), allowed)
        if eng == "dve":
            self.nc.vector.tensor_mul(dst, a, b)
        else:
            self.nc.gpsimd.tensor_mul(dst, a, b)

    def add(self, dst, a, b, kind="add16", allowed=None):
        eng = self._pick(kind, a.free_size(), allowed)
        if eng == "dve":
            self.nc.vector.tensor_add(dst, a, b)
        else:
            self.nc.gpsimd.tensor_add(dst, a, b)


def _emit(tc, nc, x_d, yb_d, xb_d, ix_d, iy_d, out_d):
    import concourse.mybir as mybir

    f32 = mybir.dt.float32
    f32r = mybir.dt.float32r
    f16 = mybir.dt.float16
    AF = mybir.ActivationFunctionType
    OP = mybir.AluOpType

    bal = _Balancer(nc)

    cpool = tc.alloc_tile_pool(name="consts", bufs=1)
    yb = cpool.tile([HL, 594], f16, name="yb")
    nc.sync.dma_start(yb[:], yb_d[:])
    xb = cpool.tile([102, 1152], f16, name="xb")
    ixb = cpool.tile([96, 2 * NWO], f16, name="ixb")
    iyb = cpool.tile([NH, NOUT], f16, name="iyb")
    c_eps = cpool.tile([128, 1], f32, name="c_eps")
    nc.vector.memset(c_eps[:], EPS)
    c_ln10 = cpool.tile([128, 1], f32, name="c_ln10")
    nc.vector.memset(c_ln10[:], math.log(10.0))
    c_ln100 = cpool.tile([128, 1], f32, name="c_ln100")
    nc.vector.memset(c_ln100[:], math.log(100.0))
    c_eps1e2 = cpool.tile([128, 1], f32, name="c_eps1e2")
    nc.vector.memset(c_eps1e2[:], 100.0 * EPS)
    c_eps1e4 = cpool.tile([128, 1], f32, name="c_eps1e4")
    nc.vector.memset(c_eps1e4[:], 10000.0 * EPS)
    # dummy Sqrt: pre-load the sqrt act-table set (which also contains
    # Square/Copy/Abs) so the ESP tail's first Sqrt needs no reload
    c_warm = cpool.tile([1, 1], f32, name="c_warm")
    nc.scalar.activation(c_warm[:], c_eps[:1], AF.Sqrt, bias=0.0, scale=1.0)
    b_eps = c_eps[:NOUT]
    b_ln10 = c_ln10[:NOUT]
    b_ln100 = c_ln100[:NOUT]
    b_eps1e2 = c_eps1e2[:NOUT]
    b_eps1e4 = c_eps1e4[:NOUT]

    spool = tc.alloc_tile_pool(name="smaps", bufs=1)
    S = {}
    for s in range(2):
        for p in range(10):
            # 4 block-pair partials x 2 hf: the x-int matmul accumulates
            # the partials in PSUM (on the idle PE) so even-numbered
            # blocks skip the cross-block accumulate add entirely
            S[(s, p)] = spool.tile(
                [96, 4, 2, NH], f16, name=f"S_{s}_{p}", tag=f"S_{s}_{p}"
            )

    xin = tc.alloc_tile_pool(name="xin", bufs=2)
    zpool = tc.alloc_tile_pool(name="zpool", bufs=2)
    lpool = tc.alloc_tile_pool(name="lpool", bufs=8)
    sqpool = tc.alloc_tile_pool(name="sqpool", bufs=6)
    ppool = tc.alloc_tile_pool(name="ppool", bufs=6)
    fpool = tc.alloc_tile_pool(name="fpool", bufs=8)
    zps = tc.alloc_tile_pool(name="zps", bufs=2, space="PSUM")
    xps = tc.alloc_tile_pool(name="xps", bufs=2, space="PSUM")

    def fold_stream(t, s, p, blk, eng):
        """t: [96, 2, CBLK, NH] fp16 product tile; fold c into the
        block-pair partial S[(s,p)][:, blk//2]. Whole chain stays on one
        engine: same-engine program order means no cross-engine hops."""
        v = nc.vector if eng == "dve" else nc.gpsimd
        h = CBLK // 2
        f1 = fpool.tile([96, 2, h, NH], f16, name="f1", tag="f1")
        v.tensor_add(f1[:], t[:, :, :h, :], t[:, :, h:, :])
        f2 = fpool.tile([96, 2, h // 2, NH], f16, name="f2", tag="f2")
        v.tensor_add(f2[:], f1[:, :, : h // 2, :], f1[:, :, h // 2 :, :])
        st = S[(s, p)][:, blk // 2]
        if blk % 2 == 0:
            v.tensor_add(st, f2[:, :, 0, :], f2[:, :, 1, :])
        else:
            f3 = fpool.tile([96, 2, NH], f16, name="f3", tag="f3")
            v.tensor_add(f3[:], f2[:, :, 0, :], f2[:, :, 1, :])
            v.tensor_add(st, st, f3[:])

    # diag pair index for map m, and cross pairs (p, m1, m2) per sigma
    _DIAG_P = {0: 0, 1: 1, 2: 3, 3: 4, 4: 7, 5: 9}
    _CROSS = [(2, 1, 2), (5, 3, 4), (6, 3, 5), (8, 4, 5)]

    def yconv_steps(blk):
        """Prepare yconv for a channel block: returns (Z, step closures).
        Each step emits one (ci, hf): 2 matmuls + 1 PSUM->SBUF copy."""
        xt = xin.tile([HL, CBLK * W], f16, name="xt", tag="xt")
        nc.sync.dma_start(xt[:], x_d[:, blk * CBLK * W : (blk + 1) * CBLK * W])
        Z = {}
        for hf in range(2):
            Z[hf] = zpool.tile([102, CBLK, 594], f16, name=f"z{hf}", tag=f"z{hf}")

        def make_step(ci, hf):
            def step():
                w0 = 0 if hf == 0 else 90
                lhs = xt[:, ci * W + w0 : ci * W + w0 + 102]
                zp = zps.tile([102, 2, 512], f32, name="zp", tag="zp")
                for s in range(2):
                    nc.tensor.matmul(
                        zp[:, s, :297],
                        lhs,
                        yb[:, s * 297 : (s + 1) * 297],
                        start=True,
                        stop=True,
                    )
                bal.copy(
                    Z[hf][:, ci, :].rearrange("p (s h) -> p s h", s=2),
                    zp[:, :, :297],
                    kind="copy_ps",
                )
            return step

        steps = [make_step(ci, hf) for ci in range(CBLK) for hf in range(2)]
        return Z, steps

    def gram_blk(blk, Z, ysteps):
        """Emit xconv + products + folds for one channel block,
        interleaving next block's yconv steps to keep PE flowing."""
        ysteps = list(ysteps)
        rows_t = 2 * CBLK * NH
        for s in range(2):
            L = {}
            SQ = {}
            diag_mode = {}
            for m in range(6):
                if ysteps:
                    ysteps.pop(0)()
                fy, fx = _MAP_FYFX[m]
                need_plain = m >= 1
                sq = sqpool.tile([96, 2, CBLK, NH], f16, name=f"sq{m}", tag="sq")
                SQ[m] = sq
                if need_plain:
                    lt = lpool.tile([96, 2, CBLK, NH], f16, name=f"l{m}", tag="lt")
                    L[m] = lt
                    eng = bal.pick_unit(2 * rows_t)
                    diag_mode[m] = (eng, eng)
                else:
                    diag_mode[m] = ("act", None)
                for hf in range(2):
                    xb_col = (s * 2 + hf) * 3 + fx
                    lhsT = xb[:, xb_col * 96 : (xb_col + 1) * 96]
                    xp = xps.tile([96, 2, 512], f32, name="xp", tag="xp")
                    for cs in range(2):
                        rhs = Z[hf][
                            :, cs * CSUB : (cs + 1) * CSUB,
                            s * 297 + fy * NH : s * 297 + (fy + 1) * NH,
                        ]
                        nc.tensor.matmul(
                            xp[:, cs, : CSUB * NH], lhsT, rhs, start=True, stop=True
                        )
                    srcap = xp[:, :, : CSUB * NH].rearrange(
                        "p b (c h) -> p b c h", c=CSUB
                    )
                    sqdst = sq[:, hf].rearrange("p (b c) h -> p b c h", b=2)
                    if need_plain:
                        dst = lt[:, hf].rearrange("p (b c) h -> p b c h", b=2)
                        bal.copy(dst, srcap, kind="copy_ps")
                    else:
                        # m=0: PSUM square must ride Act (single-src)
                        nc.scalar.activation(
                            sqdst, srcap, mybir.ActivationFunctionType.Square,
                            bias=0.0, scale=1.0,
                        )
            # products + folds, both halves at once; each product+fold
            # chain is pinned to one engine (picked by projected load)
            for m in range(6):
                if ysteps and len(ysteps) > (11 - (s * 6 + m)):
                    ysteps.pop(0)()
                sq = SQ[m]
                sqeng, feng = diag_mode[m]
                if m == 0:
                    eng = bal.pick_unit(rows_t)
                else:
                    eng = feng
                    af = L[m][:].rearrange("p f c h -> p (f c h)")
                    sqf = sq[:].rearrange("p f c h -> p (f c h)")
                    if sqeng == "dve":
                        nc.vector.tensor_mul(sqf, af, af)
                    else:
                        nc.gpsimd.tensor_mul(sqf, af, af)
                fold_stream(sq[:], s, _DIAG_P[m], blk, eng)
                for p, m1, m2 in _CROSS:
                    if m2 != m:
                        continue
                    eng = bal.pick_unit(2 * rows_t)
                    pt = ppool.tile([96, 2, CBLK, NH], f16, name="P", tag="P")
                    a1 = L[m1][:].rearrange("p f c h -> p (f c h)")
                    a2 = L[m2][:].rearrange("p f c h -> p (f c h)")
                    ptf = pt[:].rearrange("p f c h -> p (f c h)")
                    if eng == "dve":
                        nc.vector.tensor_mul(ptf, a1, a2)
                    else:
                        nc.gpsimd.tensor_mul(ptf, a1, a2)
                    fold_stream(pt[:], s, p, blk, eng)
        for st in ysteps:
            st()

    # software pipeline: blk 0's yconv runs up front; thereafter blk+1's
    # yconv steps are interleaved into gram(blk)
    Zcur, steps0 = yconv_steps(0)
    # deferred const loads: xband is first needed by xconv (after blk0's
    # yconv), the integrator bands much later -- keep them off the
    # warmup-critical DMA queue
    nc.sync.dma_start(xb[:], xb_d[:])
    nc.sync.dma_start(ixb[:], ix_d[:])
    nc.sync.dma_start(iyb[:], iy_d[:])
    for st in steps0:
        st()
    for blk in range(NBLK):
        if blk + 1 < NBLK:
            Znext, ysteps = yconv_steps(blk + 1)
        else:
            Znext, ysteps = None, []
        gram_blk(blk, Zcur, ysteps)
        Zcur = Znext

    for pool in (xps, zps, fpool, ppool, sqpool, lpool, zpool, xin):
        pool.release()

    # ---- integration -> SM maps [NOUT, NWO] (fp32)
    impool = tc.alloc_tile_pool(name="impool", bufs=1)
    SM = {}
    for s in range(2):
        for p in range(10):
            SM[(s, p)] = impool.tile(
                [NOUT, NWO], f32, name=f"SM_{s}_{p}", tag=f"SM_{s}_{p}"
            )
    sxp = tc.alloc_tile_pool(name="sxp", bufs=3)
    ips = tc.alloc_tile_pool(name="ips", bufs=2, space="PSUM")
    yps = tc.alloc_tile_pool(name="yps", bufs=2, space="PSUM")
    for s in range(2):
        for p in range(10):
            ip = ips.tile([NH, NWO], f32, name="ip", tag="ip")
            for g in range(4):
                for hf in range(2):
                    nc.tensor.matmul(
                        ip[:],
                        S[(s, p)][:, g, hf, :],
                        ixb[:, hf * NWO : (hf + 1) * NWO],
                        start=(g == 0 and hf == 0),
                        stop=(g == 3 and hf == 1),
                    )
            sx = sxp.tile([NH, NWO], f16, name="sx", tag="sx")
            if bal._pick("scale_copy", NWO) == "act":
                nc.scalar.activation(sx[:], ip[:], AF.Copy, bias=0.0, scale=W_PAIR[p])
            else:
                nc.vector.tensor_scalar(
                    sx[:], ip[:], W_PAIR[p], None, OP.mult, OP.bypass
                )
            yp = yps.tile([NOUT, NWO], f32, name="yp", tag="yp")
            nc.tensor.matmul(yp[:], iyb[:], sx[:], start=True, stop=True)
            bal.copy(SM[(s, p)][:], yp[:], kind="copy_ps")
    for pool in (yps, ips, sxp):
        pool.release()

    # ---- ESP + output
    opool = tc.alloc_tile_pool(name="opool", bufs=1)
    OUTT = opool.tile([NOUT, 12, NWO], f32, name="OUTT")
    outv = OUTT[:]
    epool = tc.alloc_tile_pool(name="epool", bufs=2)

    def et(name):
        return epool.tile([NOUT, NWO], f32, name=name, tag=name)

    roots = []  # (av_tile, ln_scale, exp_bias, exp_scale, out_ch)

    for s in range(2):
        ch0 = s * 6
        # ---- j = 0
        m0 = SM[(s, 0)]
        t0 = et("t0j0")
        nc.scalar.activation(t0[:], m0[:], AF.Abs)
        nc.vector.tensor_scalar(outv[:, ch0 + 0, :], t0[:], EPS, None, OP.add, OP.bypass)
        # ---- j = 1 : A=1 B=2 D=3
        A, Bm, D = SM[(s, 1)], SM[(s, 2)], SM[(s, 3)]
        p1 = et("p1j1")
        nc.vector.tensor_add(p1[:], A[:], D[:])
        t = et("tj1")
        nc.scalar.activation(t[:], p1[:], AF.Abs)
        nc.vector.tensor_scalar(outv[:, ch0 + 1, :], t[:], EPS, 10.0, OP.add, OP.mult)
        q = et("qj1")
        nc.gpsimd.tensor_mul(q[:], p1[:], p1[:])
        sA = et("sAj1")
        nc.gpsimd.tensor_mul(sA[:], A[:], A[:])
        sB2 = et("sBj1")
        nc.vector.scalar_tensor_tensor(sB2[:], Bm[:], 2.0, Bm[:], OP.mult, OP.mult)
        sD = et("sDj1")
        nc.gpsimd.tensor_mul(sD[:], D[:], D[:])
        p2 = et("p2j1")
        nc.vector.tensor_add(p2[:], sA[:], sB2[:])
        nc.vector.tensor_add(p2[:], p2[:], sD[:])
        v2 = et("v2j1")
        nc.vector.tensor_sub(v2[:], q[:], p2[:])
        av = et("avj1")
        nc.scalar.activation(av[:], v2[:], AF.Abs)
        # 10*(|v2|/2 + eps)^0.5 == sqrt(50*|v2| + 100*eps): one Sqrt op,
        # and Sqrt/Abs/Square/Copy share an act-function table set
        nc.scalar.activation(
            outv[:, ch0 + 2, :], av[:], AF.Sqrt, bias=b_eps1e2, scale=50.0
        )
        # ---- j = 2 : A=4 B=5 C=6 D=7 E=8 F=9
        A, Bm, Cm, D, E, F = (SM[(s, i)] for i in range(4, 10))
        sA, sB, sC, sD, sE, sF = (et(f"s{i}j2") for i in range(6))
        for i, (dst, s_in) in enumerate(
            ((sA, A), (sB, Bm), (sC, Cm), (sD, D), (sE, E), (sF, F))
        ):
            if i % 2 == 0:
                nc.gpsimd.tensor_mul(dst[:], s_in[:], s_in[:])
            else:
                nc.vector.scalar_tensor_tensor(
                    dst[:], s_in[:], 1.0, s_in[:], OP.mult, OP.mult
                )
        tAD = et("tADj2")
        nc.vector.tensor_add(tAD[:], A[:], D[:])
        p1 = et("p1j2")
        nc.vector.tensor_add(p1[:], tAD[:], F[:])
        t = et("tj2")
        nc.scalar.activation(t[:], p1[:], AF.Abs)
        nc.vector.tensor_scalar(outv[:, ch0 + 3, :], t[:], EPS, 100.0, OP.add, OP.mult)
        p2 = et("p2j2")
        nc.vector.tensor_add(p2[:], sA[:], sD[:])
        nc.vector.tensor_add(p2[:], p2[:], sF[:])
        u = et("uj2")
        nc.vector.tensor_add(u[:], sB[:], sC[:])
        nc.vector.tensor_add(u[:], u[:], sE[:])
        nc.vector.scalar_tensor_tensor(p2[:], u[:], 2.0, p2[:], OP.mult, OP.add)
        q = et("qj2")
        nc.gpsimd.tensor_mul(q[:], p1[:], p1[:])
        v2 = et("v2j2")
        nc.vector.tensor_sub(v2[:], q[:], p2[:])
        av = et("avj2")
        nc.scalar.activation(av[:], v2[:], AF.Abs)
        # 100*(|v2|/2 + eps)^0.5 == sqrt(5000*|v2| + 10000*eps)
        nc.scalar.activation(
            outv[:, ch0 + 4, :], av[:], AF.Sqrt, bias=b_eps1e4, scale=5000.0
        )
        # p3 = cubes + 3*(B^2(A+D) + C^2(A+F) + E^2(D+F)) + 6BCE
        cA = et("cAj2")
        nc.gpsimd.tensor_mul(cA[:], sA[:], A[:])
        cD = et("cDj2")
        nc.gpsimd.tensor_mul(cD[:], sD[:], D[:])
        cF = et("cFj2")
        nc.gpsimd.tensor_mul(cF[:], sF[:], F[:])
        w1 = et("w1j2")
        nc.vector.tensor_add(w1[:], cA[:], cD[:])
        nc.vector.tensor_add(w1[:], w1[:], cF[:])
        y1 = et("y1j2")
        nc.vector.scalar_tensor_tensor(y1[:], sB[:], 1.0, tAD[:], OP.mult, OP.mult)
        tAF = et("tAFj2")
        nc.vector.tensor_add(tAF[:], A[:], F[:])
        y2 = et("y2j2")
        nc.gpsimd.tensor_mul(y2[:], sC[:], tAF[:])
        tDF = et("tDFj2")
        nc.vector.tensor_add(tDF[:], D[:], F[:])
        y3 = et("y3j2")
        nc.gpsimd.tensor_mul(y3[:], sE[:], tDF[:])
        nc.vector.tensor_add(y1[:], y1[:], y2[:])
        nc.vector.tensor_add(y1[:], y1[:], y3[:])
        z = et("zj2")
        nc.vector.scalar_tensor_tensor(z[:], Bm[:], 6.0, Cm[:], OP.mult, OP.mult)
        nc.vector.scalar_tensor_tensor(z[:], z[:], 1.0, E[:], OP.mult, OP.mult)
        nc.vector.scalar_tensor_tensor(y1[:], y1[:], 3.0, z[:], OP.mult, OP.add)
        p3 = et("p3j2")
        nc.vector.tensor_add(p3[:], w1[:], y1[:])
        # e3*3 = v2/2*p1 - p1*p2 + p3
        a3 = et("a3j2")
        nc.vector.scalar_tensor_tensor(a3[:], v2[:], 0.5, p1[:], OP.mult, OP.mult)
        b3 = et("b3j2")
        nc.gpsimd.tensor_mul(b3[:], p1[:], p2[:])
        nc.vector.tensor_sub(a3[:], a3[:], b3[:])
        nc.vector.tensor_add(a3[:], a3[:], p3[:])
        av3 = et("av3j2")
        nc.scalar.activation(av3[:], a3[:], AF.Abs)
        roots.append((av3, 1.0 / 3.0, b_ln100, 1.0 / 3.0, ch0 + 5))

    # bias gate: a tiny chain reading both |a3| tiles so the batched Ln ops
    # cannot be scheduled (and force an act-table reload) before the last
    # Abs of either sigma has run
    gate = epool.tile([NOUT, 1], f32, name="gate", tag="gate")
    nc.vector.scalar_tensor_tensor(
        gate[:], roots[0][0][:, 0:1], 0.0, roots[1][0][:, 0:1], OP.mult, OP.mult
    )
    nc.vector.scalar_tensor_tensor(
        gate[:], gate[:], 1.0, c_eps[:NOUT], OP.mult, OP.add
    )
    b_eps_gated = gate

    # batched by activation function so the Act engine loads each
    # function table once instead of ping-ponging Ln/Exp per root
    lgs = []
    for i, (av, lns, ebias, escale, ch) in enumerate(roots):
        lg = et(f"lg{i}")
        nc.scalar.activation(lg[:], av[:], AF.Ln, bias=b_eps_gated, scale=lns)
        lgs.append(lg)
    for i, (av, lns, ebias, escale, ch) in enumerate(roots):
        nc.scalar.activation(outv[:, ch, :], lgs[i][:], AF.Exp, bias=ebias, scale=escale)

    # split output DMA: big non-cbrt channel groups ship while the serial
    # cube-root chains finish; only two tiny transfers gate the end
    nc.sync.dma_start(out_d[:, 0 : 5 * NWO], OUTT[:, 0:5, :])
    nc.sync.dma_start(out_d[:, 6 * NWO : 11 * NWO], OUTT[:, 6:11, :])
    nc.sync.dma_start(out_d[:, 5 * NWO : 6 * NWO], OUTT[:, 5, :])
    nc.sync.dma_start(out_d[:, 11 * NWO :], OUTT[:, 11, :])
    for pool in (epool, opool, impool, spool, cpool):
        pool.release()


def _get_module():
    key = CONV_MODE
    if key not in _CACHE:
        _CACHE[key] = _build_module()
    return _CACHE[key]


# ---------------------------------------------------------------- entry point
def kernel(inputs, kernels0, kernels1, dg_int):
    from concourse.bass_utils import run_bass_kernel_spmd

    in_maps = _make_in_maps(inputs, kernels0, kernels1, dg_int)
    nc = _get_module()
    res = run_bass_kernel_spmd(nc, in_maps, core_ids=list(range(8)), **RUN_KWARGS)
    global LAST
    LAST = res
    out = np.empty((B, NWO, NWO, 12), dtype=np.float32)
    for core in range(8):
        b, half = core // 2, core % 2
        H0 = half * NOUT
        out[b, H0 : H0 + NOUT] = (
            res.results[core]["out"].reshape(NOUT, 12, NWO).transpose(0, 2, 1)
        )
    return out
